# revision 1
# baseline (speedup 1.0000x reference)
"""Causal self-attention with RoPE on 8 Trainium2 NeuronCores.

Sharding: tensor-parallel over heads (16 heads -> 2 per core) for
QKV projections, RoPE and attention; AllToAll re-shards the attention
output from head-sharded to token-sharded; the output projection then
runs token-parallel (each core computes all 2048 output features for
its 512 tokens), so no all-reduce is needed.

Shapes (hardcoded): x [2, 2048, 2048], W_* [2048, 2048], 16 heads,
d_k = 128, fp32 in/out.

On-chip dataflow per core (all matmuls via PE, contraction on the
partition axis):
  - xT chunks [128d x (16kb x 256t)] stream in; per head h:
      qT/kT [128dk, 256t] = sum_kb Wq_h_kb.T @ xT_kb   (PSUM)
      RoPE applied with a stream_shuffle pair-swap + 2 muls + add
  - v in natural [token, d] layout: v = x_blk @ Wv.T
  - attention works on transposed scores: ST[j*128 keys, 512 q] =
      kT_j.T @ qT_i ; p = exp(ST + causal_mask); l += ones.T @ p;
      outT += v_j.T(@natural v) @ p   -- no max-subtraction needed
      (logits are O(1) by construction), no transposes anywhere.
  - normalize: r = 1/l broadcast via a K=1 matmul, y = outT * r
  - AllToAll: y (head-shard) -> yfull slice (token-shard)
  - out projection: outT_e = sum_db WoT_db_e.T @ yT_db  for the core's
    512 tokens.
"""

import sys
import time

for _p in ("/opt/trn_rl_repo", "/opt/pypackages"):
    if _p not in sys.path:
        sys.path.insert(0, _p)

import numpy as np

import concourse.bass as bass
import concourse.bacc as bacc
import concourse.mybir as mybir
import concourse.tile as tile
from concourse import bass_utils
from concourse.alu_op_type import AluOpType
from concourse.tile import add_dep_helper

# ---------------------------------------------------------------- config
N_CORES = 8
B, S, D = 2, 2048, 2048
H = 16
DK = D // H              # 128
HPC = H // N_CORES       # 2 heads per core
TOK = B * S              # 4096
SUB = 256                # token sub-chunk for projections
QCH = 512                # attention query chunk
JB = 128                 # attention key block
NSUB = TOK // SUB        # 16
KB = D // 128            # 16 contraction blocks
ROPE_BASE = 10000.0
MASK_NEG = -30000.0

# "f32" (exact, 4x slower matmul), "f32r" (full-rate fp32-storage
# reduced-precision matmul), "bf16"
DT_MODE = "f32r"

F32 = mybir.dt.float32


def _dt_mm():
    if DT_MODE == "bf16":
        return mybir.dt.bfloat16
    if DT_MODE == "f32r":
        return mybir.dt.float32r
    return F32


def _mm_view(ap):
    """Matmul-operand view; tiles are already in the matmul dtype."""
    return ap


def _np_dt():
    if DT_MODE == "bf16":
        import ml_dtypes
        return np.dtype(ml_dtypes.bfloat16)
    return np.dtype(np.float32)


# ---------------------------------------------------------------- build
_CACHE = {}


def _build_nc(repeat=1, no_cc=False):
    dt = _dt_mm()
    nc = bacc.Bacc("TRN2", target_bir_lowering=False, debug=False,
                   num_devices=N_CORES)

    xT = nc.dram_tensor("xT", [D, TOK], dt, kind="ExternalInput")
    wqT = nc.dram_tensor("wqT", [D, HPC * DK], dt, kind="ExternalInput")
    wkT = nc.dram_tensor("wkT", [D, HPC * DK], dt, kind="ExternalInput")
    wvT = nc.dram_tensor("wvT", [D, HPC * DK], dt, kind="ExternalInput")
    woT = nc.dram_tensor("woT", [D, D], dt, kind="ExternalInput")
    ropeC = nc.dram_tensor("ropeC", [DK, S], F32, kind="ExternalInput")
    ropeS = nc.dram_tensor("ropeS", [DK, S], F32, kind="ExternalInput")
    maskd = nc.dram_tensor("maskd", [JB, 4 * QCH], F32, kind="ExternalInput")
    outT = nc.dram_tensor("outT", [D, QCH], F32, kind="ExternalOutput")

    swap_mask = [i ^ 1 for i in range(32)]

    import contextlib
    with tile.TileContext(nc) as tc:
      for _rep in range(repeat):
        with contextlib.ExitStack() as st_outer:
            dram = st_outer.enter_context(
                tc.tile_pool(name="dram", bufs=1, space="DRAM"))
            # one AllToAll per head: head 0's collective runs while the
            # head-1 compute pass is still going, hiding half the
            # collective cost entirely
            y_a2a_h = [dram.tile([N_CORES * DK, QCH], dt, name=f"y_a2a{h}")
                       for h in range(HPC)]
            yfull_h = [dram.tile([N_CORES * DK, QCH], dt, name=f"yfull{h}")
                       for h in range(HPC)]

            const = st_outer.enter_context(tc.tile_pool(name="const", bufs=1))
            st_xq = st_outer.enter_context(contextlib.ExitStack())
            xpool = st_xq.enter_context(
                tc.tile_pool(name="xpool", bufs=3, side="right"))
            qpool = st_xq.enter_context(
                tc.tile_pool(name="qpool", bufs=4, side="right"))
            kvpool = st_xq.enter_context(
                tc.tile_pool(name="kvpool", bufs=8, side="right"))
            vpool = st_xq.enter_context(
                tc.tile_pool(name="vpool", bufs=32, side="right"))
            work = st_outer.enter_context(tc.tile_pool(name="work", bufs=2))
            ppool = st_outer.enter_context(tc.tile_pool(name="ppool", bufs=2))
            ps_proj = st_outer.enter_context(
                tc.tile_pool(name="ps_proj", bufs=2, space="PSUM"))
            ps_st = st_outer.enter_context(
                tc.tile_pool(name="ps_st", bufs=2, space="PSUM"))
            ps_out = st_outer.enter_context(
                tc.tile_pool(name="ps_out", bufs=2, space="PSUM"))
            ps_misc = st_outer.enter_context(
                tc.tile_pool(name="ps_misc", bufs=1, space="PSUM"))

            # chunk-0 xT goes first on its queues so the first
            # projection isn't stuck behind weight DMAs
            def xt_dma(xt, sc):
                KH = KB // 4
                for xh in range(4):
                    eng = nc.sync if xh % 2 == 0 else nc.gpsimd
                    eng.dma_start(
                        xt[:, xh * KH * SUB:(xh + 1) * KH * SUB]
                          .rearrange("p (kb t) -> p kb t", kb=KH),
                        xT.ap()[xh * KH * 128:(xh + 1) * KH * 128,
                                sc * SUB:(sc + 1) * SUB]
                          .rearrange("(kb p) t -> p kb t", p=128))

            xt0 = xpool.tile([128, KB * SUB], dt, tag="xt", name="xt")
            xt_dma(xt0, 0)

            # ---- persistent constants in SBUF
            # weight DMAs split in 4 groups, spread over queues, so the
            # first projection matmuls start early
            wq_sb = const.tile([128, KB * HPC * DK], dt)
            wk_sb = const.tile([128, KB * HPC * DK], dt)
            wv_sb = const.tile([128, KB * HPC * DK], dt)
            weng = {0: nc.scalar, 1: nc.sync, 2: nc.gpsimd}
            for ti, (sb_t, dr) in enumerate(
                    ((wq_sb, wqT), (wk_sb, wkT), (wv_sb, wvT))):
                ngrp = 8 if ti == 0 else 4
                GW = KB // ngrp
                for g in range(ngrp):
                    m0 = g * GW * HPC * DK
                    weng[ti].dma_start(
                        sb_t[:, m0:m0 + GW * HPC * DK]
                            .rearrange("p (kb m) -> p kb m", kb=GW),
                        dr.ap()[g * GW * 128:(g + 1) * GW * 128, :]
                          .rearrange("(kb p) m -> p kb m", p=128))
            ropeC_sb = const.tile([DK, S], F32)
            ropeS_sb = const.tile([DK, S], F32)
            maskd_sb = const.tile([JB, 4 * QCH], F32)
            nc.scalar.dma_start(ropeC_sb[:], ropeC[:])
            nc.scalar.dma_start(ropeS_sb[:], ropeS[:])
            nc.scalar.dma_start(maskd_sb[:], maskd[:])
            ones_col_f32 = const.tile([128, 1], F32)
            ones_row_f32 = const.tile([1, 128], F32)
            nc.vector.memset(ones_col_f32[:], 1.0)
            nc.vector.memset(ones_row_f32[:], 1.0)
            if dt == F32:
                ones_col, ones_row = ones_col_f32, ones_row_f32
            else:
                ones_col = const.tile([128, 1], dt)
                ones_row = const.tile([1, 128], dt)
                nc.vector.tensor_copy(ones_col[:], ones_col_f32[:])
                nc.vector.tensor_copy(ones_row[:], ones_row_f32[:])

            v_tiles = {}

            def rope_combine(ps_in, out_ap, s0, n):
                """out = ps_in * C + shuffle(ps_in) * S  (RoPE)."""
                qsh = work.tile([128, SUB], F32, tag="qsh")
                t1 = work.tile([128, SUB], F32, tag="t1")
                nc.vector.stream_shuffle(qsh[:, :n], ps_in, swap_mask)
                nc.any.tensor_tensor(
                    t1[:, :n], ps_in, ropeC_sb[:, s0:s0 + n], AluOpType.mult)
                nc.vector.tensor_tensor(
                    qsh[:, :n], qsh[:, :n], ropeS_sb[:, s0:s0 + n],
                    AluOpType.mult)
                nc.any.tensor_tensor(out_ap, t1[:, :n], qsh[:, :n],
                                     AluOpType.add)

            # ================= two passes over the sequence, one per head
            for h in range(HPC):
                qT_tile = [None]
                kT_tiles = {}
                for sc in range(NSUB):
                    b = sc // (NSUB // B)
                    s0 = (sc % (NSUB // B)) * SUB   # position within batch
                    half = sc % 2
                    i_q = (sc % (NSUB // B)) // 2   # query chunk in batch

                    if h == 0 and sc == 0:
                        xt = xt0
                    else:
                        xt = xpool.tile([128, KB * SUB], dt, tag="xt",
                                        name="xt")
                        xt_dma(xt, sc)

                    # ---- q/k projections + rope for this head
                    if half == 0:
                        qT_tile[0] = qpool.tile([128, QCH], dt, tag="qT",
                                                name="qT")
                    if (b, i_q) not in kT_tiles:
                        kT_tiles[(b, i_q)] = kvpool.tile(
                            [128, QCH], dt, tag="kT", name="kT")
                    for (w_sb, dst) in ((wq_sb, qT_tile[0]),
                                        (wk_sb, kT_tiles[(b, i_q)])):
                        psq = ps_proj.tile([128, SUB], F32, tag="proj")
                        for kb in range(KB):
                            nc.tensor.matmul(
                                psq[:],
                                _mm_view(w_sb[:, kb * HPC * DK + h * DK:
                                              kb * HPC * DK + (h + 1) * DK]),
                                _mm_view(xt[:, kb * SUB:(kb + 1) * SUB]),
                                start=(kb == 0), stop=(kb == KB - 1))
                        rope_combine(psq[:],
                                     dst[:, half * SUB:(half + 1) * SUB],
                                     s0, SUB)

                    # ---- v projection: both heads at once, pass 0 only
                    if h == 0:
                        for tb in range(SUB // 128):
                            jb_b = (sc % (NSUB // B)) * 2 + tb
                            psv = ps_proj.tile([128, HPC * DK], F32,
                                               tag="proj", name="psv")
                            for kb in range(KB):
                                nc.tensor.matmul(
                                    psv[:],
                                    _mm_view(xt[:, kb * SUB + tb * 128:
                                                kb * SUB + (tb + 1) * 128]),
                                    _mm_view(wv_sb[:, kb * HPC * DK:
                                                   (kb + 1) * HPC * DK]),
                                    start=(kb == 0), stop=(kb == KB - 1))
                            vt = vpool.tile([128, HPC * DK], dt, tag="v",
                                            name="vt")
                            nc.vector.tensor_copy(vt[:], psv[:])
                            v_tiles[(b, jb_b)] = vt

                    # ---- attention for the completed query chunk
                    if half != 1:
                        continue
                    n_j = 4 * i_q + 4
                    qT = qT_tile[0]
                    ps_o = ps_out.tile([128, QCH], F32, tag="att_out")
                    ps_l = ps_misc.tile([1, QCH], F32, tag="l")
                    for j in range(n_j):
                        jc, jr = j // 4, j % 4
                        # diagonal blocks with offset m have their first
                        # 128*m query columns fully masked: shrink all the
                        # work to the valid column range. j == 0 is always
                        # full width, so it opens the PSUM groups.
                        m = j - 4 * i_q
                        q0 = 128 * m if m > 0 else 0
                        ps_s = ps_st.tile([JB, QCH], F32, tag="st")
                        nc.tensor.matmul(
                            ps_s[:, q0:],
                            _mm_view(kT_tiles[(b, jc)][:, jr * 128:
                                                       (jr + 1) * 128]),
                            _mm_view(qT[:, q0:]),
                            start=True, stop=True)
                        p_t = ppool.tile([JB, QCH], dt, tag="p")
                        if m >= 0:                 # diagonal block: mask
                            nc.vector.tensor_tensor(
                                ps_s[:, q0:], ps_s[:, q0:],
                                maskd_sb[:, m * QCH + q0:(m + 1) * QCH],
                                AluOpType.add)
                        nc.scalar.activation(
                            p_t[:, q0:], ps_s[:, q0:],
                            mybir.ActivationFunctionType.Exp)
                        nc.tensor.matmul(
                            ps_l[:, q0:], _mm_view(ones_col[:]),
                            _mm_view(p_t[:, q0:]),
                            start=(j == 0), stop=(j == n_j - 1))
                        nc.tensor.matmul(
                            ps_o[:, q0:],
                            _mm_view(v_tiles[(b, j)][:, h * DK:(h + 1) * DK]),
                            _mm_view(p_t[:, q0:]),
                            start=(j == 0), stop=(j == n_j - 1))
                    # normalize: y = outT / l (broadcast 1/l over partitions)
                    r_sb = work.tile([1, QCH], F32, tag="r")
                    nc.vector.reciprocal(r_sb[:], ps_l[:])
                    if dt == F32:
                        r_mm = r_sb
                    else:
                        r_mm = work.tile([1, QCH], dt, tag="rmm")
                        nc.any.tensor_copy(r_mm[:], r_sb[:])
                    ps_r = ps_misc.tile([128, QCH], F32, tag="R")
                    nc.tensor.matmul(ps_r[:], ones_row[:], r_mm[:],
                                     start=True, stop=True)
                    r_bc = work.tile([128, QCH], F32, tag="rbc")
                    nc.any.tensor_copy(r_bc[:], ps_r[:])
                    y_sb = work.tile([128, QCH], dt, tag="y")
                    nc.any.tensor_tensor(y_sb[:], ps_o[:], r_bc[:],
                                         AluOpType.mult)
                    jc_glob = b * 4 + i_q
                    nc.sync.dma_start(
                        y_a2a_h[h][jc_glob * DK:(jc_glob + 1) * DK, :],
                        y_sb[:])

                # head-0 pass done: fire its AllToAll; it overlaps the
                # whole head-1 pass, and the even-half yT loads follow it
                # immediately (their SBUF tile lives in an early pool).
                # head 1's collective is emitted after the Wo pools +
                # even-half work below so those don't serialize behind it.
                if not no_cc and h == 0:
                    nc.gpsimd.collective_compute(
                        "AllToAll", AluOpType.bypass,
                        replica_groups=[list(range(N_CORES))],
                        ins=[y_a2a_h[h].opt()], outs=[yfull_h[h].opt()])

            # x/q/kv/v pools are dead now; free their SBUF for the Wo
            # weight stream + the yT tile, so Wo weights prefetch during
            # the AllToAll window
            st_xq.close()
            ytpool = st_outer.enter_context(
                tc.tile_pool(name="ytpool", bufs=1, side="right"))
            wopool = st_outer.enter_context(
                tc.tile_pool(name="wopool", bufs=12, side="right"))

            # ---- output projection for this core's 512 tokens
            # Two passes: even y-blocks (head 0, delivered by the first
            # AllToAll) accumulate while the second AllToAll is still in
            # flight, bouncing partials through DRAM; odd blocks finish
            # after it lands.
            # global y row-block db holds core (db//2)'s head (db%2)
            evpart = dram.tile([D, QCH], F32)
            HB = KB // 2
            yT_ev = ytpool.tile([128, HB * QCH], dt)
            yT_od = ytpool.tile([128, HB * QCH], dt)
            yT_par = [yT_ev, yT_od]
            ysrc_h = y_a2a_h if no_cc else yfull_h

            ev_anchor = []

            def yt_dmas(par):
                # odd half is gated on the second AllToAll: spread it over
                # the fast HWDGE queues, but explicitly ordered after the
                # even pass's last store so the scheduler can't hoist it
                # into head-of-line-blocking position
                engs = ((nc.sync, nc.scalar) if par == 0
                        else (nc.sync, nc.scalar, nc.gpsimd))
                for c2 in range(HB):
                    di = engs[c2 % len(engs)].dma_start(
                        yT_par[par][:, c2 * QCH:(c2 + 1) * QCH],
                        ysrc_h[par][c2 * 128:(c2 + 1) * 128, :])
                    if par == 1:
                        for a in ev_anchor:
                            add_dep_helper(
                                di.ins, a.ins, sync=True,
                                reason="odd yT after even-pass stores")

            wo_engs = [nc.sync, nc.gpsimd, nc.scalar]

            def wo_pass(par):            # 0: even db (head 0), 1: odd
                for eb in range(KB):
                    wo_eb = wopool.tile([128, HB * 128], dt, tag="wo",
                                        name="wo_eb")
                    (wo_engs[eb % 3] if par == 0 else
                     (nc.sync if eb % 2 == 0 else nc.scalar)).dma_start(
                        wo_eb.rearrange("p (db e) -> p db e", db=HB),
                        woT.ap()[:, eb * 128:(eb + 1) * 128]
                           .rearrange("(db p) e -> p db e", p=128)
                           [:, par::2, :])
                    ps_w = ps_st.tile([JB, QCH], F32, tag="st", name="ps_w")
                    for dl in range(HB):
                        nc.tensor.matmul(
                            ps_w[:],
                            _mm_view(wo_eb[:, dl * 128:(dl + 1) * 128]),
                            _mm_view(yT_par[par][:, dl * QCH:(dl + 1) * QCH]),
                            start=(dl == 0), stop=(dl == HB - 1))
                    if par == 0:
                        ev_sb = work.tile([128, QCH], F32, tag="y")
                        nc.vector.tensor_copy(ev_sb[:], ps_w[:])
                        ev_st = nc.sync.dma_start(
                            evpart[eb * 128:(eb + 1) * 128, :], ev_sb[:])
                        ev_anchor[:] = [ev_st]
                    else:
                        ev_in = work.tile([128, QCH], F32, tag="rbc")
                        nc.scalar.dma_start(
                            ev_in[:], evpart[eb * 128:(eb + 1) * 128, :])
                        o_sb = work.tile([128, QCH], F32, tag="y")
                        nc.vector.tensor_tensor(
                            o_sb[:], ps_w[:], ev_in[:], AluOpType.add)
                        nc.sync.dma_start(
                            outT[eb * 128:(eb + 1) * 128, :], o_sb[:])

            yt_dmas(0)
            wo_pass(0)
            if not no_cc:
                nc.gpsimd.collective_compute(
                    "AllToAll", AluOpType.bypass,
                    replica_groups=[list(range(N_CORES))],
                    ins=[y_a2a_h[1].opt()], outs=[yfull_h[1].opt()])
            yt_dmas(1)
            wo_pass(1)

    nc.finalize()
    return nc


# ---------------------------------------------------------------- host
def _host_inputs(x, W_q, W_k, W_v, W_o):
    np_dt = _np_dt()
    xT = np.ascontiguousarray(
        x.reshape(TOK, D).T).astype(np_dt)                     # [D, TOK]
    woT = np.ascontiguousarray(W_o.T).astype(np_dt)            # [d, e]

    # RoPE tables, expanded to [DK, S] with interleaved pairs; the sign
    # table carries -sin on even rows, +sin on odd rows.
    i = np.arange(0, DK, 2, dtype=np.float32)
    theta = 1.0 / (ROPE_BASE ** (i / DK))                      # [64]
    pos = np.arange(S, dtype=np.float32)
    freqs = pos[:, None] * theta[None, :]                      # [S, 64]
    cos_t, sin_t = np.cos(freqs), np.sin(freqs)
    ropeC = np.empty((DK, S), np.float32)
    ropeS = np.empty((DK, S), np.float32)
    ropeC[0::2] = cos_t.T
    ropeC[1::2] = cos_t.T
    ropeS[0::2] = -sin_t.T
    ropeS[1::2] = sin_t.T

    # diagonal causal masks: block m (of the 4 key blocks overlapping a
    # 512-query chunk) keeps kk <= qq - 128*m
    kk = np.arange(JB)[:, None]
    qq = np.arange(QCH)[None, :]
    maskd = np.concatenate(
        [np.where(kk <= qq - 128 * m, 0.0, MASK_NEG).astype(np.float32)
         for m in range(4)], axis=1)                           # [128, 4*512]

    scale = 1.0 / np.sqrt(np.float32(DK))
    in_maps = []
    for c in range(N_CORES):
        rows = slice(c * HPC * DK, (c + 1) * HPC * DK)
        in_maps.append({
            "xT": xT,
            "wqT": np.ascontiguousarray((W_q[rows] * scale).T).astype(np_dt),
            "wkT": np.ascontiguousarray(W_k[rows].T).astype(np_dt),
            "wvT": np.ascontiguousarray(W_v[rows].T).astype(np_dt),
            "woT": woT,
            "ropeC": ropeC,
            "ropeS": ropeS,
            "maskd": maskd,
        })
    return in_maps


def kernel(x, W_q, W_k, W_v, W_o):
    x = np.asarray(x, dtype=np.float32)
    W_q = np.asarray(W_q, dtype=np.float32)
    W_k = np.asarray(W_k, dtype=np.float32)
    W_v = np.asarray(W_v, dtype=np.float32)
    W_o = np.asarray(W_o, dtype=np.float32)

    if "nc" not in _CACHE:
        _CACHE["nc"] = _build_nc()
    nc = _CACHE["nc"]

    in_maps = _host_inputs(x, W_q, W_k, W_v, W_o)
    res = bass_utils.run_bass_kernel_spmd(
        nc, in_maps, core_ids=list(range(N_CORES)))

    # outT per core: [D, 512] fp32 for tokens [c*512:(c+1)*512]
    out_T = np.concatenate([res.results[c]["outT"] for c in range(N_CORES)],
                           axis=1)                             # [D, TOK]
    return np.ascontiguousarray(out_T.T).reshape(B, S, D).astype(np.float32)



# revision 34
# speedup vs baseline: 1.3309x; 1.3309x over previous
"""Causal self-attention with RoPE on 8 Trainium2 NeuronCores.

Sharding: tensor-parallel over heads (16 heads -> 2 per core) for the
QKV projections, RoPE and attention.  The attention output is
re-sharded token-wise with one small AllToAll per 512-token chunk
(bf16 payload), so each core ends up with the full 2048-dim y vector
for 64 tokens of every chunk; the output projection then runs
token-parallel with no all-reduce.

Single pass over the sequence: both heads' q/k/v are produced from one
streaming of x (the baseline used one pass per head and loaded x
twice).

Key engine-placement choices (cost-model driven):
  - softmax denominator l = sum_k p is NOT computed with PE matmuls
    (those cost as much as the score matmuls); instead p-blocks are
    summed with a shallow tree of tensor_tensor adds spread across
    DVE/Act/Pool and one gpsimd.partition_all_reduce, whose output is
    already broadcast across partitions (also kills the r-broadcast
    matmul of the baseline).
  - y, v and W_o are bf16 (halves the collective payload and the Wo
    weight traffic; bf16 matmuls run at full PE rate).
  - W_o streams through SBUF per token-pass; the last pass reuses the
    still-resident tiles of the previous pass for its first blocks.

Shapes (hardcoded): x [2, 2048, 2048], W_* [2048, 2048], 16 heads,
d_k = 128, fp32 in/out.
"""

import sys

for _p in ("/opt/trn_rl_repo", "/opt/pypackages"):
    if _p not in sys.path:
        sys.path.insert(0, _p)

import numpy as np

import concourse.bass as bass
import concourse.bacc as bacc
import concourse.mybir as mybir
import concourse.tile as tile
from concourse import bass_utils
from concourse import bass_isa
from concourse.alu_op_type import AluOpType
from concourse.tile import add_dep_helper

# ---------------------------------------------------------------- config
N_CORES = 8
B, S, D = 2, 2048, 2048
H = 16
DK = D // H              # 128
HPC = H // N_CORES       # 2 heads per core
TOK = B * S              # 4096
SUB = 256                # token sub-chunk for projections
QCH = 512                # attention query chunk
JB = 128                 # attention key block
NSUB = TOK // SUB        # 16
NCH = TOK // QCH         # 8 query chunks
KB = D // 128            # 16 contraction blocks
TPC = QCH // N_CORES     # 64 tokens per (chunk, core) after AllToAll
ROPE_BASE = 10000.0
MASK_NEG = -30000.0

F32 = mybir.dt.float32
F32R = mybir.dt.float32r
BF16 = mybir.dt.bfloat16

_CACHE = {}


def _build_nc():
    dt = F32R
    nc = bacc.Bacc("TRN2", target_bir_lowering=False, debug=False,
                   num_devices=N_CORES)

    xT = nc.dram_tensor("xT", [D, TOK], BF16, kind="ExternalInput")
    wqT = nc.dram_tensor("wqT", [D, HPC * DK], BF16, kind="ExternalInput")
    wkT = nc.dram_tensor("wkT", [D, HPC * DK], BF16, kind="ExternalInput")
    wvT = nc.dram_tensor("wvT", [D, HPC * DK], BF16, kind="ExternalInput")
    # W_o.T in bf16, tiled (eb, p, dl, e'): row = eb*2048 + p*16 + dl
    woT = nc.dram_tensor("woT", [KB * D, DK], BF16, kind="ExternalInput")
    ropeC = nc.dram_tensor("ropeC", [DK, S], F32, kind="ExternalInput")
    ropeS = nc.dram_tensor("ropeS", [DK, S], F32, kind="ExternalInput")
    maskd = nc.dram_tensor("maskd", [JB, 4 * QCH], BF16, kind="ExternalInput")
    # out columns ordered (jc 0..7, t 0..63): global token 512*jc + 64*c + t
    outT = nc.dram_tensor("outT", [D, QCH], BF16, kind="ExternalOutput")

    swap_mask = [i ^ 1 for i in range(32)]

    import contextlib
    with tile.TileContext(nc) as tc:
        with contextlib.ExitStack() as st:
            dram = st.enter_context(
                tc.tile_pool(name="dram", bufs=1, space="DRAM"))
            a2a_in = [dram.tile([N_CORES * HPC * DK, TPC], BF16,
                                name=f"a2ain{j}") for j in range(NCH)]
            a2a_out = [dram.tile([N_CORES * HPC * DK, TPC], BF16,
                                 name=f"a2aout{j}") for j in range(NCH)]

            const = st.enter_context(tc.tile_pool(name="const", bufs=1))
            xpool = st.enter_context(
                tc.tile_pool(name="xpool", bufs=2, side="right"))
            qpool = st.enter_context(
                tc.tile_pool(name="qpool", bufs=2, side="right"))
            kvpool = st.enter_context(
                tc.tile_pool(name="kvpool", bufs=8, side="right"))
            vpool = st.enter_context(
                tc.tile_pool(name="vpool", bufs=17, side="right"))
            ppool = st.enter_context(tc.tile_pool(name="ppool", bufs=7))
            lpool = st.enter_context(tc.tile_pool(name="lpool", bufs=2))
            work = st.enter_context(tc.tile_pool(name="work", bufs=2))
            ypool = st.enter_context(tc.tile_pool(name="ypool", bufs=3))
            ytpool = st.enter_context(tc.tile_pool(name="ytpool", bufs=1))
            stpool = st.enter_context(tc.tile_pool(name="stpool", bufs=1))
            wopool = st.enter_context(
                tc.tile_pool(name="wopool", bufs=14, side="right"))
            ps_proj = st.enter_context(
                tc.tile_pool(name="ps_proj", bufs=3, space="PSUM"))
            ps_st = st.enter_context(
                tc.tile_pool(name="ps_st", bufs=3, space="PSUM"))
            ps_out = st.enter_context(
                tc.tile_pool(name="ps_out", bufs=2, space="PSUM"))

            # chunk-0 xT goes first on its queues so the first projection
            # isn't stuck behind weight DMAs
            def xt_dma(xt, sc):
                KH = KB // 4
                di = None
                for xh in range(4):
                    di = nc.sync.dma_start(
                        xt[:, xh * KH * SUB:(xh + 1) * KH * SUB]
                          .rearrange("p (kb t) -> p kb t", kb=KH),
                        xT.ap()[xh * KH * 128:(xh + 1) * KH * 128,
                                sc * SUB:(sc + 1) * SUB]
                          .rearrange("(kb p) t -> p kb t", p=128))
                return di

            xt_anchor = {}
            store_anchor = {}
            xt0 = xpool.tile([128, KB * SUB], BF16, tag="xt", name="xt")
            xt_anchor[0] = xt_dma(xt0, 0)

            # ---- persistent constants in SBUF
            wq_sb = const.tile([128, KB * HPC * DK], BF16)
            wk_sb = const.tile([128, KB * HPC * DK], BF16)
            wv_sb = const.tile([128, KB * HPC * DK], BF16)
            weng = {0: nc.scalar, 1: nc.scalar, 2: nc.gpsimd}
            for ti, (sb_t, dr) in enumerate(
                    ((wq_sb, wqT), (wk_sb, wkT), (wv_sb, wvT))):
                ngrp = 8 if ti == 0 else 4
                GW = KB // ngrp
                for g in range(ngrp):
                    m0 = g * GW * HPC * DK
                    weng[ti].dma_start(
                        sb_t[:, m0:m0 + GW * HPC * DK]
                            .rearrange("p (kb m) -> p kb m", kb=GW),
                        dr.ap()[g * GW * 128:(g + 1) * GW * 128, :]
                          .rearrange("(kb p) m -> p kb m", p=128))
            ropeC_sb = const.tile([DK, S], F32)
            ropeS_sb = const.tile([DK, S], F32)
            maskd_sb = const.tile([JB, 4 * QCH], BF16)
            nc.scalar.dma_start(ropeC_sb[:, :SUB], ropeC[:, :SUB])
            nc.scalar.dma_start(ropeS_sb[:, :SUB], ropeS[:, :SUB])
            nc.scalar.dma_start(ropeC_sb[:, SUB:], ropeC[:, SUB:])
            nc.scalar.dma_start(ropeS_sb[:, SUB:], ropeS[:, SUB:])
            nc.scalar.dma_start(maskd_sb[:], maskd[:])

            def rope_combine(ps_in, out_ap, s0, n):
                """out = ps_in * C + shuffle(ps_in) * S  (RoPE)."""
                qsh = work.tile([128, SUB], F32, tag="qsh", name="qsh")
                t1 = work.tile([128, SUB], F32, tag="t1", name="t1")
                nc.vector.stream_shuffle(qsh[:, :n], ps_in, swap_mask)
                nc.any.tensor_tensor(
                    t1[:, :n], ps_in, ropeC_sb[:, s0:s0 + n], AluOpType.mult)
                nc.gpsimd.tensor_tensor(
                    qsh[:, :n], qsh[:, :n], ropeS_sb[:, s0:s0 + n],
                    AluOpType.mult)
                nc.any.tensor_tensor(out_ap, t1[:, :n], qsh[:, :n],
                                     AluOpType.add)

            chain_engs = [nc.vector, nc.gpsimd]

            # Wo token-passes -------------------------------------------------
            # pass 0: chunks 0-3 (cols 0:256), interleaved into the second
            # half of the main loop; pass 1: chunks 4-6 (cols 256:448);
            # pass 2: chunk 7 (cols 448:512), reusing resident wo tiles.
            wo_engs = [nc.sync, nc.scalar]

            def load_woeb(eb, ei, anchor=None):
                wo_eb = wopool.tile([128, KB * DK], BF16, tag="wo",
                                    name="wo_eb")
                di = wo_engs[ei % 2].dma_start(
                    wo_eb[:],
                    woT.ap()[eb * D:(eb + 1) * D, :]
                       .rearrange("(p dl) e -> p (dl e)", p=128))
                if anchor is not None:
                    add_dep_helper(di.ins, anchor.ins, sync=True,
                                   reason="wo load after main loop")
                return wo_eb

            def load_yt(yt, chunks, ci0=0, anchors=None, eng=None):
                for ci, jc in enumerate(chunks):
                    di = (eng or nc.scalar).dma_start(
                        yt.rearrange("p (db c) -> p db c", db=KB)
                          [:, :, (ci0 + ci) * TPC:(ci0 + ci + 1) * TPC],
                        a2a_out[jc].rearrange("(db p) t -> p db t",
                                                   p=128))
                    if anchors is not None:
                        add_dep_helper(di.ins, anchors[ci].ins, sync=True,
                                       reason="yt load after its A2A landed")

            def wo_eb_mm(eb, yt, ncols, stage, wo_eb=None, ei=0):
                if wo_eb is None:
                    wo_eb = load_woeb(eb, ei)
                ps_w = ps_proj.tile([128, SUB], F32, tag="proj", name="ps_w")
                for dl in range(KB):
                    nc.tensor.matmul(
                        ps_w[:, :ncols],
                        wo_eb[:, dl * DK:(dl + 1) * DK],
                        yt[:, dl * ncols:(dl + 1) * ncols],
                        start=(dl == 0), stop=(dl == KB - 1))
                nc.any.tensor_copy(
                    stage[:, eb * ncols:(eb + 1) * ncols], ps_w[:, :ncols])

            def store_stage(stage, ncols, col0):
                nc.sync.dma_start(
                    outT.ap().rearrange("(eb p) c -> p eb c", p=128)
                        [:, :, col0:col0 + ncols],
                    stage.rearrange("p (eb c) -> p eb c", eb=KB))

            # ---------------- main pass over the sequence -------------------
            qT_tiles = {}
            kT_tiles = {}
            v_tiles = {}
            yt_p0 = [None]

            for sc in range(NSUB):
                b = sc // (NSUB // B)
                half = sc % 2
                iq = (sc % (NSUB // B)) // 2
                jc_glob = b * 4 + iq
                s0 = (sc % (NSUB // B)) * SUB   # position within batch

                if sc == 0:
                    xt = xt0
                else:
                    xt = xpool.tile([128, KB * SUB], BF16, tag="xt", name="xt")
                    xt_anchor[sc] = xt_dma(xt, sc)

                # ---- q/k projections + rope, both heads
                if half == 0:
                    for h in range(HPC):
                        qT_tiles[h] = qpool.tile([128, QCH], dt,
                                                 tag=f"qT{h}", name="qT")
                for h in range(HPC):
                    if (b, h, iq) not in kT_tiles:
                        kT_tiles[(b, h, iq)] = kvpool.tile(
                            [128, QCH], dt, tag="kT", name="kT")
                    for (w_sb, dst) in ((wq_sb, qT_tiles[h]),
                                        (wk_sb, kT_tiles[(b, h, iq)])):
                        psq = ps_proj.tile([128, SUB], F32, tag="proj", name="proj")
                        for kb in range(KB):
                            nc.tensor.matmul(
                                psq[:],
                                w_sb[:, kb * HPC * DK + h * DK:
                                     kb * HPC * DK + (h + 1) * DK],
                                xt[:, kb * SUB:(kb + 1) * SUB],
                                start=(kb == 0), stop=(kb == KB - 1))
                        rope_combine(psq[:],
                                     dst[:, half * SUB:(half + 1) * SUB],
                                     s0, SUB)

                # ---- v projection, both heads, bf16 storage
                for tb in range(SUB // 128):
                    jb_b = (sc % (NSUB // B)) * 2 + tb
                    psv = ps_proj.tile([128, HPC * DK], F32, tag="proj",
                                       name="psv")
                    for kb in range(KB):
                        nc.tensor.matmul(
                            psv[:],
                            xt[:, kb * SUB + tb * 128:
                               kb * SUB + (tb + 1) * 128],
                            wv_sb[:, kb * HPC * DK:(kb + 1) * HPC * DK],
                            start=(kb == 0), stop=(kb == KB - 1))
                    vt = vpool.tile([128, HPC * DK], BF16, tag="v", name="vt")
                    nc.any.tensor_copy(vt[:], psv[:])
                    v_tiles[(b, jb_b)] = vt

                # ---- attention for the completed query chunk
                if half == 1:
                    n_j = 4 * iq + 4
                    for h in range(HPC):
                        qT = qT_tiles[h]
                        ps_o = ps_out.tile([128, QCH], F32, tag="att_out", name="att_out")
                        p_tiles = {}

                        def emit_block(j):
                            jck, jr = j // 4, j % 4
                            m = j - 4 * iq
                            # diagonal block m: columns [0, 128m) are fully
                            # masked -> skip them and zero-fill p instead
                            q0 = 128 * m if m > 0 else 0
                            ps_s = ps_st.tile([JB, QCH], F32, tag="st", name="st")
                            nc.tensor.matmul(
                                ps_s[:, q0:],
                                kT_tiles[(b, h, jck)][:, jr * 128:
                                                      (jr + 1) * 128],
                                qT[:, q0:], start=True, stop=True)
                            if m >= 0:           # diagonal block: mask
                                nc.vector.tensor_tensor(
                                    ps_s[:, q0:], ps_s[:, q0:],
                                    maskd_sb[:, m * QCH + q0:(m + 1) * QCH],
                                    AluOpType.add)
                            p_t = ppool.tile([JB, QCH], BF16, tag="p", name="p")
                            if q0 > 0:
                                nc.gpsimd.memset(p_t[:, :q0], 0.0)
                            nc.scalar.activation(
                                p_t[:, q0:], ps_s[:, q0:],
                                mybir.ActivationFunctionType.Exp)
                            p_tiles[j] = p_t

                        # scores/exp run two blocks ahead of the AV matmuls
                        # so PE never waits on the Act engine
                        emit_block(0)
                        if n_j > 1:
                            emit_block(1)
                        # l = sum_k p: two sequential accumulator chains
                        # (even/odd j) spread across DVE/Pool/Act
                        acc = [None, None]
                        n_add = 0
                        cengs = (chain_engs if jc_glob < NCH - 1
                                 else [nc.vector])
                        for j in range(n_j):
                            if j + 2 < n_j:
                                emit_block(j + 2)
                            p_t = p_tiles[j]
                            nc.tensor.matmul(
                                ps_o[:], v_tiles[(b, j)][:, h * DK:
                                                         (h + 1) * DK],
                                p_t[:],
                                start=(j == 0), stop=(j == n_j - 1))
                            c = j % 2
                            if j >= 2:
                                eng = cengs[0]
                                n_add += 1
                                if acc[c] is None:
                                    a_t = lpool.tile([128, QCH], BF16,
                                                     tag="acc", name="acc")
                                    eng.tensor_tensor(
                                        a_t[:], p_tiles[c][:], p_t[:],
                                        AluOpType.add)
                                    acc[c] = a_t
                                else:
                                    eng.tensor_tensor(
                                        acc[c][:], acc[c][:], p_t[:],
                                        AluOpType.add)
                        if acc[0] is None:       # n_j == 4 has j = 0..3
                            acc = [p_tiles[0], p_tiles[1]]
                        p_acc = lpool.tile([128, QCH], BF16, tag="acc2", name="acc2")
                        cengs[0].tensor_tensor(
                            p_acc[:], acc[0][:], acc[1][:], AluOpType.add)
                        # gpsimd all-reduce output is broadcast across
                        # partitions -> no r-broadcast matmul needed
                        l_bc = lpool.tile([128, QCH], F32, tag="lbc", name="lbc")
                        nc.gpsimd.partition_all_reduce(
                            l_bc[:], p_acc[:], channels=128,
                            reduce_op=bass_isa.ReduceOp.add)
                        r_bc = lpool.tile([128, QCH], F32, tag="rbc", name="rbc")
                        nc.vector.reciprocal(r_bc[:], l_bc[:])
                        y_sb = ypool.tile([128, QCH], BF16, tag="y", name="y")
                        nc.any.tensor_tensor(y_sb[:], ps_o[:], r_bc[:],
                                             AluOpType.mult)
                        # scatter into the AllToAll source layout:
                        # dst row m*256 + h*128 + p, col t <- src[p, m*64+t]
                        di = nc.gpsimd.dma_start(
                            a2a_in[jc_glob]
                            .rearrange("(m hp) t -> hp m t", m=N_CORES)
                            [h * 128:(h + 1) * 128],
                            y_sb.rearrange("p (m t) -> p m t", m=N_CORES))
                        if h == HPC - 1:
                            store_anchor[jc_glob] = di
                    nc.gpsimd.collective_compute(
                        "AllToAll", AluOpType.bypass,
                        replica_groups=[list(range(N_CORES))],
                        ins=[a2a_in[jc_glob].opt()],
                        outs=[a2a_out[jc_glob].opt()])


            # ---- Wo pass A: chunks 0-3 (cols 0:256).  Runs entirely in
            # the shadow of the last AllToAll: those chunks' y landed long
            # ago.  Every staging DMA is anchored so the dataflow
            # scheduler cannot hoist its wait into the main loop.
            yt_p0[0] = ytpool.tile([128, KB * 4 * TPC], BF16,
                                   tag="yt0", name="yt0")
            load_yt(yt_p0[0], range(4),
                    anchors=[store_anchor[c + 1] for c in range(4)])
            yt_p1 = ytpool.tile([128, KB * 3 * TPC], BF16, tag="yt1",
                                name="yt1")
            load_yt(yt_p1, range(4, 7),
                    anchors=[store_anchor[5], store_anchor[6],
                             store_anchor[7]])
            stage_a = stpool.tile([128, KB * 4 * TPC], BF16, tag="sta",
                                  name="sta")
            stage_b1 = stpool.tile([128, KB * 3 * TPC], BF16, tag="stb1",
                                   name="stb1")
            stage_b2 = stpool.tile([128, KB * TPC], BF16, tag="stb2",
                                   name="stb2")
            p0_pref = {eb: load_woeb(eb, eb, anchor=xt_anchor[11])
                       for eb in range(2)}
            for eb in range(KB):
                if eb + 2 < KB:       # stay 2 loads ahead of the matmuls
                    p0_pref[eb + 2] = load_woeb(eb + 2, eb,
                                                anchor=xt_anchor[11])
                wo_eb_mm(eb, yt_p0[0], 4 * TPC, stage_a,
                         wo_eb=p0_pref.pop(eb), ei=eb)

            store_stage(stage_a, 4 * TPC, 0)

            # ---- Wo pass B1: chunks 4-6 (cols 256:448) -- none of this
            # waits on the final AllToAll, so it fills the A2A-7 shadow
            # right after pass A.
            b1_tiles = {eb: load_woeb(eb, eb, anchor=xt_anchor[13])
                        for eb in range(2)}
            for eb in range(KB):
                if eb + 2 < KB:
                    b1_tiles[eb + 2] = load_woeb(eb + 2, eb,
                                                 anchor=xt_anchor[13])
                wo_eb_mm(eb, yt_p1, 3 * TPC, stage_b1,
                         wo_eb=b1_tiles[eb], ei=eb)

            store_stage(stage_b1, 3 * TPC, 4 * TPC)

            # ---- Wo pass B2: chunk 7 (cols 448:512), gated on the final
            # AllToAll.  The last wopool-1 tiles of B1 are still resident:
            # run those first, re-stream the rest.
            yt_p2 = ytpool.tile([128, KB * TPC], BF16, tag="yt2",
                                name="yt2")
            load_yt(yt_p2, range(7, 8),
                    anchors=[store_anchor[7]], eng=nc.gpsimd)
            for eb in range(KB - 13, KB):
                wo_eb_mm(eb, yt_p2, TPC, stage_b2, wo_eb=b1_tiles[eb])
            for ei, eb in enumerate(range(KB - 13)):
                wo_eb_mm(eb, yt_p2, TPC, stage_b2, ei=ei)
            store_stage(stage_b2, TPC, 7 * TPC)

    nc.finalize()
    return nc


# ---------------------------------------------------------------- host
def _host_inputs(x, W_q, W_k, W_v, W_o):
    import ml_dtypes
    bf = np.dtype(ml_dtypes.bfloat16)
    xT = np.ascontiguousarray(
        x.reshape(TOK, D).T).astype(bf)                        # [D, TOK]

    # W_o.T tiled (eb, p, dl, e'): row eb*2048 + p*16 + dl, col e'
    woT = np.ascontiguousarray(
        W_o.T.reshape(KB, 128, KB, DK).transpose(2, 1, 0, 3)
        .reshape(KB * D, DK)).astype(bf)

    # RoPE tables, expanded to [DK, S] with interleaved pairs; the sign
    # table carries -sin on even rows, +sin on odd rows.
    i = np.arange(0, DK, 2, dtype=np.float32)
    theta = 1.0 / (ROPE_BASE ** (i / DK))                      # [64]
    pos = np.arange(S, dtype=np.float32)
    freqs = pos[:, None] * theta[None, :]                      # [S, 64]
    cos_t, sin_t = np.cos(freqs), np.sin(freqs)
    ropeC = np.empty((DK, S), np.float32)
    ropeS = np.empty((DK, S), np.float32)
    ropeC[0::2] = cos_t.T
    ropeC[1::2] = cos_t.T
    ropeS[0::2] = -sin_t.T
    ropeS[1::2] = sin_t.T

    # diagonal causal masks: block m (of the 4 key blocks overlapping a
    # 512-query chunk) keeps kk <= qq - 128*m
    kk = np.arange(JB)[:, None]
    qq = np.arange(QCH)[None, :]
    maskd = np.concatenate(
        [np.where(kk <= qq - 128 * m, 0.0, MASK_NEG).astype(np.float32)
         for m in range(4)], axis=1).astype(bf)                # [128, 4*512]

    scale = 1.0 / np.sqrt(np.float32(DK))
    in_maps = []
    for c in range(N_CORES):
        rows = slice(c * HPC * DK, (c + 1) * HPC * DK)
        in_maps.append({
            "xT": xT,
            "wqT": np.ascontiguousarray(
                (W_q[rows] * scale).T).astype(bf),
            "wkT": np.ascontiguousarray(W_k[rows].T).astype(bf),
            "wvT": np.ascontiguousarray(W_v[rows].T).astype(bf),
            "woT": woT,
            "ropeC": ropeC,
            "ropeS": ropeS,
            "maskd": maskd,
        })
    return in_maps


def kernel(x, W_q, W_k, W_v, W_o):
    x = np.asarray(x, dtype=np.float32)
    W_q = np.asarray(W_q, dtype=np.float32)
    W_k = np.asarray(W_k, dtype=np.float32)
    W_v = np.asarray(W_v, dtype=np.float32)
    W_o = np.asarray(W_o, dtype=np.float32)

    if "nc" not in _CACHE:
        _CACHE["nc"] = _build_nc()
    nc = _CACHE["nc"]

    in_maps = _host_inputs(x, W_q, W_k, W_v, W_o)
    res = bass_utils.run_bass_kernel_spmd(
        nc, in_maps, core_ids=list(range(N_CORES)))

    # outT per core: [D, 512] f32; col 64*jc + t -> token 512*jc + 64*c + t
    out_T = np.empty((D, TOK), np.float32)
    for c in range(N_CORES):
        cols = (QCH * np.arange(NCH)[:, None] + TPC * c
                + np.arange(TPC)[None, :]).ravel()
        out_T[:, cols] = res.results[c]["outT"].astype(np.float32)
    return np.ascontiguousarray(out_T.T).reshape(B, S, D).astype(np.float32)


# revision 40
# speedup vs baseline: 1.3859x; 1.0413x over previous
"""Causal self-attention with RoPE on 8 Trainium2 NeuronCores.

Sharding: tensor-parallel over heads (16 heads -> 2 per core) for the
QKV projections, RoPE and attention.  The attention output is
re-sharded token-wise with one small AllToAll per 512-token chunk
(bf16 payload), so each core ends up with the full 2048-dim y vector
for 64 tokens of every chunk; the output projection then runs
token-parallel with no all-reduce.

Single pass over the sequence: both heads' q/k/v are produced from one
streaming of x (the baseline used one pass per head and loaded x
twice).

Key engine-placement choices (cost-model driven):
  - softmax denominator l = sum_k p is NOT computed with PE matmuls
    (those cost as much as the score matmuls); instead p-blocks are
    summed with a shallow tree of tensor_tensor adds spread across
    DVE/Act/Pool and one gpsimd.partition_all_reduce, whose output is
    already broadcast across partitions (also kills the r-broadcast
    matmul of the baseline).
  - y, v and W_o are bf16 (halves the collective payload and the Wo
    weight traffic; bf16 matmuls run at full PE rate).
  - W_o streams through SBUF per token-pass; the last pass reuses the
    still-resident tiles of the previous pass for its first blocks.

Shapes (hardcoded): x [2, 2048, 2048], W_* [2048, 2048], 16 heads,
d_k = 128, fp32 in/out.
"""

import sys

for _p in ("/opt/trn_rl_repo", "/opt/pypackages"):
    if _p not in sys.path:
        sys.path.insert(0, _p)

import numpy as np

import concourse.bass as bass
import concourse.bacc as bacc
import concourse.mybir as mybir
import concourse.tile as tile
from concourse import bass_utils
from concourse import bass_isa
from concourse.alu_op_type import AluOpType
from concourse.tile import add_dep_helper

# ---------------------------------------------------------------- config
N_CORES = 8
B, S, D = 2, 2048, 2048
H = 16
DK = D // H              # 128
HPC = H // N_CORES       # 2 heads per core
TOK = B * S              # 4096
SUB = 256                # token sub-chunk for projections
QCH = 512                # attention query chunk
JB = 128                 # attention key block
NSUB = TOK // SUB        # 16
NCH = TOK // QCH         # 8 query chunks
KB = D // 128            # 16 contraction blocks
TPC = QCH // N_CORES     # 64 tokens per (chunk, core) after AllToAll
ROPE_BASE = 10000.0
MASK_NEG = -30000.0

F32 = mybir.dt.float32
F32R = mybir.dt.float32r
BF16 = mybir.dt.bfloat16

_CACHE = {}


def _build_nc():
    dt = F32R
    nc = bacc.Bacc("TRN2", target_bir_lowering=False, debug=False,
                   num_devices=N_CORES)

    xT = nc.dram_tensor("xT", [D, TOK], BF16, kind="ExternalInput")
    wqT = nc.dram_tensor("wqT", [D, HPC * DK], BF16, kind="ExternalInput")
    wkT = nc.dram_tensor("wkT", [D, HPC * DK], BF16, kind="ExternalInput")
    wvT = nc.dram_tensor("wvT", [D, HPC * DK], BF16, kind="ExternalInput")
    # W_o.T in bf16, tiled (eb, p, dl, e'): row = eb*2048 + p*16 + dl
    woT = nc.dram_tensor("woT", [KB * D, DK], BF16, kind="ExternalInput")
    ropeC = nc.dram_tensor("ropeC", [DK, S], F32, kind="ExternalInput")
    ropeS = nc.dram_tensor("ropeS", [DK, S], F32, kind="ExternalInput")
    maskd = nc.dram_tensor("maskd", [JB, 4 * QCH], BF16, kind="ExternalInput")
    # out columns ordered (jc 0..7, t 0..63): global token 512*jc + 64*c + t
    outT = nc.dram_tensor("outT", [D, QCH], BF16, kind="ExternalOutput")

    swap_mask = [i ^ 1 for i in range(32)]

    import contextlib
    with tile.TileContext(nc) as tc:
        with contextlib.ExitStack() as st:
            dram = st.enter_context(
                tc.tile_pool(name="dram", bufs=1, space="DRAM"))
            a2a_in = [dram.tile([N_CORES * HPC * DK, TPC], BF16,
                                name=f"a2ain{j}") for j in range(NCH)]
            a2a_out = [dram.tile([N_CORES * HPC * DK, TPC], BF16,
                                 name=f"a2aout{j}") for j in range(NCH)]

            const = st.enter_context(tc.tile_pool(name="const", bufs=1))
            xpool = st.enter_context(
                tc.tile_pool(name="xpool", bufs=3, side="right"))
            qpool = st.enter_context(
                tc.tile_pool(name="qpool", bufs=2, side="right"))
            kvpool = st.enter_context(
                tc.tile_pool(name="kvpool", bufs=8, side="right"))
            vpool = st.enter_context(
                tc.tile_pool(name="vpool", bufs=17, side="right"))
            ppool = st.enter_context(tc.tile_pool(name="ppool", bufs=7))
            lpool = st.enter_context(tc.tile_pool(name="lpool", bufs=2))
            work = st.enter_context(tc.tile_pool(name="work", bufs=2))
            ypool = st.enter_context(tc.tile_pool(name="ypool", bufs=3))
            ytpool = st.enter_context(tc.tile_pool(name="ytpool", bufs=1))
            stpool = st.enter_context(tc.tile_pool(name="stpool", bufs=1))
            wopool = st.enter_context(
                tc.tile_pool(name="wopool", bufs=14, side="right"))
            ps_proj = st.enter_context(
                tc.tile_pool(name="ps_proj", bufs=3, space="PSUM"))
            ps_st = st.enter_context(
                tc.tile_pool(name="ps_st", bufs=3, space="PSUM"))
            ps_out = st.enter_context(
                tc.tile_pool(name="ps_out", bufs=2, space="PSUM"))

            # chunk-0 xT goes first on its queues so the first projection
            # isn't stuck behind weight DMAs
            def xt_dma(xt, sc):
                KH = KB // 4
                di = None
                for xh in range(4):
                    di = nc.sync.dma_start(
                        xt[:, xh * KH * SUB:(xh + 1) * KH * SUB]
                          .rearrange("p (kb t) -> p kb t", kb=KH),
                        xT.ap()[xh * KH * 128:(xh + 1) * KH * 128,
                                sc * SUB:(sc + 1) * SUB]
                          .rearrange("(kb p) t -> p kb t", p=128))
                return di

            xt_anchor = {}
            store_anchor = {}
            a2a_inst = {}
            xt0 = xpool.tile([128, KB * SUB], BF16, tag="xt", name="xt")
            xt_anchor[0] = xt_dma(xt0, 0)

            # ---- persistent constants in SBUF
            wq_sb = const.tile([128, KB * HPC * DK], BF16)
            wk_sb = const.tile([128, KB * HPC * DK], BF16)
            wv_sb = const.tile([128, KB * HPC * DK], BF16)
            weng = {0: nc.scalar, 1: nc.scalar, 2: nc.gpsimd}
            for ti, (sb_t, dr) in enumerate(
                    ((wq_sb, wqT), (wk_sb, wkT), (wv_sb, wvT))):
                ngrp = 8 if ti == 0 else 4
                GW = KB // ngrp
                for g in range(ngrp):
                    m0 = g * GW * HPC * DK
                    weng[ti].dma_start(
                        sb_t[:, m0:m0 + GW * HPC * DK]
                            .rearrange("p (kb m) -> p kb m", kb=GW),
                        dr.ap()[g * GW * 128:(g + 1) * GW * 128, :]
                          .rearrange("(kb p) m -> p kb m", p=128))
            ropeC_sb = const.tile([DK, S], F32)
            ropeS_sb = const.tile([DK, S], F32)
            maskd_sb = const.tile([JB, 4 * QCH], BF16)
            nc.scalar.dma_start(ropeC_sb[:, :SUB], ropeC[:, :SUB])
            nc.scalar.dma_start(ropeS_sb[:, :SUB], ropeS[:, :SUB])
            nc.scalar.dma_start(ropeC_sb[:, SUB:], ropeC[:, SUB:])
            nc.scalar.dma_start(ropeS_sb[:, SUB:], ropeS[:, SUB:])
            nc.scalar.dma_start(maskd_sb[:], maskd[:])

            def rope_combine(ps_in, out_ap, s0, n):
                """out = ps_in * C + shuffle(ps_in) * S  (RoPE)."""
                qsh = work.tile([128, SUB], F32, tag="qsh", name="qsh")
                t1 = work.tile([128, SUB], F32, tag="t1", name="t1")
                nc.vector.stream_shuffle(qsh[:, :n], ps_in, swap_mask)
                nc.any.tensor_tensor(
                    t1[:, :n], ps_in, ropeC_sb[:, s0:s0 + n], AluOpType.mult)
                nc.vector.tensor_tensor(
                    qsh[:, :n], qsh[:, :n], ropeS_sb[:, s0:s0 + n],
                    AluOpType.mult)
                nc.any.tensor_tensor(out_ap, t1[:, :n], qsh[:, :n],
                                     AluOpType.add)

            chain_engs = [nc.vector, nc.gpsimd]

            # Wo token-passes -------------------------------------------------
            # pass 0: chunks 0-3 (cols 0:256), interleaved into the second
            # half of the main loop; pass 1: chunks 4-6 (cols 256:448);
            # pass 2: chunk 7 (cols 448:512), reusing resident wo tiles.
            wo_engs = [nc.sync, nc.scalar]

            def load_woeb(eb, ei, anchor=None):
                wo_eb = wopool.tile([128, KB * DK], BF16, tag="wo",
                                    name="wo_eb")
                di = wo_engs[ei % 2].dma_start(
                    wo_eb[:],
                    woT.ap()[eb * D:(eb + 1) * D, :]
                       .rearrange("(p dl) e -> p (dl e)", p=128))
                if anchor is not None:
                    add_dep_helper(di.ins, anchor.ins, sync=True,
                                   reason="wo load after main loop")
                return wo_eb

            def load_yt(yt, chunks, ci0=0, anchors=None, eng=None):
                for ci, jc in enumerate(chunks):
                    di = (eng or nc.scalar).dma_start(
                        yt.rearrange("p (db c) -> p db c", db=KB)
                          [:, :, (ci0 + ci) * TPC:(ci0 + ci + 1) * TPC],
                        a2a_out[jc].rearrange("(db p) t -> p db t",
                                                   p=128))
                    if anchors is not None:
                        add_dep_helper(di.ins, anchors[ci].ins, sync=True,
                                       reason="yt load after its A2A landed")

            def wo_eb_mm(eb, yt, ncols, stage, wo_eb=None, ei=0):
                if wo_eb is None:
                    wo_eb = load_woeb(eb, ei)
                ps_w = ps_proj.tile([128, SUB], F32, tag="proj", name="ps_w")
                for dl in range(KB):
                    nc.tensor.matmul(
                        ps_w[:, :ncols],
                        wo_eb[:, dl * DK:(dl + 1) * DK],
                        yt[:, dl * ncols:(dl + 1) * ncols],
                        start=(dl == 0), stop=(dl == KB - 1))
                nc.any.tensor_copy(
                    stage[:, eb * ncols:(eb + 1) * ncols], ps_w[:, :ncols])

            def store_stage(stage, ncols, col0):
                nc.sync.dma_start(
                    outT.ap().rearrange("(eb p) c -> p eb c", p=128)
                        [:, :, col0:col0 + ncols],
                    stage.rearrange("p (eb c) -> p eb c", eb=KB))

            # ---------------- main pass over the sequence -------------------
            qT_tiles = {}
            kT_tiles = {}
            v_tiles = {}
            yt_p0 = [None]

            for sc in range(NSUB):
                b = sc // (NSUB // B)
                half = sc % 2
                iq = (sc % (NSUB // B)) // 2
                jc_glob = b * 4 + iq
                s0 = (sc % (NSUB // B)) * SUB   # position within batch

                if sc == 0:
                    xt = xt0
                else:
                    xt = xpool.tile([128, KB * SUB], BF16, tag="xt", name="xt")
                    xt_anchor[sc] = xt_dma(xt, sc)

                # ---- q/k projections + rope, both heads
                if half == 0:
                    for h in range(HPC):
                        qT_tiles[h] = qpool.tile([128, QCH], BF16,
                                                 tag=f"qT{h}", name="qT")
                for h in range(HPC):
                    if (b, h, iq) not in kT_tiles:
                        kT_tiles[(b, h, iq)] = kvpool.tile(
                            [128, QCH], BF16, tag="kT", name="kT")
                    for (w_sb, dst) in ((wq_sb, qT_tiles[h]),
                                        (wk_sb, kT_tiles[(b, h, iq)])):
                        psq = ps_proj.tile([128, SUB], F32, tag="proj", name="proj")
                        for kb in range(KB):
                            nc.tensor.matmul(
                                psq[:],
                                w_sb[:, kb * HPC * DK + h * DK:
                                     kb * HPC * DK + (h + 1) * DK],
                                xt[:, kb * SUB:(kb + 1) * SUB],
                                start=(kb == 0), stop=(kb == KB - 1))
                        rope_combine(psq[:],
                                     dst[:, half * SUB:(half + 1) * SUB],
                                     s0, SUB)

                # ---- v projection, both heads, bf16 storage
                for tb in range(SUB // 128):
                    jb_b = (sc % (NSUB // B)) * 2 + tb
                    psv = ps_proj.tile([128, HPC * DK], F32, tag="proj",
                                       name="psv")
                    for kb in range(KB):
                        nc.tensor.matmul(
                            psv[:],
                            xt[:, kb * SUB + tb * 128:
                               kb * SUB + (tb + 1) * 128],
                            wv_sb[:, kb * HPC * DK:(kb + 1) * HPC * DK],
                            start=(kb == 0), stop=(kb == KB - 1))
                    vt = vpool.tile([128, HPC * DK], BF16, tag="v", name="vt")
                    nc.any.tensor_copy(vt[:], psv[:])
                    v_tiles[(b, jb_b)] = vt

                # ---- attention for the completed query chunk
                if half == 1:
                    n_j = 4 * iq + 4
                    for h in range(HPC):
                        qT = qT_tiles[h]
                        ps_o = ps_out.tile([128, QCH], F32, tag="att_out", name="att_out")
                        p_tiles = {}

                        def emit_pair(jp):
                            # two key blocks share one [128, 2*QCH] PSUM
                            # tile and a single exp: halves Act-engine
                            # per-call overhead.
                            ps_s = ps_st.tile([JB, 2 * QCH], F32, tag="st",
                                              name="st")
                            for j2 in range(2):
                                j = 2 * jp + j2
                                jck, jr = j // 4, j % 4
                                m = j - 4 * iq
                                q0 = 128 * m if m > 0 else 0
                                nc.tensor.matmul(
                                    ps_s[:, j2 * QCH + q0:(j2 + 1) * QCH],
                                    kT_tiles[(b, h, jck)][:, jr * 128:
                                                          (jr + 1) * 128],
                                    qT[:, q0:], start=True, stop=True)
                            m0 = 2 * jp - 4 * iq
                            if m0 >= 0:        # diagonal pair: mask both
                                nc.vector.tensor_tensor(
                                    ps_s[:], ps_s[:],
                                    maskd_sb[:, m0 * QCH:(m0 + 2) * QCH],
                                    AluOpType.add)
                            p_t = ppool.tile([JB, 2 * QCH], BF16, tag="p",
                                             name="p")
                            nc.scalar.activation(
                                p_t[:], ps_s[:],
                                mybir.ActivationFunctionType.Exp)
                            p_tiles[jp] = p_t

                        n_p = n_j // 2
                        emit_pair(0)
                        # l = sum_k p: two sequential accumulator chains
                        # over pair tiles, spread across DVE/Pool
                        acc = [None, None]
                        n_add = 0
                        cengs = (chain_engs if jc_glob < NCH - 1
                                 else [nc.vector])
                        for jp in range(n_p):
                            if jp + 1 < n_p:
                                emit_pair(jp + 1)
                            p_t = p_tiles[jp]
                            for j2 in range(2):
                                j = 2 * jp + j2
                                nc.tensor.matmul(
                                    ps_o[:], v_tiles[(b, j)][:, h * DK:
                                                             (h + 1) * DK],
                                    p_t[:, j2 * QCH:(j2 + 1) * QCH],
                                    start=(j == 0), stop=(j == n_j - 1))
                            c = jp % 2
                            if jp >= 2:
                                eng = cengs[n_add % len(cengs)]
                                n_add += 1
                                if acc[c] is None:
                                    a_t = lpool.tile([128, 2 * QCH], BF16,
                                                     tag="acc", name="acc")
                                    eng.tensor_tensor(
                                        a_t[:], p_tiles[c][:], p_t[:],
                                        AluOpType.add)
                                    acc[c] = a_t
                                else:
                                    eng.tensor_tensor(
                                        acc[c][:], acc[c][:], p_t[:],
                                        AluOpType.add)
                        if acc[0] is None:       # n_p == 2
                            acc = [p_tiles[0], p_tiles[1]]
                                                p_acc = lpool.tile([128, 2 * QCH], BF16,
                                           tag="acc2", name="acc2")
                        cengs[n_add % len(cengs)].tensor_tensor(
                            p_acc[:], acc[0][:], acc[1][:], AluOpType.add)
                        p_fold = lpool.tile([128, QCH], BF16, tag="fold",
                                            name="fold")
                        nc.vector.tensor_tensor(
                            p_fold[:], p_acc[:, :QCH], p_acc[:, QCH:],
                            AluOpType.add)
                                                l_bc = lpool.tile([128, QCH], F32, tag="lbc", name="lbc")
                        nc.gpsimd.partition_all_reduce(
                            l_bc[:], p_fold[:], channels=128,
                            reduce_op=bass_isa.ReduceOp.add)
                        r_bc = lpool.tile([128, QCH], F32, tag="rbc", name="rbc")
                        nc.vector.reciprocal(r_bc[:], l_bc[:])
                        y_sb = ypool.tile([128, QCH], BF16, tag="y", name="y")
                        nc.any.tensor_tensor(y_sb[:], ps_o[:], r_bc[:],
                                             AluOpType.mult)
                        # scatter into the AllToAll source layout:
                        # dst row m*256 + h*128 + p, col t <- src[p, m*64+t]
                        di = nc.gpsimd.dma_start(
                            a2a_in[jc_glob]
                            .rearrange("(m hp) t -> hp m t", m=N_CORES)
                            [h * 128:(h + 1) * 128],
                            y_sb.rearrange("p (m t) -> p m t", m=N_CORES))
                        if h == HPC - 1:
                            store_anchor[jc_glob] = di
                    a2a_inst[jc_glob] = nc.gpsimd.collective_compute(
                        "AllToAll", AluOpType.bypass,
                        replica_groups=[list(range(N_CORES))],
                        ins=[a2a_in[jc_glob].opt()],
                        outs=[a2a_out[jc_glob].opt()])


            # ---- Wo pass A: chunks 0-3 (cols 0:256).  Runs entirely in
            # the shadow of the last AllToAll: those chunks' y landed long
            # ago.  Every staging DMA is anchored so the dataflow
            # scheduler cannot hoist its wait into the main loop.
            yt_p0[0] = ytpool.tile([128, KB * 4 * TPC], BF16,
                                   tag="yt0", name="yt0")
            load_yt(yt_p0[0], range(4),
                    anchors=[store_anchor[c + 1] for c in range(4)])
            yt_p1 = ytpool.tile([128, KB * 3 * TPC], BF16, tag="yt1",
                                name="yt1")
            load_yt(yt_p1, range(4, 7),
                    anchors=[store_anchor[5], store_anchor[6],
                             store_anchor[7]])
            stage_a = stpool.tile([128, KB * 4 * TPC], BF16, tag="sta",
                                  name="sta")
            stage_b1 = stpool.tile([128, KB * 3 * TPC], BF16, tag="stb1",
                                   name="stb1")
            stage_b2 = stpool.tile([128, KB * TPC], BF16, tag="stb2",
                                   name="stb2")
            p0_pref = {eb: load_woeb(eb, eb, anchor=xt_anchor[11])
                       for eb in range(2)}
            for eb in range(KB):
                if eb + 2 < KB:       # stay 2 loads ahead of the matmuls
                    p0_pref[eb + 2] = load_woeb(eb + 2, eb,
                                                anchor=xt_anchor[11])
                wo_eb_mm(eb, yt_p0[0], 4 * TPC, stage_a,
                         wo_eb=p0_pref.pop(eb), ei=eb)

            store_stage(stage_a, 4 * TPC, 0)

            # ---- Wo pass B1: chunks 4-6 (cols 256:448) -- none of this
            # waits on the final AllToAll, so it fills the A2A-7 shadow
            # right after pass A.
            b1_tiles = {eb: load_woeb(eb, eb, anchor=xt_anchor[13])
                        for eb in range(2)}
            for eb in range(KB):
                if eb + 2 < KB:
                    b1_tiles[eb + 2] = load_woeb(eb + 2, eb,
                                                 anchor=xt_anchor[13])
                wo_eb_mm(eb, yt_p1, 3 * TPC, stage_b1,
                         wo_eb=b1_tiles[eb], ei=eb)

            store_stage(stage_b1, 3 * TPC, 4 * TPC)

            # ---- Wo pass B2: chunk 7 (cols 448:512), gated on the final
            # AllToAll.  The last wopool-1 tiles of B1 are still resident:
            # run those first, re-stream the rest.
            yt_p2 = ytpool.tile([128, KB * TPC], BF16, tag="yt2",
                                name="yt2")
            load_yt(yt_p2, range(7, 8),
                    anchors=[store_anchor[7]], eng=nc.gpsimd)
            for eb in range(KB - 13, KB):
                wo_eb_mm(eb, yt_p2, TPC, stage_b2, wo_eb=b1_tiles[eb])
            for ei, eb in enumerate(range(KB - 13)):
                wo_eb_mm(eb, yt_p2, TPC, stage_b2, ei=ei)
            store_stage(stage_b2, TPC, 7 * TPC)

    nc.finalize()
    return nc


# ---------------------------------------------------------------- host
def _host_inputs(x, W_q, W_k, W_v, W_o):
    import ml_dtypes
    bf = np.dtype(ml_dtypes.bfloat16)
    xT = np.ascontiguousarray(
        x.reshape(TOK, D).T).astype(bf)                        # [D, TOK]

    # W_o.T tiled (eb, p, dl, e'): row eb*2048 + p*16 + dl, col e'
    woT = np.ascontiguousarray(
        W_o.T.reshape(KB, 128, KB, DK).transpose(2, 1, 0, 3)
        .reshape(KB * D, DK)).astype(bf)

    # RoPE tables, expanded to [DK, S] with interleaved pairs; the sign
    # table carries -sin on even rows, +sin on odd rows.
    i = np.arange(0, DK, 2, dtype=np.float32)
    theta = 1.0 / (ROPE_BASE ** (i / DK))                      # [64]
    pos = np.arange(S, dtype=np.float32)
    freqs = pos[:, None] * theta[None, :]                      # [S, 64]
    cos_t, sin_t = np.cos(freqs), np.sin(freqs)
    ropeC = np.empty((DK, S), np.float32)
    ropeS = np.empty((DK, S), np.float32)
    ropeC[0::2] = cos_t.T
    ropeC[1::2] = cos_t.T
    ropeS[0::2] = -sin_t.T
    ropeS[1::2] = sin_t.T

    # diagonal causal masks: block m (of the 4 key blocks overlapping a
    # 512-query chunk) keeps kk <= qq - 128*m
    kk = np.arange(JB)[:, None]
    qq = np.arange(QCH)[None, :]
    maskd = np.concatenate(
        [np.where(kk <= qq - 128 * m, 0.0, MASK_NEG).astype(np.float32)
         for m in range(4)], axis=1).astype(bf)                # [128, 4*512]

    scale = 1.0 / np.sqrt(np.float32(DK))
    in_maps = []
    for c in range(N_CORES):
        rows = slice(c * HPC * DK, (c + 1) * HPC * DK)
        in_maps.append({
            "xT": xT,
            "wqT": np.ascontiguousarray(
                (W_q[rows] * scale).T).astype(bf),
            "wkT": np.ascontiguousarray(W_k[rows].T).astype(bf),
            "wvT": np.ascontiguousarray(W_v[rows].T).astype(bf),
            "woT": woT,
            "ropeC": ropeC,
            "ropeS": ropeS,
            "maskd": maskd,
        })
    return in_maps


def kernel(x, W_q, W_k, W_v, W_o):
    x = np.asarray(x, dtype=np.float32)
    W_q = np.asarray(W_q, dtype=np.float32)
    W_k = np.asarray(W_k, dtype=np.float32)
    W_v = np.asarray(W_v, dtype=np.float32)
    W_o = np.asarray(W_o, dtype=np.float32)

    if "nc" not in _CACHE:
        _CACHE["nc"] = _build_nc()
    nc = _CACHE["nc"]

    in_maps = _host_inputs(x, W_q, W_k, W_v, W_o)
    res = bass_utils.run_bass_kernel_spmd(
        nc, in_maps, core_ids=list(range(N_CORES)))

    # outT per core: [D, 512] f32; col 64*jc + t -> token 512*jc + 64*c + t
    out_T = np.empty((D, TOK), np.float32)
    for c in range(N_CORES):
        cols = (QCH * np.arange(NCH)[:, None] + TPC * c
                + np.arange(TPC)[None, :]).ravel()
        out_T[:, cols] = res.results[c]["outT"].astype(np.float32)
    return np.ascontiguousarray(out_T.T).reshape(B, S, D).astype(np.float32)


# revision 53
# speedup vs baseline: 1.4495x; 1.0459x over previous
"""Causal self-attention with RoPE on 8 Trainium2 NeuronCores.

Sharding: tensor-parallel over heads (16 heads -> 2 per core) for the
QKV projections, RoPE and attention.  The attention output is
re-sharded token-wise with one small AllToAll per 512-token chunk
(bf16 payload), so each core ends up with the full 2048-dim y vector
for 64 tokens of every chunk; the output projection then runs
token-parallel with no all-reduce.

Single pass over the sequence: both heads' q/k/v are produced from one
streaming of x (the baseline used one pass per head and loaded x
twice).

Key engine-placement choices (cost-model driven):
  - softmax denominator l = sum_k p is NOT computed with PE matmuls
    (those cost as much as the score matmuls); instead p-blocks are
    summed with a shallow tree of tensor_tensor adds spread across
    DVE/Act/Pool and one gpsimd.partition_all_reduce, whose output is
    already broadcast across partitions (also kills the r-broadcast
    matmul of the baseline).
  - y, v and W_o are bf16 (halves the collective payload and the Wo
    weight traffic; bf16 matmuls run at full PE rate).
  - W_o streams through SBUF per token-pass; the last pass reuses the
    still-resident tiles of the previous pass for its first blocks.

Shapes (hardcoded): x [2, 2048, 2048], W_* [2048, 2048], 16 heads,
d_k = 128, fp32 in/out.
"""

import sys

for _p in ("/opt/trn_rl_repo", "/opt/pypackages"):
    if _p not in sys.path:
        sys.path.insert(0, _p)

import numpy as np

import concourse.bass as bass
import concourse.bacc as bacc
import concourse.mybir as mybir
import concourse.tile as tile
from concourse import bass_utils
from concourse import bass_isa
from concourse.alu_op_type import AluOpType
from concourse.tile import add_dep_helper

# ---------------------------------------------------------------- config
N_CORES = 8
B, S, D = 2, 2048, 2048
H = 16
DK = D // H              # 128
HPC = H // N_CORES       # 2 heads per core
TOK = B * S              # 4096
SUB = 512                # token sub-chunk for projections (= one chunk)
QCH = 512                # attention query chunk
JB = 128                 # attention key block
NSUB = TOK // SUB        # 16
NCH = TOK // QCH         # 8 query chunks
KB = D // 128            # 16 contraction blocks
TPC = QCH // N_CORES     # 64 tokens per (chunk, core) after AllToAll
ROPE_BASE = 10000.0
MASK_NEG = -30000.0

F32 = mybir.dt.float32
F32R = mybir.dt.float32r
BF16 = mybir.dt.bfloat16

_CACHE = {}


def _build_nc():
    dt = F32R
    nc = bacc.Bacc("TRN2", target_bir_lowering=False, debug=False,
                   num_devices=N_CORES)

    xT = nc.dram_tensor("xT", [D, TOK], BF16, kind="ExternalInput")
    wqT = nc.dram_tensor("wqT", [D, HPC * DK], BF16, kind="ExternalInput")
    wkT = nc.dram_tensor("wkT", [D, HPC * DK], BF16, kind="ExternalInput")
    wvT = nc.dram_tensor("wvT", [D, HPC * DK], BF16, kind="ExternalInput")
    # W_o.T in bf16, tiled (eb, p, dl, e'): row = eb*2048 + p*16 + dl
    woT = nc.dram_tensor("woT", [KB * D, DK], BF16, kind="ExternalInput")
    ropeC = nc.dram_tensor("ropeC", [DK, S], F32, kind="ExternalInput")
    ropeS = nc.dram_tensor("ropeS", [DK, S], F32, kind="ExternalInput")
    maskd = nc.dram_tensor("maskd", [JB, 4 * QCH], BF16, kind="ExternalInput")
    # out columns ordered (jc 0..7, t 0..63): global token 512*jc + 64*c + t
    outT = nc.dram_tensor("outT", [D, QCH], BF16, kind="ExternalOutput")

    swap_mask = [i ^ 1 for i in range(32)]

    import contextlib
    with tile.TileContext(nc) as tc:
        with contextlib.ExitStack() as st:
            dram = st.enter_context(
                tc.tile_pool(name="dram", bufs=1, space="DRAM"))
            a2a_in = [dram.tile([N_CORES * HPC * DK, TPC], BF16,
                                name=f"a2ain{j}") for j in range(NCH)]
            a2a_out = [dram.tile([N_CORES * HPC * DK, TPC], BF16,
                                 name=f"a2aout{j}") for j in range(NCH)]

            const = st.enter_context(tc.tile_pool(name="const", bufs=1))
            xpool = st.enter_context(
                tc.tile_pool(name="xpool", bufs=2, side="right"))
            qpool = st.enter_context(
                tc.tile_pool(name="qpool", bufs=2, side="right"))
            kvpool = st.enter_context(
                tc.tile_pool(name="kvpool", bufs=8, side="right"))
            vpool = st.enter_context(
                tc.tile_pool(name="vpool", bufs=15, side="right"))
            ppool = st.enter_context(tc.tile_pool(name="ppool", bufs=7))
            lpool = st.enter_context(tc.tile_pool(name="lpool", bufs=2))
            work = st.enter_context(tc.tile_pool(name="work", bufs=2))
            ypool = st.enter_context(tc.tile_pool(name="ypool", bufs=3))
            ytpool = st.enter_context(tc.tile_pool(name="ytpool", bufs=1))
            stpool = st.enter_context(tc.tile_pool(name="stpool", bufs=1))
            wopool = st.enter_context(
                tc.tile_pool(name="wopool", bufs=14, side="right"))
            ps_proj = st.enter_context(
                tc.tile_pool(name="ps_proj", bufs=3, space="PSUM"))
            ps_st = st.enter_context(
                tc.tile_pool(name="ps_st", bufs=3, space="PSUM"))
            ps_out = st.enter_context(
                tc.tile_pool(name="ps_out", bufs=2, space="PSUM"))

            # chunk-0 xT goes first on its queues so the first projection
            # isn't stuck behind weight DMAs
            def xt_dma(xt, sc):
                KH = KB // 4
                di = None
                for xh in range(4):
                    di = nc.sync.dma_start(
                        xt[:, xh * KH * SUB:(xh + 1) * KH * SUB]
                          .rearrange("p (kb t) -> p kb t", kb=KH),
                        xT.ap()[xh * KH * 128:(xh + 1) * KH * 128,
                                sc * SUB:(sc + 1) * SUB]
                          .rearrange("(kb p) t -> p kb t", p=128))
                return di

            xt_anchor = {}
            store_anchor = {}
            a2a_inst = {}
            xt0 = xpool.tile([128, KB * SUB], BF16, tag="xt", name="xt")
            xt_anchor[0] = xt_dma(xt0, 0)

            # ---- persistent constants in SBUF
            wq_sb = const.tile([128, KB * HPC * DK], BF16)
            wk_sb = const.tile([128, KB * HPC * DK], BF16)
            wv_sb = const.tile([128, KB * HPC * DK], BF16)
            weng = {0: nc.scalar, 1: nc.scalar, 2: nc.gpsimd}
            for ti, (sb_t, dr) in enumerate(
                    ((wq_sb, wqT), (wk_sb, wkT), (wv_sb, wvT))):
                ngrp = 8 if ti == 0 else 4
                GW = KB // ngrp
                for g in range(ngrp):
                    m0 = g * GW * HPC * DK
                    weng[ti].dma_start(
                        sb_t[:, m0:m0 + GW * HPC * DK]
                            .rearrange("p (kb m) -> p kb m", kb=GW),
                        dr.ap()[g * GW * 128:(g + 1) * GW * 128, :]
                          .rearrange("(kb p) m -> p kb m", p=128))
            ropeC_sb = const.tile([DK, S], F32)
            ropeS_sb = const.tile([DK, S], F32)
            maskd_sb = const.tile([JB, 4 * QCH], BF16)
            nc.scalar.dma_start(ropeC_sb[:, :SUB], ropeC[:, :SUB])
            nc.scalar.dma_start(ropeS_sb[:, :SUB], ropeS[:, :SUB])
            nc.scalar.dma_start(ropeC_sb[:, SUB:], ropeC[:, SUB:])
            nc.scalar.dma_start(ropeS_sb[:, SUB:], ropeS[:, SUB:])
            nc.scalar.dma_start(maskd_sb[:], maskd[:])

            def rope_combine(ps_in, out_ap, s0, n):
                """out = ps_in * C + shuffle(ps_in) * S  (RoPE)."""
                qsh = work.tile([128, SUB], F32, tag="qsh", name="qsh")
                t1 = work.tile([128, SUB], BF16, tag="t1", name="t1")
                nc.vector.stream_shuffle(qsh[:, :n], ps_in, swap_mask)
                nc.any.tensor_tensor(
                    t1[:, :n], ps_in, ropeC_sb[:, s0:s0 + n], AluOpType.mult)
                nc.vector.tensor_tensor(
                    qsh[:, :n], qsh[:, :n], ropeS_sb[:, s0:s0 + n],
                    AluOpType.mult)
                nc.any.tensor_tensor(out_ap, t1[:, :n], qsh[:, :n],
                                     AluOpType.add)

            chain_engs = [nc.vector, nc.gpsimd]

            # Wo token-passes -------------------------------------------------
            # pass 0: chunks 0-3 (cols 0:256), interleaved into the second
            # half of the main loop; pass 1: chunks 4-6 (cols 256:448);
            # pass 2: chunk 7 (cols 448:512), reusing resident wo tiles.
            wo_engs = [nc.sync, nc.scalar]

            def load_woeb(eb, ei, anchor=None):
                wo_eb = wopool.tile([128, KB * DK], BF16, tag="wo",
                                    name="wo_eb")
                di = wo_engs[ei % 2].dma_start(
                    wo_eb[:],
                    woT.ap()[eb * D:(eb + 1) * D, :]
                       .rearrange("(p dl) e -> p (dl e)", p=128))
                if anchor is not None:
                    add_dep_helper(di.ins, anchor.ins, sync=True,
                                   reason="wo load after main loop")
                return wo_eb

            def load_yt(yt, chunks, ci0=0, anchors=None, eng=None):
                for ci, jc in enumerate(chunks):
                    di = (eng or nc.scalar).dma_start(
                        yt.rearrange("p (db c) -> p db c", db=KB)
                          [:, :, (ci0 + ci) * TPC:(ci0 + ci + 1) * TPC],
                        a2a_out[jc].rearrange("(db p) t -> p db t",
                                                   p=128))
                    if anchors is not None:
                        add_dep_helper(di.ins, anchors[ci].ins, sync=True,
                                       reason="yt load after its A2A landed")

            def wo_eb_mm(eb, yt, ncols, stage, wo_eb=None, ei=0):
                if wo_eb is None:
                    wo_eb = load_woeb(eb, ei)
                ps_w = ps_proj.tile([128, SUB], F32, tag="proj", name="ps_w")
                for dl in range(KB):
                    nc.tensor.matmul(
                        ps_w[:, :ncols],
                        wo_eb[:, dl * DK:(dl + 1) * DK],
                        yt[:, dl * ncols:(dl + 1) * ncols],
                        start=(dl == 0), stop=(dl == KB - 1))
                nc.any.tensor_copy(
                    stage[:, eb * ncols:(eb + 1) * ncols], ps_w[:, :ncols])

            def store_stage(stage, ncols, col0):
                nc.sync.dma_start(
                    outT.ap().rearrange("(eb p) c -> p eb c", p=128)
                        [:, :, col0:col0 + ncols],
                    stage.rearrange("p (eb c) -> p eb c", eb=KB))

            # ---------------- main pass over the sequence -------------------
            qT_tiles = {}
            kT_tiles = {}
            v_tiles = {}
            yt_p0 = [None]

            for sc in range(NSUB):
                b = sc // (NSUB // B)
                iq = sc % (NSUB // B)
                jc_glob = sc
                s0 = iq * SUB                   # position within batch

                if sc == 0:
                    xt = xt0
                else:
                    xt = xpool.tile([128, KB * SUB], BF16, tag="xt", name="xt")
                    xt_anchor[sc] = xt_dma(xt, sc)

                # ---- q/k projections + rope, both heads
                for h in range(HPC):
                    qT_tiles[h] = qpool.tile([128, QCH], BF16,
                                             tag=f"qT{h}", name="qT")
                    kT_tiles[(b, h, iq)] = kvpool.tile(
                        [128, QCH], BF16, tag="kT", name="kT")
                    for (w_sb, dst) in ((wq_sb, qT_tiles[h]),
                                        (wk_sb, kT_tiles[(b, h, iq)])):
                        psq = ps_proj.tile([128, SUB], F32, tag="proj", name="proj")
                        for kb in range(KB):
                            nc.tensor.matmul(
                                psq[:],
                                w_sb[:, kb * HPC * DK + h * DK:
                                     kb * HPC * DK + (h + 1) * DK],
                                xt[:, kb * SUB:(kb + 1) * SUB],
                                start=(kb == 0), stop=(kb == KB - 1))
                        rope_combine(psq[:], dst[:], s0, SUB)

                # ---- v projection, both heads, bf16 storage
                for tb in range(SUB // 128):
                    jb_b = iq * (SUB // 128) + tb
                    psv = ps_proj.tile([128, HPC * DK], F32, tag="proj",
                                       name="psv")
                    for kb in range(KB):
                        nc.tensor.matmul(
                            psv[:],
                            xt[:, kb * SUB + tb * 128:
                               kb * SUB + (tb + 1) * 128],
                            wv_sb[:, kb * HPC * DK:(kb + 1) * HPC * DK],
                            start=(kb == 0), stop=(kb == KB - 1))
                    vt = vpool.tile([128, HPC * DK], BF16, tag="v", name="vt")
                    nc.any.tensor_copy(vt[:], psv[:])
                    v_tiles[(b, jb_b)] = vt

                # ---- attention for the completed query chunk
                if True:
                    n_j = 4 * iq + 4
                    for h in range(HPC):
                        qT = qT_tiles[h]
                        ps_o = ps_out.tile([128, QCH], F32, tag="att_out", name="att_out")
                        p_tiles = {}

                        def emit_block(j):
                            jck, jr = j // 4, j % 4
                            m = j - 4 * iq
                            # diagonal block m: columns [0, 128m) are fully
                            # masked -> skip them and zero-fill p instead
                            q0 = 128 * m if m > 0 else 0
                            ps_s = ps_st.tile([JB, QCH], F32, tag="st", name="st")
                            nc.tensor.matmul(
                                ps_s[:, q0:],
                                kT_tiles[(b, h, jck)][:, jr * 128:
                                                      (jr + 1) * 128],
                                qT[:, q0:], start=True, stop=True)
                            if m >= 0:           # diagonal block: mask
                                nc.vector.tensor_tensor(
                                    ps_s[:, q0:], ps_s[:, q0:],
                                    maskd_sb[:, m * QCH + q0:(m + 1) * QCH],
                                    AluOpType.add)
                            p_t = ppool.tile([JB, QCH], BF16, tag="p", name="p")
                            if q0 > 0:
                                nc.gpsimd.memset(p_t[:, :q0], 0.0)
                            nc.scalar.activation(
                                p_t[:, q0:], ps_s[:, q0:],
                                mybir.ActivationFunctionType.Exp)
                            p_tiles[j] = p_t

                        # scores/exp run two blocks ahead of the AV matmuls
                        # so PE never waits on the Act engine
                        emit_block(0)
                        if n_j > 1:
                            emit_block(1)
                        # l = sum_k p: two sequential accumulator chains
                        # (even/odd j) spread across DVE/Pool/Act
                        acc = [None, None]
                        n_add = 0
                        cengs = (chain_engs if jc_glob < NCH - 1
                                 else [nc.vector])
                        for j in range(n_j):
                            if j + 2 < n_j:
                                emit_block(j + 2)
                            p_t = p_tiles[j]
                            nc.tensor.matmul(
                                ps_o[:], v_tiles[(b, j)][:, h * DK:
                                                         (h + 1) * DK],
                                p_t[:],
                                start=(j == 0), stop=(j == n_j - 1))
                            c = j % 2
                            if j >= 2:
                                eng = cengs[0]
                                n_add += 1
                                if acc[c] is None:
                                    a_t = lpool.tile([128, QCH], BF16,
                                                     tag="acc", name="acc")
                                    eng.tensor_tensor(
                                        a_t[:], p_tiles[c][:], p_t[:],
                                        AluOpType.add)
                                    acc[c] = a_t
                                else:
                                    eng.tensor_tensor(
                                        acc[c][:], acc[c][:], p_t[:],
                                        AluOpType.add)
                        if acc[0] is None:       # n_j == 4 has j = 0..3
                            acc = [p_tiles[0], p_tiles[1]]
                        p_acc = lpool.tile([128, QCH], BF16, tag="acc2", name="acc2")
                        cengs[0].tensor_tensor(
                            p_acc[:], acc[0][:], acc[1][:], AluOpType.add)
                        # gpsimd all-reduce output is broadcast across
                        # partitions -> no r-broadcast matmul needed
                        l_bc = lpool.tile([128, QCH], F32, tag="lbc", name="lbc")
                        nc.gpsimd.partition_all_reduce(
                            l_bc[:], p_acc[:], channels=128,
                            reduce_op=bass_isa.ReduceOp.add)
                        nc.vector.reciprocal(l_bc[:], l_bc[:])
                        y_sb = ypool.tile([128, QCH], BF16, tag="y", name="y")
                        nc.any.tensor_tensor(y_sb[:], ps_o[:], l_bc[:],
                                             AluOpType.mult)
                        # scatter into the AllToAll source layout:
                        # dst row m*256 + h*128 + p, col t <- src[p, m*64+t]
                        di = nc.gpsimd.dma_start(
                            a2a_in[jc_glob]
                            .rearrange("(m hp) t -> hp m t", m=N_CORES)
                            [h * 128:(h + 1) * 128],
                            y_sb.rearrange("p (m t) -> p m t", m=N_CORES))
                        if h == HPC - 1:
                            store_anchor[jc_glob] = di
                    a2a_inst[jc_glob] = nc.gpsimd.collective_compute(
                        "AllToAll", AluOpType.bypass,
                        replica_groups=[list(range(N_CORES))],
                        ins=[a2a_in[jc_glob].opt()],
                        outs=[a2a_out[jc_glob].opt()])


            # ---- Wo pass A: chunks 0-3 (cols 0:256).  Runs entirely in
            # the shadow of the last AllToAll: those chunks' y landed long
            # ago.  Every staging DMA is anchored so the dataflow
            # scheduler cannot hoist its wait into the main loop.
            yt_p0[0] = ytpool.tile([128, KB * 4 * TPC], BF16,
                                   tag="yt0", name="yt0")
            load_yt(yt_p0[0], range(4),
                    anchors=[store_anchor[c + 1] for c in range(4)])
            yt_p1 = ytpool.tile([128, KB * 3 * TPC], BF16, tag="yt1",
                                name="yt1")
            load_yt(yt_p1, range(4, 7),
                    anchors=[store_anchor[5], store_anchor[6],
                             store_anchor[7]])
            stage_a = stpool.tile([128, KB * 4 * TPC], BF16, tag="sta",
                                  name="sta")
            stage_b1 = stpool.tile([128, KB * 3 * TPC], BF16, tag="stb1",
                                   name="stb1")
            stage_b2 = stpool.tile([128, KB * TPC], BF16, tag="stb2",
                                   name="stb2")
            p0_pref = {eb: load_woeb(eb, eb, anchor=xt_anchor[5])
                       for eb in range(2)}
            for eb in range(KB):
                if eb + 2 < KB:       # stay 2 loads ahead of the matmuls
                    p0_pref[eb + 2] = load_woeb(eb + 2, eb,
                                                anchor=xt_anchor[5])
                wo_eb_mm(eb, yt_p0[0], 4 * TPC, stage_a,
                         wo_eb=p0_pref.pop(eb), ei=eb)

            store_stage(stage_a, 4 * TPC, 0)

            # ---- Wo pass B1: chunks 4-6 (cols 256:448) -- none of this
            # waits on the final AllToAll, so it fills the A2A-7 shadow
            # right after pass A.
            b1_tiles = {eb: load_woeb(eb, eb, anchor=xt_anchor[6])
                        for eb in range(2)}
            for eb in range(KB):
                if eb + 2 < KB:
                    b1_tiles[eb + 2] = load_woeb(eb + 2, eb,
                                                 anchor=xt_anchor[6])
                wo_eb_mm(eb, yt_p1, 3 * TPC, stage_b1,
                         wo_eb=b1_tiles[eb], ei=eb)

            store_stage(stage_b1, 3 * TPC, 4 * TPC)

            # ---- Wo pass B2: chunk 7 (cols 448:512), gated on the final
            # AllToAll.  The last wopool-1 tiles of B1 are still resident:
            # run those first, re-stream the rest.
            yt_p2 = ytpool.tile([128, KB * TPC], BF16, tag="yt2",
                                name="yt2")
            load_yt(yt_p2, range(7, 8),
                    anchors=[store_anchor[7]], eng=nc.gpsimd)
            for eb in range(KB - 13, KB):
                wo_eb_mm(eb, yt_p2, TPC, stage_b2, wo_eb=b1_tiles[eb])
            for ei, eb in enumerate(range(KB - 13)):
                wo_eb_mm(eb, yt_p2, TPC, stage_b2, ei=ei)
            store_stage(stage_b2, TPC, 7 * TPC)

    nc.finalize()
    return nc


# ---------------------------------------------------------------- host
def _host_inputs(x, W_q, W_k, W_v, W_o):
    import ml_dtypes
    bf = np.dtype(ml_dtypes.bfloat16)
    xT = np.ascontiguousarray(
        x.reshape(TOK, D).T).astype(bf)                        # [D, TOK]

    # W_o.T tiled (eb, p, dl, e'): row eb*2048 + p*16 + dl, col e'
    woT = np.ascontiguousarray(
        W_o.T.reshape(KB, 128, KB, DK).transpose(2, 1, 0, 3)
        .reshape(KB * D, DK)).astype(bf)

    # RoPE tables, expanded to [DK, S] with interleaved pairs; the sign
    # table carries -sin on even rows, +sin on odd rows.
    i = np.arange(0, DK, 2, dtype=np.float32)
    theta = 1.0 / (ROPE_BASE ** (i / DK))                      # [64]
    pos = np.arange(S, dtype=np.float32)
    freqs = pos[:, None] * theta[None, :]                      # [S, 64]
    cos_t, sin_t = np.cos(freqs), np.sin(freqs)
    ropeC = np.empty((DK, S), np.float32)
    ropeS = np.empty((DK, S), np.float32)
    ropeC[0::2] = cos_t.T
    ropeC[1::2] = cos_t.T
    ropeS[0::2] = -sin_t.T
    ropeS[1::2] = sin_t.T

    # diagonal causal masks: block m (of the 4 key blocks overlapping a
    # 512-query chunk) keeps kk <= qq - 128*m
    kk = np.arange(JB)[:, None]
    qq = np.arange(QCH)[None, :]
    maskd = np.concatenate(
        [np.where(kk <= qq - 128 * m, 0.0, MASK_NEG).astype(np.float32)
         for m in range(4)], axis=1).astype(bf)                # [128, 4*512]

    scale = 1.0 / np.sqrt(np.float32(DK))
    in_maps = []
    for c in range(N_CORES):
        rows = slice(c * HPC * DK, (c + 1) * HPC * DK)
        in_maps.append({
            "xT": xT,
            "wqT": np.ascontiguousarray(
                (W_q[rows] * scale).T).astype(bf),
            "wkT": np.ascontiguousarray(W_k[rows].T).astype(bf),
            "wvT": np.ascontiguousarray(W_v[rows].T).astype(bf),
            "woT": woT,
            "ropeC": ropeC,
            "ropeS": ropeS,
            "maskd": maskd,
        })
    return in_maps


def kernel(x, W_q, W_k, W_v, W_o):
    x = np.asarray(x, dtype=np.float32)
    W_q = np.asarray(W_q, dtype=np.float32)
    W_k = np.asarray(W_k, dtype=np.float32)
    W_v = np.asarray(W_v, dtype=np.float32)
    W_o = np.asarray(W_o, dtype=np.float32)

    if "nc" not in _CACHE:
        _CACHE["nc"] = _build_nc()
    nc = _CACHE["nc"]

    in_maps = _host_inputs(x, W_q, W_k, W_v, W_o)
    res = bass_utils.run_bass_kernel_spmd(
        nc, in_maps, core_ids=list(range(N_CORES)))

    # outT per core: [D, 512] f32; col 64*jc + t -> token 512*jc + 64*c + t
    out_T = np.empty((D, TOK), np.float32)
    for c in range(N_CORES):
        cols = (QCH * np.arange(NCH)[:, None] + TPC * c
                + np.arange(TPC)[None, :]).ravel()
        out_T[:, cols] = res.results[c]["outT"].astype(np.float32)
    return np.ascontiguousarray(out_T.T).reshape(B, S, D).astype(np.float32)


# revision 58
# speedup vs baseline: 1.4556x; 1.0042x over previous
"""Causal self-attention with RoPE on 8 Trainium2 NeuronCores.

Sharding: tensor-parallel over heads (16 heads -> 2 per core) for the
QKV projections, RoPE and attention.  The attention output is
re-sharded token-wise with one small AllToAll per 512-token chunk
(bf16 payload), so each core ends up with the full 2048-dim y vector
for 64 tokens of every chunk; the output projection then runs
token-parallel with no all-reduce.

Single pass over the sequence: both heads' q/k/v are produced from one
streaming of x (the baseline used one pass per head and loaded x
twice).

Key engine-placement choices (cost-model driven):
  - softmax denominator l = sum_k p is NOT computed with PE matmuls
    (those cost as much as the score matmuls); instead p-blocks are
    summed with a shallow tree of tensor_tensor adds spread across
    DVE/Act/Pool and one gpsimd.partition_all_reduce, whose output is
    already broadcast across partitions (also kills the r-broadcast
    matmul of the baseline).
  - y, v and W_o are bf16 (halves the collective payload and the Wo
    weight traffic; bf16 matmuls run at full PE rate).
  - W_o streams through SBUF per token-pass; the last pass reuses the
    still-resident tiles of the previous pass for its first blocks.

Shapes (hardcoded): x [2, 2048, 2048], W_* [2048, 2048], 16 heads,
d_k = 128, fp32 in/out.
"""

import sys

for _p in ("/opt/trn_rl_repo", "/opt/pypackages"):
    if _p not in sys.path:
        sys.path.insert(0, _p)

import numpy as np

import concourse.bass as bass
import concourse.bacc as bacc
import concourse.mybir as mybir
import concourse.tile as tile
from concourse import bass_utils
from concourse import bass_isa
from concourse.alu_op_type import AluOpType
from concourse.tile import add_dep_helper

# ---------------------------------------------------------------- config
N_CORES = 8
B, S, D = 2, 2048, 2048
H = 16
DK = D // H              # 128
HPC = H // N_CORES       # 2 heads per core
TOK = B * S              # 4096
SUB = 512                # token sub-chunk for projections (= one chunk)
QCH = 512                # attention query chunk
JB = 128                 # attention key block
NSUB = TOK // SUB        # 16
NCH = TOK // QCH         # 8 query chunks
KB = D // 128            # 16 contraction blocks
TPC = QCH // N_CORES     # 64 tokens per (chunk, core) after AllToAll
ROPE_BASE = 10000.0
MASK_NEG = -30000.0

F32 = mybir.dt.float32
F32R = mybir.dt.float32r
BF16 = mybir.dt.bfloat16

_CACHE = {}


def _build_nc():
    dt = F32R
    nc = bacc.Bacc("TRN2", target_bir_lowering=False, debug=False,
                   num_devices=N_CORES)

    xT = nc.dram_tensor("xT", [D, TOK], BF16, kind="ExternalInput")
    wqT = nc.dram_tensor("wqT", [D, HPC * DK], BF16, kind="ExternalInput")
    wkT = nc.dram_tensor("wkT", [D, HPC * DK], BF16, kind="ExternalInput")
    wvT = nc.dram_tensor("wvT", [D, HPC * DK], BF16, kind="ExternalInput")
    # W_o.T in bf16, tiled (eb, p, dl, e'): row = eb*2048 + p*16 + dl
    woT = nc.dram_tensor("woT", [KB * D, DK], BF16, kind="ExternalInput")
    ropeC = nc.dram_tensor("ropeC", [DK, S], F32, kind="ExternalInput")
    ropeS = nc.dram_tensor("ropeS", [DK, S], F32, kind="ExternalInput")
    maskd = nc.dram_tensor("maskd", [JB, 4 * QCH], BF16, kind="ExternalInput")
    # out columns ordered (jc 0..7, t 0..63): global token 512*jc + 64*c + t
    outT = nc.dram_tensor("outT", [D, QCH], BF16, kind="ExternalOutput")

    swap_mask = [i ^ 1 for i in range(32)]

    import contextlib
    with tile.TileContext(nc) as tc:
        with contextlib.ExitStack() as st:
            dram = st.enter_context(
                tc.tile_pool(name="dram", bufs=1, space="DRAM"))
            a2a_in = [dram.tile([N_CORES * HPC * DK, TPC], BF16,
                                name=f"a2ain{j}") for j in range(NCH)]
            a2a_out = [dram.tile([N_CORES * HPC * DK, TPC], BF16,
                                 name=f"a2aout{j}") for j in range(NCH)]

            const = st.enter_context(tc.tile_pool(name="const", bufs=1))
            xpool = st.enter_context(
                tc.tile_pool(name="xpool", bufs=2, side="right"))
            qpool = st.enter_context(
                tc.tile_pool(name="qpool", bufs=2, side="right"))
            kvpool = st.enter_context(
                tc.tile_pool(name="kvpool", bufs=8, side="right"))
            vpool = st.enter_context(
                tc.tile_pool(name="vpool", bufs=15, side="right"))
            ppool = st.enter_context(tc.tile_pool(name="ppool", bufs=7))
            lpool = st.enter_context(tc.tile_pool(name="lpool", bufs=2))
            work = st.enter_context(tc.tile_pool(name="work", bufs=2))
            ypool = st.enter_context(tc.tile_pool(name="ypool", bufs=3))
            ytpool = st.enter_context(tc.tile_pool(name="ytpool", bufs=1))
            stpool = st.enter_context(tc.tile_pool(name="stpool", bufs=1))
            wopool = st.enter_context(
                tc.tile_pool(name="wopool", bufs=14, side="right"))
            ps_proj = st.enter_context(
                tc.tile_pool(name="ps_proj", bufs=3, space="PSUM"))
            ps_st = st.enter_context(
                tc.tile_pool(name="ps_st", bufs=3, space="PSUM"))
            ps_out = st.enter_context(
                tc.tile_pool(name="ps_out", bufs=2, space="PSUM"))

            # chunk-0 xT goes first on its queues so the first projection
            # isn't stuck behind weight DMAs
            def xt_dma(xt, sc):
                KH = KB // 4
                di = None
                for xh in range(4):
                    di = nc.sync.dma_start(
                        xt[:, xh * KH * SUB:(xh + 1) * KH * SUB]
                          .rearrange("p (kb t) -> p kb t", kb=KH),
                        xT.ap()[xh * KH * 128:(xh + 1) * KH * 128,
                                sc * SUB:(sc + 1) * SUB]
                          .rearrange("(kb p) t -> p kb t", p=128))
                return di

            xt_anchor = {}
            store_anchor = {}
            a2a_inst = {}
            xt0 = xpool.tile([128, KB * SUB], BF16, tag="xt", name="xt")
            KH0 = KB // 8
            for xh in range(8):
                di = nc.sync.dma_start(
                    xt0[:, xh * KH0 * SUB:(xh + 1) * KH0 * SUB]
                       .rearrange("p (kb t) -> p kb t", kb=KH0),
                    xT.ap()[xh * KH0 * 128:(xh + 1) * KH0 * 128, 0:SUB]
                      .rearrange("(kb p) t -> p kb t", p=128))
            xt_anchor[0] = di

            # ---- persistent constants in SBUF
            wq_sb = const.tile([128, KB * HPC * DK], BF16)
            wk_sb = const.tile([128, KB * HPC * DK], BF16)
            wv_sb = const.tile([128, KB * HPC * DK], BF16)
            weng = {0: nc.scalar, 1: nc.scalar, 2: nc.gpsimd}
            for ti, (sb_t, dr) in enumerate(
                    ((wq_sb, wqT), (wk_sb, wkT), (wv_sb, wvT))):
                ngrp = 8 if ti == 0 else 4
                GW = KB // ngrp
                for g in range(ngrp):
                    m0 = g * GW * HPC * DK
                    weng[ti].dma_start(
                        sb_t[:, m0:m0 + GW * HPC * DK]
                            .rearrange("p (kb m) -> p kb m", kb=GW),
                        dr.ap()[g * GW * 128:(g + 1) * GW * 128, :]
                          .rearrange("(kb p) m -> p kb m", p=128))
            ropeC_sb = const.tile([DK, S], F32)
            ropeS_sb = const.tile([DK, S], F32)
            maskd_sb = const.tile([JB, 4 * QCH], BF16)
            nc.scalar.dma_start(ropeC_sb[:, :SUB], ropeC[:, :SUB])
            nc.scalar.dma_start(ropeS_sb[:, :SUB], ropeS[:, :SUB])
            nc.scalar.dma_start(ropeC_sb[:, SUB:], ropeC[:, SUB:])
            nc.scalar.dma_start(ropeS_sb[:, SUB:], ropeS[:, SUB:])
            nc.scalar.dma_start(maskd_sb[:], maskd[:])

            def rope_combine(ps_in, out_ap, s0, n):
                """out = ps_in * C + shuffle(ps_in) * S  (RoPE)."""
                qsh = work.tile([128, SUB], F32, tag="qsh", name="qsh")
                t1 = work.tile([128, SUB], BF16, tag="t1", name="t1")
                nc.vector.stream_shuffle(qsh[:, :n], ps_in, swap_mask)
                nc.any.tensor_tensor(
                    t1[:, :n], ps_in, ropeC_sb[:, s0:s0 + n], AluOpType.mult)
                nc.vector.tensor_tensor(
                    qsh[:, :n], qsh[:, :n], ropeS_sb[:, s0:s0 + n],
                    AluOpType.mult)
                nc.any.tensor_tensor(out_ap, t1[:, :n], qsh[:, :n],
                                     AluOpType.add)

            chain_engs = [nc.vector, nc.gpsimd]

            # Wo token-passes -------------------------------------------------
            # pass 0: chunks 0-3 (cols 0:256), interleaved into the second
            # half of the main loop; pass 1: chunks 4-6 (cols 256:448);
            # pass 2: chunk 7 (cols 448:512), reusing resident wo tiles.
            wo_engs = [nc.sync, nc.scalar]

            def load_woeb(eb, ei, anchor=None):
                wo_eb = wopool.tile([128, KB * DK], BF16, tag="wo",
                                    name="wo_eb")
                di = wo_engs[ei % 2].dma_start(
                    wo_eb[:],
                    woT.ap()[eb * D:(eb + 1) * D, :]
                       .rearrange("(p dl) e -> p (dl e)", p=128))
                if anchor is not None:
                    add_dep_helper(di.ins, anchor.ins, sync=True,
                                   reason="wo load after main loop")
                return wo_eb

            def load_yt(yt, chunks, ci0=0, anchors=None, eng=None):
                for ci, jc in enumerate(chunks):
                    di = (eng or nc.scalar).dma_start(
                        yt.rearrange("p (db c) -> p db c", db=KB)
                          [:, :, (ci0 + ci) * TPC:(ci0 + ci + 1) * TPC],
                        a2a_out[jc].rearrange("(db p) t -> p db t",
                                                   p=128))
                    if anchors is not None:
                        add_dep_helper(di.ins, anchors[ci].ins, sync=True,
                                       reason="yt load after its A2A landed")

            def wo_eb_mm(eb, yt, ncols, stage, wo_eb=None, ei=0):
                if wo_eb is None:
                    wo_eb = load_woeb(eb, ei)
                ps_w = ps_proj.tile([128, SUB], F32, tag="proj", name="ps_w")
                for dl in range(KB):
                    nc.tensor.matmul(
                        ps_w[:, :ncols],
                        wo_eb[:, dl * DK:(dl + 1) * DK],
                        yt[:, dl * ncols:(dl + 1) * ncols],
                        start=(dl == 0), stop=(dl == KB - 1))
                nc.any.tensor_copy(
                    stage[:, eb * ncols:(eb + 1) * ncols], ps_w[:, :ncols])

            def store_stage(stage, ncols, col0):
                nc.sync.dma_start(
                    outT.ap().rearrange("(eb p) c -> p eb c", p=128)
                        [:, :, col0:col0 + ncols],
                    stage.rearrange("p (eb c) -> p eb c", eb=KB))

            # ---------------- main pass over the sequence -------------------
            qT_tiles = {}
            kT_tiles = {}
            v_tiles = {}
            yt_p0 = [None]

            for sc in range(NSUB):
                b = sc // (NSUB // B)
                iq = sc % (NSUB // B)
                jc_glob = sc
                s0 = iq * SUB                   # position within batch

                if sc == 0:
                    xt = xt0
                else:
                    xt = xpool.tile([128, KB * SUB], BF16, tag="xt", name="xt")
                    xt_anchor[sc] = xt_dma(xt, sc)

                # ---- q/k projections + rope, both heads
                for h in range(HPC):
                    qT_tiles[h] = qpool.tile([128, QCH], BF16,
                                             tag=f"qT{h}", name="qT")
                    kT_tiles[(b, h, iq)] = kvpool.tile(
                        [128, QCH], BF16, tag="kT", name="kT")
                    for (w_sb, dst) in ((wq_sb, qT_tiles[h]),
                                        (wk_sb, kT_tiles[(b, h, iq)])):
                        psq = ps_proj.tile([128, SUB], F32, tag="proj", name="proj")
                        for kb in range(KB):
                            nc.tensor.matmul(
                                psq[:],
                                w_sb[:, kb * HPC * DK + h * DK:
                                     kb * HPC * DK + (h + 1) * DK],
                                xt[:, kb * SUB:(kb + 1) * SUB],
                                start=(kb == 0), stop=(kb == KB - 1))
                        rope_combine(psq[:], dst[:], s0, SUB)

                # ---- v projection, both heads, bf16 storage
                for tb in range(SUB // 128):
                    jb_b = iq * (SUB // 128) + tb
                    psv = ps_proj.tile([128, HPC * DK], F32, tag="proj",
                                       name="psv")
                    for kb in range(KB):
                        nc.tensor.matmul(
                            psv[:],
                            xt[:, kb * SUB + tb * 128:
                               kb * SUB + (tb + 1) * 128],
                            wv_sb[:, kb * HPC * DK:(kb + 1) * HPC * DK],
                            start=(kb == 0), stop=(kb == KB - 1))
                    vt = vpool.tile([128, HPC * DK], BF16, tag="v", name="vt")
                    nc.any.tensor_copy(vt[:], psv[:])
                    v_tiles[(b, jb_b)] = vt

                # ---- attention for the completed query chunk
                if True:
                    n_j = 4 * iq + 4
                    for h in range(HPC):
                        qT = qT_tiles[h]
                        ps_o = ps_out.tile([128, QCH], F32, tag="att_out", name="att_out")
                        p_tiles = {}

                        def emit_block(j):
                            jck, jr = j // 4, j % 4
                            m = j - 4 * iq
                            # diagonal block m: columns [0, 128m) are fully
                            # masked -> skip them and zero-fill p instead
                            q0 = 128 * m if m > 0 else 0
                            ps_s = ps_st.tile([JB, QCH], F32, tag="st", name="st")
                            nc.tensor.matmul(
                                ps_s[:, q0:],
                                kT_tiles[(b, h, jck)][:, jr * 128:
                                                      (jr + 1) * 128],
                                qT[:, q0:], start=True, stop=True)
                            if m >= 0:           # diagonal block: mask
                                nc.vector.tensor_tensor(
                                    ps_s[:, q0:], ps_s[:, q0:],
                                    maskd_sb[:, m * QCH + q0:(m + 1) * QCH],
                                    AluOpType.add)
                            p_t = ppool.tile([JB, QCH], BF16, tag="p", name="p")
                            if q0 > 0:
                                nc.gpsimd.memset(p_t[:, :q0], 0.0)
                            nc.scalar.activation(
                                p_t[:, q0:], ps_s[:, q0:],
                                mybir.ActivationFunctionType.Exp)
                            p_tiles[j] = p_t

                        # scores/exp run two blocks ahead of the AV matmuls
                        # so PE never waits on the Act engine
                        emit_block(0)
                        if n_j > 1:
                            emit_block(1)
                        # l = sum_k p: two sequential accumulator chains
                        # (even/odd j) spread across DVE/Pool/Act
                        acc = [None, None]
                        n_add = 0
                        cengs = (chain_engs if jc_glob < NCH - 1
                                 else [nc.vector])
                        for j in range(n_j):
                            if j + 2 < n_j:
                                emit_block(j + 2)
                            p_t = p_tiles[j]
                            nc.tensor.matmul(
                                ps_o[:], v_tiles[(b, j)][:, h * DK:
                                                         (h + 1) * DK],
                                p_t[:],
                                start=(j == 0), stop=(j == n_j - 1))
                            c = j % 2
                            if j >= 2:
                                eng = cengs[0]
                                n_add += 1
                                if acc[c] is None:
                                    a_t = lpool.tile([128, QCH], BF16,
                                                     tag="acc", name="acc")
                                    eng.tensor_tensor(
                                        a_t[:], p_tiles[c][:], p_t[:],
                                        AluOpType.add)
                                    acc[c] = a_t
                                else:
                                    eng.tensor_tensor(
                                        acc[c][:], acc[c][:], p_t[:],
                                        AluOpType.add)
                        if acc[0] is None:       # n_j == 4 has j = 0..3
                            acc = [p_tiles[0], p_tiles[1]]
                        p_acc = lpool.tile([128, QCH], BF16, tag="acc2", name="acc2")
                        cengs[0].tensor_tensor(
                            p_acc[:], acc[0][:], acc[1][:], AluOpType.add)
                        # gpsimd all-reduce output is broadcast across
                        # partitions -> no r-broadcast matmul needed
                        l_bc = lpool.tile([128, QCH], F32, tag="lbc", name="lbc")
                        nc.gpsimd.partition_all_reduce(
                            l_bc[:], p_acc[:], channels=128,
                            reduce_op=bass_isa.ReduceOp.add)
                        nc.vector.reciprocal(l_bc[:], l_bc[:])
                        y_sb = ypool.tile([128, QCH], BF16, tag="y", name="y")
                        nc.any.tensor_tensor(y_sb[:], ps_o[:], l_bc[:],
                                             AluOpType.mult)
                        # scatter into the AllToAll source layout:
                        # dst row m*256 + h*128 + p, col t <- src[p, m*64+t]
                        di = nc.gpsimd.dma_start(
                            a2a_in[jc_glob]
                            .rearrange("(m hp) t -> hp m t", m=N_CORES)
                            [h * 128:(h + 1) * 128],
                            y_sb.rearrange("p (m t) -> p m t", m=N_CORES))
                        if h == HPC - 1:
                            store_anchor[jc_glob] = di
                    a2a_inst[jc_glob] = nc.gpsimd.collective_compute(
                        "AllToAll", AluOpType.bypass,
                        replica_groups=[list(range(N_CORES))],
                        ins=[a2a_in[jc_glob].opt()],
                        outs=[a2a_out[jc_glob].opt()])


            # ---- Wo pass A: chunks 0-3 (cols 0:256).  Runs entirely in
            # the shadow of the last AllToAll: those chunks' y landed long
            # ago.  Every staging DMA is anchored so the dataflow
            # scheduler cannot hoist its wait into the main loop.
            yt_p0[0] = ytpool.tile([128, KB * 4 * TPC], BF16,
                                   tag="yt0", name="yt0")
            load_yt(yt_p0[0], range(4),
                    anchors=[store_anchor[c + 1] for c in range(4)])
            yt_p1 = ytpool.tile([128, KB * 3 * TPC], BF16, tag="yt1",
                                name="yt1")
            load_yt(yt_p1, range(4, 7),
                    anchors=[store_anchor[5], store_anchor[6],
                             store_anchor[7]])
            stage_a = stpool.tile([128, KB * 4 * TPC], BF16, tag="sta",
                                  name="sta")
            stage_b1 = stpool.tile([128, KB * 3 * TPC], BF16, tag="stb1",
                                   name="stb1")
            stage_b2 = stpool.tile([128, KB * TPC], BF16, tag="stb2",
                                   name="stb2")
            p0_pref = {eb: load_woeb(eb, eb, anchor=xt_anchor[5])
                       for eb in range(2)}
            for eb in range(KB):
                if eb + 2 < KB:       # stay 2 loads ahead of the matmuls
                    p0_pref[eb + 2] = load_woeb(eb + 2, eb,
                                                anchor=xt_anchor[5])
                wo_eb_mm(eb, yt_p0[0], 4 * TPC, stage_a,
                         wo_eb=p0_pref.pop(eb), ei=eb)

            store_stage(stage_a, 4 * TPC, 0)

            # ---- Wo pass B1: chunks 4-6 (cols 256:448) -- none of this
            # waits on the final AllToAll, so it fills the A2A-7 shadow
            # right after pass A.
            b1_tiles = {eb: load_woeb(eb, eb, anchor=xt_anchor[6])
                        for eb in range(2)}
            for eb in range(KB):
                if eb + 2 < KB:
                    b1_tiles[eb + 2] = load_woeb(eb + 2, eb,
                                                 anchor=xt_anchor[6])
                wo_eb_mm(eb, yt_p1, 3 * TPC, stage_b1,
                         wo_eb=b1_tiles[eb], ei=eb)

            store_stage(stage_b1, 3 * TPC, 4 * TPC)

            # ---- Wo pass B2: chunk 7 (cols 448:512), gated on the final
            # AllToAll.  The last wopool-1 tiles of B1 are still resident:
            # run those first, re-stream the rest.
            yt_p2 = ytpool.tile([128, KB * TPC], BF16, tag="yt2",
                                name="yt2")
            load_yt(yt_p2, range(7, 8),
                    anchors=[store_anchor[7]], eng=nc.gpsimd)
            for eb in range(KB - 13, KB):
                wo_eb_mm(eb, yt_p2, TPC, stage_b2, wo_eb=b1_tiles[eb])
            for ei, eb in enumerate(range(KB - 13)):
                wo_eb_mm(eb, yt_p2, TPC, stage_b2, ei=ei)
            store_stage(stage_b2, TPC, 7 * TPC)

    nc.finalize()
    return nc


# ---------------------------------------------------------------- host
def _host_inputs(x, W_q, W_k, W_v, W_o):
    import ml_dtypes
    bf = np.dtype(ml_dtypes.bfloat16)
    xT = np.ascontiguousarray(
        x.reshape(TOK, D).T).astype(bf)                        # [D, TOK]

    # W_o.T tiled (eb, p, dl, e'): row eb*2048 + p*16 + dl, col e'
    woT = np.ascontiguousarray(
        W_o.T.reshape(KB, 128, KB, DK).transpose(2, 1, 0, 3)
        .reshape(KB * D, DK)).astype(bf)

    # RoPE tables, expanded to [DK, S] with interleaved pairs; the sign
    # table carries -sin on even rows, +sin on odd rows.
    i = np.arange(0, DK, 2, dtype=np.float32)
    theta = 1.0 / (ROPE_BASE ** (i / DK))                      # [64]
    pos = np.arange(S, dtype=np.float32)
    freqs = pos[:, None] * theta[None, :]                      # [S, 64]
    cos_t, sin_t = np.cos(freqs), np.sin(freqs)
    ropeC = np.empty((DK, S), np.float32)
    ropeS = np.empty((DK, S), np.float32)
    ropeC[0::2] = cos_t.T
    ropeC[1::2] = cos_t.T
    ropeS[0::2] = -sin_t.T
    ropeS[1::2] = sin_t.T

    # diagonal causal masks: block m (of the 4 key blocks overlapping a
    # 512-query chunk) keeps kk <= qq - 128*m
    kk = np.arange(JB)[:, None]
    qq = np.arange(QCH)[None, :]
    maskd = np.concatenate(
        [np.where(kk <= qq - 128 * m, 0.0, MASK_NEG).astype(np.float32)
         for m in range(4)], axis=1).astype(bf)                # [128, 4*512]

    scale = 1.0 / np.sqrt(np.float32(DK))
    in_maps = []
    for c in range(N_CORES):
        rows = slice(c * HPC * DK, (c + 1) * HPC * DK)
        in_maps.append({
            "xT": xT,
            "wqT": np.ascontiguousarray(
                (W_q[rows] * scale).T).astype(bf),
            "wkT": np.ascontiguousarray(W_k[rows].T).astype(bf),
            "wvT": np.ascontiguousarray(W_v[rows].T).astype(bf),
            "woT": woT,
            "ropeC": ropeC,
            "ropeS": ropeS,
            "maskd": maskd,
        })
    return in_maps


def kernel(x, W_q, W_k, W_v, W_o):
    x = np.asarray(x, dtype=np.float32)
    W_q = np.asarray(W_q, dtype=np.float32)
    W_k = np.asarray(W_k, dtype=np.float32)
    W_v = np.asarray(W_v, dtype=np.float32)
    W_o = np.asarray(W_o, dtype=np.float32)

    if "nc" not in _CACHE:
        _CACHE["nc"] = _build_nc()
    nc = _CACHE["nc"]

    in_maps = _host_inputs(x, W_q, W_k, W_v, W_o)
    res = bass_utils.run_bass_kernel_spmd(
        nc, in_maps, core_ids=list(range(N_CORES)))

    # outT per core: [D, 512] f32; col 64*jc + t -> token 512*jc + 64*c + t
    out_T = np.empty((D, TOK), np.float32)
    for c in range(N_CORES):
        cols = (QCH * np.arange(NCH)[:, None] + TPC * c
                + np.arange(TPC)[None, :]).ravel()
        out_T[:, cols] = res.results[c]["outT"].astype(np.float32)
    return np.ascontiguousarray(out_T.T).reshape(B, S, D).astype(np.float32)


# revision 60
# speedup vs baseline: 1.4642x; 1.0059x over previous
"""Causal self-attention with RoPE on 8 Trainium2 NeuronCores.

Sharding: tensor-parallel over heads (16 heads -> 2 per core) for the
QKV projections, RoPE and attention.  The attention output is
re-sharded token-wise with one small AllToAll per 512-token chunk
(bf16 payload), so each core ends up with the full 2048-dim y vector
for 64 tokens of every chunk; the output projection then runs
token-parallel with no all-reduce.

Single pass over the sequence: both heads' q/k/v are produced from one
streaming of x (the baseline used one pass per head and loaded x
twice).

Key engine-placement choices (cost-model driven):
  - softmax denominator l = sum_k p is NOT computed with PE matmuls
    (those cost as much as the score matmuls); instead p-blocks are
    summed with a shallow tree of tensor_tensor adds spread across
    DVE/Act/Pool and one gpsimd.partition_all_reduce, whose output is
    already broadcast across partitions (also kills the r-broadcast
    matmul of the baseline).
  - y, v and W_o are bf16 (halves the collective payload and the Wo
    weight traffic; bf16 matmuls run at full PE rate).
  - W_o streams through SBUF per token-pass; the last pass reuses the
    still-resident tiles of the previous pass for its first blocks.

Shapes (hardcoded): x [2, 2048, 2048], W_* [2048, 2048], 16 heads,
d_k = 128, fp32 in/out.
"""

import sys

for _p in ("/opt/trn_rl_repo", "/opt/pypackages"):
    if _p not in sys.path:
        sys.path.insert(0, _p)

import numpy as np

import concourse.bass as bass
import concourse.bacc as bacc
import concourse.mybir as mybir
import concourse.tile as tile
from concourse import bass_utils
from concourse import bass_isa
from concourse.alu_op_type import AluOpType
from concourse.tile import add_dep_helper

# ---------------------------------------------------------------- config
N_CORES = 8
B, S, D = 2, 2048, 2048
H = 16
DK = D // H              # 128
HPC = H // N_CORES       # 2 heads per core
TOK = B * S              # 4096
SUB = 512                # token sub-chunk for projections (= one chunk)
QCH = 512                # attention query chunk
JB = 128                 # attention key block
NSUB = TOK // SUB        # 16
NCH = TOK // QCH         # 8 query chunks
KB = D // 128            # 16 contraction blocks
TPC = QCH // N_CORES     # 64 tokens per (chunk, core) after AllToAll
ROPE_BASE = 10000.0
MASK_NEG = -30000.0

F32 = mybir.dt.float32
F32R = mybir.dt.float32r
BF16 = mybir.dt.bfloat16

_CACHE = {}


def _build_nc():
    dt = F32R
    nc = bacc.Bacc("TRN2", target_bir_lowering=False, debug=False,
                   num_devices=N_CORES)

    xT = nc.dram_tensor("xT", [D, TOK], BF16, kind="ExternalInput")
    wqT = nc.dram_tensor("wqT", [D, HPC * DK], BF16, kind="ExternalInput")
    wkT = nc.dram_tensor("wkT", [D, HPC * DK], BF16, kind="ExternalInput")
    wvT = nc.dram_tensor("wvT", [D, HPC * DK], BF16, kind="ExternalInput")
    # W_o.T in bf16, tiled (eb, p, dl, e'): row = eb*2048 + p*16 + dl
    woT = nc.dram_tensor("woT", [KB * D, DK], BF16, kind="ExternalInput")
    ropeC = nc.dram_tensor("ropeC", [DK, S], F32, kind="ExternalInput")
    ropeS = nc.dram_tensor("ropeS", [DK, S], F32, kind="ExternalInput")
    maskd = nc.dram_tensor("maskd", [JB, 4 * QCH], BF16, kind="ExternalInput")
    # out columns ordered (jc 0..7, t 0..63): global token 512*jc + 64*c + t
    outT = nc.dram_tensor("outT", [D, QCH], BF16, kind="ExternalOutput")

    swap_mask = [i ^ 1 for i in range(32)]

    import contextlib
    with tile.TileContext(nc) as tc:
        with contextlib.ExitStack() as st:
            dram = st.enter_context(
                tc.tile_pool(name="dram", bufs=1, space="DRAM"))
            a2a_in = [dram.tile([N_CORES * HPC * DK, TPC], BF16,
                                name=f"a2ain{j}") for j in range(NCH)]
            a2a_out = [dram.tile([N_CORES * HPC * DK, TPC], BF16,
                                 name=f"a2aout{j}") for j in range(NCH)]

            const = st.enter_context(tc.tile_pool(name="const", bufs=1))
            xpool = st.enter_context(
                tc.tile_pool(name="xpool", bufs=2, side="right"))
            qpool = st.enter_context(
                tc.tile_pool(name="qpool", bufs=2, side="right"))
            kvpool = st.enter_context(
                tc.tile_pool(name="kvpool", bufs=8, side="right"))
            vpool = st.enter_context(
                tc.tile_pool(name="vpool", bufs=15, side="right"))
            ppool = st.enter_context(tc.tile_pool(name="ppool", bufs=7))
            lpool = st.enter_context(tc.tile_pool(name="lpool", bufs=2))
            work = st.enter_context(tc.tile_pool(name="work", bufs=2))
            ypool = st.enter_context(tc.tile_pool(name="ypool", bufs=3))
            ytpool = st.enter_context(tc.tile_pool(name="ytpool", bufs=1))
            stpool = st.enter_context(tc.tile_pool(name="stpool", bufs=1))
            wopool = st.enter_context(
                tc.tile_pool(name="wopool", bufs=14, side="right"))
            ps_proj = st.enter_context(
                tc.tile_pool(name="ps_proj", bufs=3, space="PSUM"))
            ps_st = st.enter_context(
                tc.tile_pool(name="ps_st", bufs=3, space="PSUM"))
            ps_out = st.enter_context(
                tc.tile_pool(name="ps_out", bufs=2, space="PSUM"))

            # chunk-0 xT goes first on its queues so the first projection
            # isn't stuck behind weight DMAs
            def xt_dma(xt, sc):
                KH = KB // 4
                di = None
                for xh in range(4):
                    di = nc.sync.dma_start(
                        xt[:, xh * KH * SUB:(xh + 1) * KH * SUB]
                          .rearrange("p (kb t) -> p kb t", kb=KH),
                        xT.ap()[xh * KH * 128:(xh + 1) * KH * 128,
                                sc * SUB:(sc + 1) * SUB]
                          .rearrange("(kb p) t -> p kb t", p=128))
                return di

            xt_anchor = {}
            store_anchor = {}
            a2a_inst = {}
            xt0 = xpool.tile([128, KB * SUB], BF16, tag="xt", name="xt")
            KH0 = KB // 8
            for xh in range(8):
                di = nc.sync.dma_start(
                    xt0[:, xh * KH0 * SUB:(xh + 1) * KH0 * SUB]
                       .rearrange("p (kb t) -> p kb t", kb=KH0),
                    xT.ap()[xh * KH0 * 128:(xh + 1) * KH0 * 128, 0:SUB]
                      .rearrange("(kb p) t -> p kb t", p=128))
            xt_anchor[0] = di

            # ---- persistent constants in SBUF
            wq_sb = const.tile([128, KB * HPC * DK], BF16)
            wk_sb = const.tile([128, KB * HPC * DK], BF16)
            wv_sb = const.tile([128, KB * HPC * DK], BF16)
            weng = {0: nc.scalar, 1: nc.scalar, 2: nc.gpsimd}
            for ti, (sb_t, dr) in enumerate(
                    ((wq_sb, wqT), (wk_sb, wkT), (wv_sb, wvT))):
                ngrp = 8 if ti == 0 else 4
                GW = KB // ngrp
                for g in range(ngrp):
                    m0 = g * GW * HPC * DK
                    weng[ti].dma_start(
                        sb_t[:, m0:m0 + GW * HPC * DK]
                            .rearrange("p (kb m) -> p kb m", kb=GW),
                        dr.ap()[g * GW * 128:(g + 1) * GW * 128, :]
                          .rearrange("(kb p) m -> p kb m", p=128))
            ropeC_sb = const.tile([DK, S], F32)
            ropeS_sb = const.tile([DK, S], F32)
            maskd_sb = const.tile([JB, 4 * QCH], BF16)
            nc.scalar.dma_start(ropeC_sb[:, :SUB], ropeC[:, :SUB])
            nc.scalar.dma_start(ropeS_sb[:, :SUB], ropeS[:, :SUB])
            nc.scalar.dma_start(ropeC_sb[:, SUB:], ropeC[:, SUB:])
            nc.scalar.dma_start(ropeS_sb[:, SUB:], ropeS[:, SUB:])
            nc.scalar.dma_start(maskd_sb[:], maskd[:])

            def rope_combine(ps_in, out_ap, s0, n):
                """out = ps_in * C + shuffle(ps_in) * S  (RoPE)."""
                qsh = work.tile([128, SUB], F32, tag="qsh", name="qsh")
                t1 = work.tile([128, SUB], BF16, tag="t1", name="t1")
                nc.vector.stream_shuffle(qsh[:, :n], ps_in, swap_mask)
                nc.any.tensor_tensor(
                    t1[:, :n], ps_in, ropeC_sb[:, s0:s0 + n], AluOpType.mult)
                nc.vector.tensor_tensor(
                    qsh[:, :n], qsh[:, :n], ropeS_sb[:, s0:s0 + n],
                    AluOpType.mult)
                nc.any.tensor_tensor(out_ap, t1[:, :n], qsh[:, :n],
                                     AluOpType.add)

            chain_engs = [nc.vector, nc.gpsimd]

            # Wo token-passes -------------------------------------------------
            # pass 0: chunks 0-3 (cols 0:256), interleaved into the second
            # half of the main loop; pass 1: chunks 4-6 (cols 256:448);
            # pass 2: chunk 7 (cols 448:512), reusing resident wo tiles.
            wo_engs = [nc.sync, nc.scalar]

            def load_woeb(eb, ei, anchor=None):
                wo_eb = wopool.tile([128, KB * DK], BF16, tag="wo",
                                    name="wo_eb")
                di = wo_engs[ei % 2].dma_start(
                    wo_eb[:],
                    woT.ap()[eb * D:(eb + 1) * D, :]
                       .rearrange("(p dl) e -> p (dl e)", p=128))
                if anchor is not None:
                    add_dep_helper(di.ins, anchor.ins, sync=True,
                                   reason="wo load after main loop")
                return wo_eb

            def load_yt(yt, chunks, ci0=0, anchors=None, eng=None):
                for ci, jc in enumerate(chunks):
                    di = (eng or nc.scalar).dma_start(
                        yt.rearrange("p (db c) -> p db c", db=KB)
                          [:, :, (ci0 + ci) * TPC:(ci0 + ci + 1) * TPC],
                        a2a_out[jc].rearrange("(db p) t -> p db t",
                                                   p=128))
                    if anchors is not None:
                        add_dep_helper(di.ins, anchors[ci].ins, sync=True,
                                       reason="yt load after its A2A landed")

            def wo_eb_mm(eb, yt, ncols, stage, wo_eb=None, ei=0):
                if wo_eb is None:
                    wo_eb = load_woeb(eb, ei)
                ps_w = ps_proj.tile([128, SUB], F32, tag="proj", name="ps_w")
                for dl in range(KB):
                    nc.tensor.matmul(
                        ps_w[:, :ncols],
                        wo_eb[:, dl * DK:(dl + 1) * DK],
                        yt[:, dl * ncols:(dl + 1) * ncols],
                        start=(dl == 0), stop=(dl == KB - 1))
                nc.any.tensor_copy(
                    stage[:, eb * ncols:(eb + 1) * ncols], ps_w[:, :ncols])

            def store_stage(stage, ncols, col0):
                nc.sync.dma_start(
                    outT.ap().rearrange("(eb p) c -> p eb c", p=128)
                        [:, :, col0:col0 + ncols],
                    stage.rearrange("p (eb c) -> p eb c", eb=KB))

            # ---------------- main pass over the sequence -------------------
            qT_tiles = {}
            kT_tiles = {}
            v_tiles = {}
            yt_p0 = [None]

            for sc in range(NSUB):
                b = sc // (NSUB // B)
                iq = sc % (NSUB // B)
                jc_glob = sc
                s0 = iq * SUB                   # position within batch

                if sc == 0:
                    xt = xt0
                else:
                    xt = xpool.tile([128, KB * SUB], BF16, tag="xt", name="xt")
                    xt_anchor[sc] = xt_dma(xt, sc)

                # ---- q/k projections + rope, both heads
                for h in range(HPC):
                    qT_tiles[h] = qpool.tile([128, QCH], BF16,
                                             tag=f"qT{h}", name="qT")
                    kT_tiles[(b, h, iq)] = kvpool.tile(
                        [128, QCH], BF16, tag="kT", name="kT")
                    for (w_sb, dst) in ((wq_sb, qT_tiles[h]),
                                        (wk_sb, kT_tiles[(b, h, iq)])):
                        psq = ps_proj.tile([128, SUB], F32, tag="proj", name="proj")
                        for kb in range(KB):
                            nc.tensor.matmul(
                                psq[:],
                                w_sb[:, kb * HPC * DK + h * DK:
                                     kb * HPC * DK + (h + 1) * DK],
                                xt[:, kb * SUB:(kb + 1) * SUB],
                                start=(kb == 0), stop=(kb == KB - 1))
                        rope_combine(psq[:], dst[:], s0, SUB)

                # ---- v projection, both heads, bf16 storage
                for tb in range(SUB // 128):
                    jb_b = iq * (SUB // 128) + tb
                    psv = ps_proj.tile([128, HPC * DK], F32, tag="proj",
                                       name="psv")
                    for kb in range(KB):
                        nc.tensor.matmul(
                            psv[:],
                            xt[:, kb * SUB + tb * 128:
                               kb * SUB + (tb + 1) * 128],
                            wv_sb[:, kb * HPC * DK:(kb + 1) * HPC * DK],
                            start=(kb == 0), stop=(kb == KB - 1))
                    vt = vpool.tile([128, HPC * DK], BF16, tag="v", name="vt")
                    nc.any.tensor_copy(vt[:], psv[:])
                    v_tiles[(b, jb_b)] = vt

                # ---- attention for the completed query chunk
                if True:
                    n_j = 4 * iq + 4
                    for h in range(HPC):
                        qT = qT_tiles[h]
                        ps_o = ps_out.tile([128, QCH], F32, tag="att_out", name="att_out")
                        p_tiles = {}

                        def emit_block(j):
                            jck, jr = j // 4, j % 4
                            m = j - 4 * iq
                            # diagonal block m: columns [0, 128m) are fully
                            # masked -> skip them and zero-fill p instead
                            q0 = 128 * m if m > 0 else 0
                            ps_s = ps_st.tile([JB, QCH], F32, tag="st", name="st")
                            nc.tensor.matmul(
                                ps_s[:, q0:],
                                kT_tiles[(b, h, jck)][:, jr * 128:
                                                      (jr + 1) * 128],
                                qT[:, q0:], start=True, stop=True)
                            if m >= 0:           # diagonal block: mask
                                # full width: the un-written [0,q0) region
                                # holds bounded stale scores; -30000 sends
                                # them to exp()==0, zero-filling p for free
                                nc.vector.tensor_tensor(
                                    ps_s[:], ps_s[:],
                                    maskd_sb[:, m * QCH:(m + 1) * QCH],
                                    AluOpType.add)
                            p_t = ppool.tile([JB, QCH], BF16, tag="p", name="p")
                            nc.scalar.activation(
                                p_t[:], ps_s[:],
                                mybir.ActivationFunctionType.Exp)
                            p_tiles[j] = p_t

                        # scores/exp run two blocks ahead of the AV matmuls
                        # so PE never waits on the Act engine
                        emit_block(0)
                        if n_j > 1:
                            emit_block(1)
                        # l = sum_k p: two sequential accumulator chains
                        # (even/odd j) spread across DVE/Pool/Act
                        acc = [None, None]
                        n_add = 0
                        cengs = (chain_engs if jc_glob < NCH - 1
                                 else [nc.vector])
                        for j in range(n_j):
                            if j + 2 < n_j:
                                emit_block(j + 2)
                            p_t = p_tiles[j]
                            nc.tensor.matmul(
                                ps_o[:], v_tiles[(b, j)][:, h * DK:
                                                         (h + 1) * DK],
                                p_t[:],
                                start=(j == 0), stop=(j == n_j - 1))
                            c = j % 2
                            if j >= 2:
                                eng = cengs[0]
                                n_add += 1
                                if acc[c] is None:
                                    a_t = lpool.tile([128, QCH], BF16,
                                                     tag="acc", name="acc")
                                    eng.tensor_tensor(
                                        a_t[:], p_tiles[c][:], p_t[:],
                                        AluOpType.add)
                                    acc[c] = a_t
                                else:
                                    eng.tensor_tensor(
                                        acc[c][:], acc[c][:], p_t[:],
                                        AluOpType.add)
                        if acc[0] is None:       # n_j == 4 has j = 0..3
                            acc = [p_tiles[0], p_tiles[1]]
                        p_acc = lpool.tile([128, QCH], BF16, tag="acc2", name="acc2")
                        cengs[0].tensor_tensor(
                            p_acc[:], acc[0][:], acc[1][:], AluOpType.add)
                        # gpsimd all-reduce output is broadcast across
                        # partitions -> no r-broadcast matmul needed
                        l_bc = lpool.tile([128, QCH], F32, tag="lbc", name="lbc")
                        nc.gpsimd.partition_all_reduce(
                            l_bc[:], p_acc[:], channels=128,
                            reduce_op=bass_isa.ReduceOp.add)
                        nc.vector.reciprocal(l_bc[:], l_bc[:])
                        y_sb = ypool.tile([128, QCH], BF16, tag="y", name="y")
                        nc.any.tensor_tensor(y_sb[:], ps_o[:], l_bc[:],
                                             AluOpType.mult)
                        # scatter into the AllToAll source layout:
                        # dst row m*256 + h*128 + p, col t <- src[p, m*64+t]
                        di = nc.gpsimd.dma_start(
                            a2a_in[jc_glob]
                            .rearrange("(m hp) t -> hp m t", m=N_CORES)
                            [h * 128:(h + 1) * 128],
                            y_sb.rearrange("p (m t) -> p m t", m=N_CORES))
                        if h == HPC - 1:
                            store_anchor[jc_glob] = di
                    a2a_inst[jc_glob] = nc.gpsimd.collective_compute(
                        "AllToAll", AluOpType.bypass,
                        replica_groups=[list(range(N_CORES))],
                        ins=[a2a_in[jc_glob].opt()],
                        outs=[a2a_out[jc_glob].opt()])


            # ---- Wo pass A: chunks 0-3 (cols 0:256).  Runs entirely in
            # the shadow of the last AllToAll: those chunks' y landed long
            # ago.  Every staging DMA is anchored so the dataflow
            # scheduler cannot hoist its wait into the main loop.
            yt_p0[0] = ytpool.tile([128, KB * 4 * TPC], BF16,
                                   tag="yt0", name="yt0")
            load_yt(yt_p0[0], range(4),
                    anchors=[store_anchor[c + 1] for c in range(4)])
            yt_p1 = ytpool.tile([128, KB * 3 * TPC], BF16, tag="yt1",
                                name="yt1")
            load_yt(yt_p1, range(4, 7),
                    anchors=[store_anchor[5], store_anchor[6],
                             store_anchor[7]])
            stage_a = stpool.tile([128, KB * 4 * TPC], BF16, tag="sta",
                                  name="sta")
            stage_b1 = stpool.tile([128, KB * 3 * TPC], BF16, tag="stb1",
                                   name="stb1")
            stage_b2 = stpool.tile([128, KB * TPC], BF16, tag="stb2",
                                   name="stb2")
            p0_pref = {eb: load_woeb(eb, eb, anchor=xt_anchor[5])
                       for eb in range(2)}
            for eb in range(KB):
                if eb + 2 < KB:       # stay 2 loads ahead of the matmuls
                    p0_pref[eb + 2] = load_woeb(eb + 2, eb,
                                                anchor=xt_anchor[5])
                wo_eb_mm(eb, yt_p0[0], 4 * TPC, stage_a,
                         wo_eb=p0_pref.pop(eb), ei=eb)

            store_stage(stage_a, 4 * TPC, 0)

            # ---- Wo pass B1: chunks 4-6 (cols 256:448) -- none of this
            # waits on the final AllToAll, so it fills the A2A-7 shadow
            # right after pass A.
            b1_tiles = {eb: load_woeb(eb, eb, anchor=xt_anchor[6])
                        for eb in range(2)}
            for eb in range(KB):
                if eb + 2 < KB:
                    b1_tiles[eb + 2] = load_woeb(eb + 2, eb,
                                                 anchor=xt_anchor[6])
                wo_eb_mm(eb, yt_p1, 3 * TPC, stage_b1,
                         wo_eb=b1_tiles[eb], ei=eb)

            store_stage(stage_b1, 3 * TPC, 4 * TPC)

            # ---- Wo pass B2: chunk 7 (cols 448:512), gated on the final
            # AllToAll.  The last wopool-1 tiles of B1 are still resident:
            # run those first, re-stream the rest.
            yt_p2 = ytpool.tile([128, KB * TPC], BF16, tag="yt2",
                                name="yt2")
            load_yt(yt_p2, range(7, 8),
                    anchors=[store_anchor[7]], eng=nc.gpsimd)
            for eb in range(KB - 13, KB):
                wo_eb_mm(eb, yt_p2, TPC, stage_b2, wo_eb=b1_tiles[eb])
            for ei, eb in enumerate(range(KB - 13)):
                wo_eb_mm(eb, yt_p2, TPC, stage_b2, ei=ei)
            # rows 3..15 (resident-weight blocks) finish first: store them
            # while the three re-streamed blocks compute, leaving only a
            # tiny final store on the critical path
            nc.sync.dma_start(
                outT.ap().rearrange("(eb p) c -> p eb c", p=128)
                    [:, KB - 13:, 7 * TPC:],
                stage_b2.rearrange("p (eb c) -> p eb c", eb=KB)
                        [:, KB - 13:])
            nc.sync.dma_start(
                outT.ap().rearrange("(eb p) c -> p eb c", p=128)
                    [:, :KB - 13, 7 * TPC:],
                stage_b2.rearrange("p (eb c) -> p eb c", eb=KB)
                        [:, :KB - 13])

    nc.finalize()
    return nc


# ---------------------------------------------------------------- host
def _host_inputs(x, W_q, W_k, W_v, W_o):
    import ml_dtypes
    bf = np.dtype(ml_dtypes.bfloat16)
    xT = np.ascontiguousarray(
        x.reshape(TOK, D).T).astype(bf)                        # [D, TOK]

    # W_o.T tiled (eb, p, dl, e'): row eb*2048 + p*16 + dl, col e'
    woT = np.ascontiguousarray(
        W_o.T.reshape(KB, 128, KB, DK).transpose(2, 1, 0, 3)
        .reshape(KB * D, DK)).astype(bf)

    # RoPE tables, expanded to [DK, S] with interleaved pairs; the sign
    # table carries -sin on even rows, +sin on odd rows.
    i = np.arange(0, DK, 2, dtype=np.float32)
    theta = 1.0 / (ROPE_BASE ** (i / DK))                      # [64]
    pos = np.arange(S, dtype=np.float32)
    freqs = pos[:, None] * theta[None, :]                      # [S, 64]
    cos_t, sin_t = np.cos(freqs), np.sin(freqs)
    ropeC = np.empty((DK, S), np.float32)
    ropeS = np.empty((DK, S), np.float32)
    ropeC[0::2] = cos_t.T
    ropeC[1::2] = cos_t.T
    ropeS[0::2] = -sin_t.T
    ropeS[1::2] = sin_t.T

    # diagonal causal masks: block m (of the 4 key blocks overlapping a
    # 512-query chunk) keeps kk <= qq - 128*m
    kk = np.arange(JB)[:, None]
    qq = np.arange(QCH)[None, :]
    maskd = np.concatenate(
        [np.where(kk <= qq - 128 * m, 0.0, MASK_NEG).astype(np.float32)
         for m in range(4)], axis=1).astype(bf)                # [128, 4*512]

    scale = 1.0 / np.sqrt(np.float32(DK))
    in_maps = []
    for c in range(N_CORES):
        rows = slice(c * HPC * DK, (c + 1) * HPC * DK)
        in_maps.append({
            "xT": xT,
            "wqT": np.ascontiguousarray(
                (W_q[rows] * scale).T).astype(bf),
            "wkT": np.ascontiguousarray(W_k[rows].T).astype(bf),
            "wvT": np.ascontiguousarray(W_v[rows].T).astype(bf),
            "woT": woT,
            "ropeC": ropeC,
            "ropeS": ropeS,
            "maskd": maskd,
        })
    return in_maps


def kernel(x, W_q, W_k, W_v, W_o):
    x = np.asarray(x, dtype=np.float32)
    W_q = np.asarray(W_q, dtype=np.float32)
    W_k = np.asarray(W_k, dtype=np.float32)
    W_v = np.asarray(W_v, dtype=np.float32)
    W_o = np.asarray(W_o, dtype=np.float32)

    if "nc" not in _CACHE:
        _CACHE["nc"] = _build_nc()
    nc = _CACHE["nc"]

    in_maps = _host_inputs(x, W_q, W_k, W_v, W_o)
    res = bass_utils.run_bass_kernel_spmd(
        nc, in_maps, core_ids=list(range(N_CORES)))

    # outT per core: [D, 512] f32; col 64*jc + t -> token 512*jc + 64*c + t
    out_T = np.empty((D, TOK), np.float32)
    for c in range(N_CORES):
        cols = (QCH * np.arange(NCH)[:, None] + TPC * c
                + np.arange(TPC)[None, :]).ravel()
        out_T[:, cols] = res.results[c]["outT"].astype(np.float32)
    return np.ascontiguousarray(out_T.T).reshape(B, S, D).astype(np.float32)


# revision 61
# speedup vs baseline: 1.4656x; 1.0009x over previous
"""Causal self-attention with RoPE on 8 Trainium2 NeuronCores.

Sharding: tensor-parallel over heads (16 heads -> 2 per core) for the
QKV projections, RoPE and attention.  The attention output is
re-sharded token-wise with one small AllToAll per 512-token chunk
(bf16 payload), so each core ends up with the full 2048-dim y vector
for 64 tokens of every chunk; the output projection then runs
token-parallel with no all-reduce.

Single pass over the sequence: both heads' q/k/v are produced from one
streaming of x (the baseline used one pass per head and loaded x
twice).

Key engine-placement choices (cost-model driven):
  - softmax denominator l = sum_k p is NOT computed with PE matmuls
    (those cost as much as the score matmuls); instead p-blocks are
    summed with a shallow tree of tensor_tensor adds spread across
    DVE/Act/Pool and one gpsimd.partition_all_reduce, whose output is
    already broadcast across partitions (also kills the r-broadcast
    matmul of the baseline).
  - y, v and W_o are bf16 (halves the collective payload and the Wo
    weight traffic; bf16 matmuls run at full PE rate).
  - W_o streams through SBUF per token-pass; the last pass reuses the
    still-resident tiles of the previous pass for its first blocks.

Shapes (hardcoded): x [2, 2048, 2048], W_* [2048, 2048], 16 heads,
d_k = 128, fp32 in/out.
"""

import sys

for _p in ("/opt/trn_rl_repo", "/opt/pypackages"):
    if _p not in sys.path:
        sys.path.insert(0, _p)

import numpy as np

import concourse.bass as bass
import concourse.bacc as bacc
import concourse.mybir as mybir
import concourse.tile as tile
from concourse import bass_utils
from concourse import bass_isa
from concourse.alu_op_type import AluOpType
from concourse.tile import add_dep_helper

# ---------------------------------------------------------------- config
N_CORES = 8
B, S, D = 2, 2048, 2048
H = 16
DK = D // H              # 128
HPC = H // N_CORES       # 2 heads per core
TOK = B * S              # 4096
SUB = 512                # token sub-chunk for projections (= one chunk)
QCH = 512                # attention query chunk
JB = 128                 # attention key block
NSUB = TOK // SUB        # 16
NCH = TOK // QCH         # 8 query chunks
KB = D // 128            # 16 contraction blocks
TPC = QCH // N_CORES     # 64 tokens per (chunk, core) after AllToAll
ROPE_BASE = 10000.0
MASK_NEG = -30000.0

F32 = mybir.dt.float32
F32R = mybir.dt.float32r
BF16 = mybir.dt.bfloat16

_CACHE = {}


def _build_nc():
    dt = F32R
    nc = bacc.Bacc("TRN2", target_bir_lowering=False, debug=False,
                   num_devices=N_CORES)

    xT = nc.dram_tensor("xT", [D, TOK], BF16, kind="ExternalInput")
    wqT = nc.dram_tensor("wqT", [D, HPC * DK], BF16, kind="ExternalInput")
    wkT = nc.dram_tensor("wkT", [D, HPC * DK], BF16, kind="ExternalInput")
    wvT = nc.dram_tensor("wvT", [D, HPC * DK], BF16, kind="ExternalInput")
    # W_o.T in bf16, tiled (eb, p, dl, e'): row = eb*2048 + p*16 + dl
    woT = nc.dram_tensor("woT", [KB * D, DK], BF16, kind="ExternalInput")
    ropeC = nc.dram_tensor("ropeC", [DK, S], BF16, kind="ExternalInput")
    ropeS = nc.dram_tensor("ropeS", [DK, S], BF16, kind="ExternalInput")
    maskd = nc.dram_tensor("maskd", [JB, 4 * QCH], BF16, kind="ExternalInput")
    # out columns ordered (jc 0..7, t 0..63): global token 512*jc + 64*c + t
    outT = nc.dram_tensor("outT", [D, QCH], BF16, kind="ExternalOutput")

    swap_mask = [i ^ 1 for i in range(32)]

    import contextlib
    with tile.TileContext(nc) as tc:
        with contextlib.ExitStack() as st:
            dram = st.enter_context(
                tc.tile_pool(name="dram", bufs=1, space="DRAM"))
            a2a_in = [dram.tile([N_CORES * HPC * DK, TPC], BF16,
                                name=f"a2ain{j}") for j in range(NCH)]
            a2a_out = [dram.tile([N_CORES * HPC * DK, TPC], BF16,
                                 name=f"a2aout{j}") for j in range(NCH)]

            const = st.enter_context(tc.tile_pool(name="const", bufs=1))
            xpool = st.enter_context(
                tc.tile_pool(name="xpool", bufs=2, side="right"))
            qpool = st.enter_context(
                tc.tile_pool(name="qpool", bufs=2, side="right"))
            kvpool = st.enter_context(
                tc.tile_pool(name="kvpool", bufs=8, side="right"))
            vpool = st.enter_context(
                tc.tile_pool(name="vpool", bufs=15, side="right"))
            ppool = st.enter_context(tc.tile_pool(name="ppool", bufs=7))
            lpool = st.enter_context(tc.tile_pool(name="lpool", bufs=2))
            work = st.enter_context(tc.tile_pool(name="work", bufs=2))
            ypool = st.enter_context(tc.tile_pool(name="ypool", bufs=3))
            ytpool = st.enter_context(tc.tile_pool(name="ytpool", bufs=1))
            stpool = st.enter_context(tc.tile_pool(name="stpool", bufs=1))
            wopool = st.enter_context(
                tc.tile_pool(name="wopool", bufs=14, side="right"))
            ps_proj = st.enter_context(
                tc.tile_pool(name="ps_proj", bufs=3, space="PSUM"))
            ps_st = st.enter_context(
                tc.tile_pool(name="ps_st", bufs=3, space="PSUM"))
            ps_out = st.enter_context(
                tc.tile_pool(name="ps_out", bufs=2, space="PSUM"))

            # chunk-0 xT goes first on its queues so the first projection
            # isn't stuck behind weight DMAs
            def xt_dma(xt, sc):
                KH = KB // 4
                di = None
                for xh in range(4):
                    di = nc.sync.dma_start(
                        xt[:, xh * KH * SUB:(xh + 1) * KH * SUB]
                          .rearrange("p (kb t) -> p kb t", kb=KH),
                        xT.ap()[xh * KH * 128:(xh + 1) * KH * 128,
                                sc * SUB:(sc + 1) * SUB]
                          .rearrange("(kb p) t -> p kb t", p=128))
                return di

            xt_anchor = {}
            store_anchor = {}
            a2a_inst = {}
            xt0 = xpool.tile([128, KB * SUB], BF16, tag="xt", name="xt")
            KH0 = KB // 8
            for xh in range(8):
                di = nc.sync.dma_start(
                    xt0[:, xh * KH0 * SUB:(xh + 1) * KH0 * SUB]
                       .rearrange("p (kb t) -> p kb t", kb=KH0),
                    xT.ap()[xh * KH0 * 128:(xh + 1) * KH0 * 128, 0:SUB]
                      .rearrange("(kb p) t -> p kb t", p=128))
            xt_anchor[0] = di

            # ---- persistent constants in SBUF
            wq_sb = const.tile([128, KB * HPC * DK], BF16)
            wk_sb = const.tile([128, KB * HPC * DK], BF16)
            wv_sb = const.tile([128, KB * HPC * DK], BF16)
            weng = {0: nc.scalar, 1: nc.scalar, 2: nc.gpsimd}
            for ti, (sb_t, dr) in enumerate(
                    ((wq_sb, wqT), (wk_sb, wkT), (wv_sb, wvT))):
                ngrp = 8 if ti == 0 else 4
                GW = KB // ngrp
                for g in range(ngrp):
                    m0 = g * GW * HPC * DK
                    weng[ti].dma_start(
                        sb_t[:, m0:m0 + GW * HPC * DK]
                            .rearrange("p (kb m) -> p kb m", kb=GW),
                        dr.ap()[g * GW * 128:(g + 1) * GW * 128, :]
                          .rearrange("(kb p) m -> p kb m", p=128))
            ropeC_sb = const.tile([DK, S], BF16)
            ropeS_sb = const.tile([DK, S], BF16)
            maskd_sb = const.tile([JB, 4 * QCH], BF16)
            nc.scalar.dma_start(ropeC_sb[:, :SUB], ropeC[:, :SUB])
            nc.scalar.dma_start(ropeS_sb[:, :SUB], ropeS[:, :SUB])
            nc.scalar.dma_start(ropeC_sb[:, SUB:], ropeC[:, SUB:])
            nc.scalar.dma_start(ropeS_sb[:, SUB:], ropeS[:, SUB:])
            nc.scalar.dma_start(maskd_sb[:], maskd[:])

            def rope_combine(ps_in, out_ap, s0, n):
                """out = ps_in * C + shuffle(ps_in) * S  (RoPE)."""
                qsh = work.tile([128, SUB], F32, tag="qsh", name="qsh")
                t1 = work.tile([128, SUB], BF16, tag="t1", name="t1")
                nc.vector.stream_shuffle(qsh[:, :n], ps_in, swap_mask)
                nc.any.tensor_tensor(
                    t1[:, :n], ps_in, ropeC_sb[:, s0:s0 + n], AluOpType.mult)
                nc.vector.tensor_tensor(
                    qsh[:, :n], qsh[:, :n], ropeS_sb[:, s0:s0 + n],
                    AluOpType.mult)
                nc.any.tensor_tensor(out_ap, t1[:, :n], qsh[:, :n],
                                     AluOpType.add)

            chain_engs = [nc.vector, nc.gpsimd]

            # Wo token-passes -------------------------------------------------
            # pass 0: chunks 0-3 (cols 0:256), interleaved into the second
            # half of the main loop; pass 1: chunks 4-6 (cols 256:448);
            # pass 2: chunk 7 (cols 448:512), reusing resident wo tiles.
            wo_engs = [nc.sync, nc.scalar]

            def load_woeb(eb, ei, anchor=None):
                wo_eb = wopool.tile([128, KB * DK], BF16, tag="wo",
                                    name="wo_eb")
                di = wo_engs[ei % 2].dma_start(
                    wo_eb[:],
                    woT.ap()[eb * D:(eb + 1) * D, :]
                       .rearrange("(p dl) e -> p (dl e)", p=128))
                if anchor is not None:
                    add_dep_helper(di.ins, anchor.ins, sync=True,
                                   reason="wo load after main loop")
                return wo_eb

            def load_yt(yt, chunks, ci0=0, anchors=None, eng=None):
                for ci, jc in enumerate(chunks):
                    di = (eng or nc.scalar).dma_start(
                        yt.rearrange("p (db c) -> p db c", db=KB)
                          [:, :, (ci0 + ci) * TPC:(ci0 + ci + 1) * TPC],
                        a2a_out[jc].rearrange("(db p) t -> p db t",
                                                   p=128))
                    if anchors is not None:
                        add_dep_helper(di.ins, anchors[ci].ins, sync=True,
                                       reason="yt load after its A2A landed")

            def wo_eb_mm(eb, yt, ncols, stage, wo_eb=None, ei=0):
                if wo_eb is None:
                    wo_eb = load_woeb(eb, ei)
                ps_w = ps_proj.tile([128, SUB], F32, tag="proj", name="ps_w")
                for dl in range(KB):
                    nc.tensor.matmul(
                        ps_w[:, :ncols],
                        wo_eb[:, dl * DK:(dl + 1) * DK],
                        yt[:, dl * ncols:(dl + 1) * ncols],
                        start=(dl == 0), stop=(dl == KB - 1))
                nc.any.tensor_copy(
                    stage[:, eb * ncols:(eb + 1) * ncols], ps_w[:, :ncols])

            def store_stage(stage, ncols, col0):
                nc.sync.dma_start(
                    outT.ap().rearrange("(eb p) c -> p eb c", p=128)
                        [:, :, col0:col0 + ncols],
                    stage.rearrange("p (eb c) -> p eb c", eb=KB))

            # ---------------- main pass over the sequence -------------------
            qT_tiles = {}
            kT_tiles = {}
            v_tiles = {}
            yt_p0 = [None]

            for sc in range(NSUB):
                b = sc // (NSUB // B)
                iq = sc % (NSUB // B)
                jc_glob = sc
                s0 = iq * SUB                   # position within batch

                if sc == 0:
                    xt = xt0
                else:
                    xt = xpool.tile([128, KB * SUB], BF16, tag="xt", name="xt")
                    xt_anchor[sc] = xt_dma(xt, sc)

                # ---- q/k projections + rope, both heads
                for h in range(HPC):
                    qT_tiles[h] = qpool.tile([128, QCH], BF16,
                                             tag=f"qT{h}", name="qT")
                    kT_tiles[(b, h, iq)] = kvpool.tile(
                        [128, QCH], BF16, tag="kT", name="kT")
                    for (w_sb, dst) in ((wq_sb, qT_tiles[h]),
                                        (wk_sb, kT_tiles[(b, h, iq)])):
                        psq = ps_proj.tile([128, SUB], F32, tag="proj", name="proj")
                        for kb in range(KB):
                            nc.tensor.matmul(
                                psq[:],
                                w_sb[:, kb * HPC * DK + h * DK:
                                     kb * HPC * DK + (h + 1) * DK],
                                xt[:, kb * SUB:(kb + 1) * SUB],
                                start=(kb == 0), stop=(kb == KB - 1))
                        rope_combine(psq[:], dst[:], s0, SUB)

                # ---- v projection, both heads, bf16 storage
                for tb in range(SUB // 128):
                    jb_b = iq * (SUB // 128) + tb
                    psv = ps_proj.tile([128, HPC * DK], F32, tag="proj",
                                       name="psv")
                    for kb in range(KB):
                        nc.tensor.matmul(
                            psv[:],
                            xt[:, kb * SUB + tb * 128:
                               kb * SUB + (tb + 1) * 128],
                            wv_sb[:, kb * HPC * DK:(kb + 1) * HPC * DK],
                            start=(kb == 0), stop=(kb == KB - 1))
                    vt = vpool.tile([128, HPC * DK], BF16, tag="v", name="vt")
                    nc.any.tensor_copy(vt[:], psv[:])
                    v_tiles[(b, jb_b)] = vt

                # ---- attention for the completed query chunk
                if True:
                    n_j = 4 * iq + 4
                    for h in range(HPC):
                        qT = qT_tiles[h]
                        ps_o = ps_out.tile([128, QCH], F32, tag="att_out", name="att_out")
                        p_tiles = {}

                        def emit_block(j):
                            jck, jr = j // 4, j % 4
                            m = j - 4 * iq
                            # diagonal block m: columns [0, 128m) are fully
                            # masked -> skip them and zero-fill p instead
                            q0 = 128 * m if m > 0 else 0
                            ps_s = ps_st.tile([JB, QCH], F32, tag="st", name="st")
                            nc.tensor.matmul(
                                ps_s[:, q0:],
                                kT_tiles[(b, h, jck)][:, jr * 128:
                                                      (jr + 1) * 128],
                                qT[:, q0:], start=True, stop=True)
                            if m >= 0:           # diagonal block: mask
                                # full width: the un-written [0,q0) region
                                # holds bounded stale scores; -30000 sends
                                # them to exp()==0, zero-filling p for free
                                nc.vector.tensor_tensor(
                                    ps_s[:], ps_s[:],
                                    maskd_sb[:, m * QCH:(m + 1) * QCH],
                                    AluOpType.add)
                            p_t = ppool.tile([JB, QCH], BF16, tag="p", name="p")
                            nc.scalar.activation(
                                p_t[:], ps_s[:],
                                mybir.ActivationFunctionType.Exp)
                            p_tiles[j] = p_t

                        # scores/exp run two blocks ahead of the AV matmuls
                        # so PE never waits on the Act engine
                        emit_block(0)
                        if n_j > 1:
                            emit_block(1)
                        # l = sum_k p: two sequential accumulator chains
                        # (even/odd j) spread across DVE/Pool/Act
                        acc = [None, None]
                        n_add = 0
                        cengs = (chain_engs if jc_glob < NCH - 1
                                 else [nc.vector])
                        for j in range(n_j):
                            if j + 2 < n_j:
                                emit_block(j + 2)
                            p_t = p_tiles[j]
                            nc.tensor.matmul(
                                ps_o[:], v_tiles[(b, j)][:, h * DK:
                                                         (h + 1) * DK],
                                p_t[:],
                                start=(j == 0), stop=(j == n_j - 1))
                            c = j % 2
                            if j >= 2:
                                eng = cengs[0]
                                n_add += 1
                                if acc[c] is None:
                                    a_t = lpool.tile([128, QCH], BF16,
                                                     tag="acc", name="acc")
                                    eng.tensor_tensor(
                                        a_t[:], p_tiles[c][:], p_t[:],
                                        AluOpType.add)
                                    acc[c] = a_t
                                else:
                                    eng.tensor_tensor(
                                        acc[c][:], acc[c][:], p_t[:],
                                        AluOpType.add)
                        if acc[0] is None:       # n_j == 4 has j = 0..3
                            acc = [p_tiles[0], p_tiles[1]]
                        p_acc = lpool.tile([128, QCH], BF16, tag="acc2", name="acc2")
                        cengs[0].tensor_tensor(
                            p_acc[:], acc[0][:], acc[1][:], AluOpType.add)
                        # gpsimd all-reduce output is broadcast across
                        # partitions -> no r-broadcast matmul needed
                        l_bc = lpool.tile([128, QCH], F32, tag="lbc", name="lbc")
                        nc.gpsimd.partition_all_reduce(
                            l_bc[:], p_acc[:], channels=128,
                            reduce_op=bass_isa.ReduceOp.add)
                        nc.vector.reciprocal(l_bc[:], l_bc[:])
                        y_sb = ypool.tile([128, QCH], BF16, tag="y", name="y")
                        nc.any.tensor_tensor(y_sb[:], ps_o[:], l_bc[:],
                                             AluOpType.mult)
                        # scatter into the AllToAll source layout:
                        # dst row m*256 + h*128 + p, col t <- src[p, m*64+t]
                        di = nc.gpsimd.dma_start(
                            a2a_in[jc_glob]
                            .rearrange("(m hp) t -> hp m t", m=N_CORES)
                            [h * 128:(h + 1) * 128],
                            y_sb.rearrange("p (m t) -> p m t", m=N_CORES))
                        if h == HPC - 1:
                            store_anchor[jc_glob] = di
                    a2a_inst[jc_glob] = nc.gpsimd.collective_compute(
                        "AllToAll", AluOpType.bypass,
                        replica_groups=[list(range(N_CORES))],
                        ins=[a2a_in[jc_glob].opt()],
                        outs=[a2a_out[jc_glob].opt()])


            # ---- Wo pass A: chunks 0-3 (cols 0:256).  Runs entirely in
            # the shadow of the last AllToAll: those chunks' y landed long
            # ago.  Every staging DMA is anchored so the dataflow
            # scheduler cannot hoist its wait into the main loop.
            yt_p0[0] = ytpool.tile([128, KB * 4 * TPC], BF16,
                                   tag="yt0", name="yt0")
            load_yt(yt_p0[0], range(4),
                    anchors=[store_anchor[c + 1] for c in range(4)])
            yt_p1 = ytpool.tile([128, KB * 3 * TPC], BF16, tag="yt1",
                                name="yt1")
            load_yt(yt_p1, range(4, 7),
                    anchors=[store_anchor[5], store_anchor[6],
                             store_anchor[7]])
            stage_a = stpool.tile([128, KB * 4 * TPC], BF16, tag="sta",
                                  name="sta")
            stage_b1 = stpool.tile([128, KB * 3 * TPC], BF16, tag="stb1",
                                   name="stb1")
            stage_b2 = stpool.tile([128, KB * TPC], BF16, tag="stb2",
                                   name="stb2")
            p0_pref = {eb: load_woeb(eb, eb, anchor=xt_anchor[5])
                       for eb in range(2)}
            for eb in range(KB):
                if eb + 2 < KB:       # stay 2 loads ahead of the matmuls
                    p0_pref[eb + 2] = load_woeb(eb + 2, eb,
                                                anchor=xt_anchor[5])
                wo_eb_mm(eb, yt_p0[0], 4 * TPC, stage_a,
                         wo_eb=p0_pref.pop(eb), ei=eb)

            store_stage(stage_a, 4 * TPC, 0)

            # ---- Wo pass B1: chunks 4-6 (cols 256:448) -- none of this
            # waits on the final AllToAll, so it fills the A2A-7 shadow
            # right after pass A.
            b1_tiles = {eb: load_woeb(eb, eb, anchor=xt_anchor[6])
                        for eb in range(2)}
            for eb in range(KB):
                if eb + 2 < KB:
                    b1_tiles[eb + 2] = load_woeb(eb + 2, eb,
                                                 anchor=xt_anchor[6])
                wo_eb_mm(eb, yt_p1, 3 * TPC, stage_b1,
                         wo_eb=b1_tiles[eb], ei=eb)

            store_stage(stage_b1, 3 * TPC, 4 * TPC)

            # ---- Wo pass B2: chunk 7 (cols 448:512), gated on the final
            # AllToAll.  The last wopool-1 tiles of B1 are still resident:
            # run those first, re-stream the rest.
            yt_p2 = ytpool.tile([128, KB * TPC], BF16, tag="yt2",
                                name="yt2")
            load_yt(yt_p2, range(7, 8),
                    anchors=[store_anchor[7]], eng=nc.gpsimd)
            for eb in range(KB - 13, KB):
                wo_eb_mm(eb, yt_p2, TPC, stage_b2, wo_eb=b1_tiles[eb])
            for ei, eb in enumerate(range(KB - 13)):
                wo_eb_mm(eb, yt_p2, TPC, stage_b2, ei=ei)
            # rows 3..15 (resident-weight blocks) finish first: store them
            # while the three re-streamed blocks compute, leaving only a
            # tiny final store on the critical path
            nc.sync.dma_start(
                outT.ap().rearrange("(eb p) c -> p eb c", p=128)
                    [:, KB - 13:, 7 * TPC:],
                stage_b2.rearrange("p (eb c) -> p eb c", eb=KB)
                        [:, KB - 13:])
            nc.sync.dma_start(
                outT.ap().rearrange("(eb p) c -> p eb c", p=128)
                    [:, :KB - 13, 7 * TPC:],
                stage_b2.rearrange("p (eb c) -> p eb c", eb=KB)
                        [:, :KB - 13])

    nc.finalize()
    return nc


# ---------------------------------------------------------------- host
def _host_inputs(x, W_q, W_k, W_v, W_o):
    import ml_dtypes
    bf = np.dtype(ml_dtypes.bfloat16)
    xT = np.ascontiguousarray(
        x.reshape(TOK, D).T).astype(bf)                        # [D, TOK]

    # W_o.T tiled (eb, p, dl, e'): row eb*2048 + p*16 + dl, col e'
    woT = np.ascontiguousarray(
        W_o.T.reshape(KB, 128, KB, DK).transpose(2, 1, 0, 3)
        .reshape(KB * D, DK)).astype(bf)

    # RoPE tables, expanded to [DK, S] with interleaved pairs; the sign
    # table carries -sin on even rows, +sin on odd rows.
    i = np.arange(0, DK, 2, dtype=np.float32)
    theta = 1.0 / (ROPE_BASE ** (i / DK))                      # [64]
    pos = np.arange(S, dtype=np.float32)
    freqs = pos[:, None] * theta[None, :]                      # [S, 64]
    cos_t, sin_t = np.cos(freqs), np.sin(freqs)
    ropeC = np.empty((DK, S), np.float32)
    ropeS = np.empty((DK, S), np.float32)
    ropeC[0::2] = cos_t.T
    ropeC[1::2] = cos_t.T
    ropeS[0::2] = -sin_t.T
    ropeS[1::2] = sin_t.T

    # diagonal causal masks: block m (of the 4 key blocks overlapping a
    # 512-query chunk) keeps kk <= qq - 128*m
    kk = np.arange(JB)[:, None]
    qq = np.arange(QCH)[None, :]
    maskd = np.concatenate(
        [np.where(kk <= qq - 128 * m, 0.0, MASK_NEG).astype(np.float32)
         for m in range(4)], axis=1).astype(bf)                # [128, 4*512]

    scale = 1.0 / np.sqrt(np.float32(DK))
    in_maps = []
    for c in range(N_CORES):
        rows = slice(c * HPC * DK, (c + 1) * HPC * DK)
        in_maps.append({
            "xT": xT,
            "wqT": np.ascontiguousarray(
                (W_q[rows] * scale).T).astype(bf),
            "wkT": np.ascontiguousarray(W_k[rows].T).astype(bf),
            "wvT": np.ascontiguousarray(W_v[rows].T).astype(bf),
            "woT": woT,
            "ropeC": ropeC.astype(bf),
            "ropeS": ropeS.astype(bf),
            "maskd": maskd,
        })
    return in_maps


def kernel(x, W_q, W_k, W_v, W_o):
    x = np.asarray(x, dtype=np.float32)
    W_q = np.asarray(W_q, dtype=np.float32)
    W_k = np.asarray(W_k, dtype=np.float32)
    W_v = np.asarray(W_v, dtype=np.float32)
    W_o = np.asarray(W_o, dtype=np.float32)

    if "nc" not in _CACHE:
        _CACHE["nc"] = _build_nc()
    nc = _CACHE["nc"]

    in_maps = _host_inputs(x, W_q, W_k, W_v, W_o)
    res = bass_utils.run_bass_kernel_spmd(
        nc, in_maps, core_ids=list(range(N_CORES)))

    # outT per core: [D, 512] f32; col 64*jc + t -> token 512*jc + 64*c + t
    out_T = np.empty((D, TOK), np.float32)
    for c in range(N_CORES):
        cols = (QCH * np.arange(NCH)[:, None] + TPC * c
                + np.arange(TPC)[None, :]).ravel()
        out_T[:, cols] = res.results[c]["outT"].astype(np.float32)
    return np.ascontiguousarray(out_T.T).reshape(B, S, D).astype(np.float32)


# revision 63
# speedup vs baseline: 1.4764x; 1.0074x over previous
"""Causal self-attention with RoPE on 8 Trainium2 NeuronCores.

Sharding: tensor-parallel over heads (16 heads -> 2 per core) for the
QKV projections, RoPE and attention.  The attention output is
re-sharded token-wise with one small AllToAll per 512-token chunk
(bf16 payload), so each core ends up with the full 2048-dim y vector
for 64 tokens of every chunk; the output projection then runs
token-parallel with no all-reduce.

Single pass over the sequence: both heads' q/k/v are produced from one
streaming of x (the baseline used one pass per head and loaded x
twice).

Key engine-placement choices (cost-model driven):
  - softmax denominator l = sum_k p is NOT computed with PE matmuls
    (those cost as much as the score matmuls); instead p-blocks are
    summed with a shallow tree of tensor_tensor adds spread across
    DVE/Act/Pool and one gpsimd.partition_all_reduce, whose output is
    already broadcast across partitions (also kills the r-broadcast
    matmul of the baseline).
  - y, v and W_o are bf16 (halves the collective payload and the Wo
    weight traffic; bf16 matmuls run at full PE rate).
  - W_o streams through SBUF per token-pass; the last pass reuses the
    still-resident tiles of the previous pass for its first blocks.

Shapes (hardcoded): x [2, 2048, 2048], W_* [2048, 2048], 16 heads,
d_k = 128, fp32 in/out.
"""

import sys

for _p in ("/opt/trn_rl_repo", "/opt/pypackages"):
    if _p not in sys.path:
        sys.path.insert(0, _p)

import numpy as np

import concourse.bass as bass
import concourse.bacc as bacc
import concourse.mybir as mybir
import concourse.tile as tile
from concourse import bass_utils
from concourse import bass_isa
from concourse.alu_op_type import AluOpType
from concourse.tile import add_dep_helper

# ---------------------------------------------------------------- config
N_CORES = 8
B, S, D = 2, 2048, 2048
H = 16
DK = D // H              # 128
HPC = H // N_CORES       # 2 heads per core
TOK = B * S              # 4096
SUB = 512                # token sub-chunk for projections (= one chunk)
QCH = 512                # attention query chunk
JB = 128                 # attention key block
NSUB = TOK // SUB        # 16
NCH = TOK // QCH         # 8 query chunks
KB = D // 128            # 16 contraction blocks
TPC = QCH // N_CORES     # 64 tokens per (chunk, core) after AllToAll
ROPE_BASE = 10000.0
MASK_NEG = -30000.0

F32 = mybir.dt.float32
F32R = mybir.dt.float32r
BF16 = mybir.dt.bfloat16

_CACHE = {}


def _build_nc():
    dt = F32R
    nc = bacc.Bacc("TRN2", target_bir_lowering=False, debug=False,
                   num_devices=N_CORES)

    xT = nc.dram_tensor("xT", [D, TOK], BF16, kind="ExternalInput")
    wqT = nc.dram_tensor("wqT", [D, HPC * DK], BF16, kind="ExternalInput")
    wkT = nc.dram_tensor("wkT", [D, HPC * DK], BF16, kind="ExternalInput")
    wvT = nc.dram_tensor("wvT", [D, HPC * DK], BF16, kind="ExternalInput")
    # W_o.T in bf16, tiled (eb, p, dl, e'): row = eb*2048 + p*16 + dl
    woT = nc.dram_tensor("woT", [KB * D, DK], BF16, kind="ExternalInput")
    ropeC = nc.dram_tensor("ropeC", [DK, S], BF16, kind="ExternalInput")
    ropeS = nc.dram_tensor("ropeS", [DK, S], BF16, kind="ExternalInput")
    maskd = nc.dram_tensor("maskd", [JB, 4 * QCH], BF16, kind="ExternalInput")
    # out columns ordered (jc 0..7, t 0..63): global token 512*jc + 64*c + t
    outT = nc.dram_tensor("outT", [D, QCH], BF16, kind="ExternalOutput")

    swap_mask = [i ^ 1 for i in range(32)]

    import contextlib
    with tile.TileContext(nc) as tc:
        with contextlib.ExitStack() as st:
            dram = st.enter_context(
                tc.tile_pool(name="dram", bufs=1, space="DRAM"))
            a2a_in = [dram.tile([N_CORES * HPC * DK, TPC], BF16,
                                name=f"a2ain{j}") for j in range(NCH)]
            a2a_out = [dram.tile([N_CORES * HPC * DK, TPC], BF16,
                                 name=f"a2aout{j}") for j in range(NCH)]

            const = st.enter_context(tc.tile_pool(name="const", bufs=1))
            xpool = st.enter_context(
                tc.tile_pool(name="xpool", bufs=2, side="right"))
            qpool = st.enter_context(
                tc.tile_pool(name="qpool", bufs=2, side="right"))
            kvpool = st.enter_context(
                tc.tile_pool(name="kvpool", bufs=8, side="right"))
            vpool = st.enter_context(
                tc.tile_pool(name="vpool", bufs=15, side="right"))
            ppool = st.enter_context(tc.tile_pool(name="ppool", bufs=7))
            lpool = st.enter_context(tc.tile_pool(name="lpool", bufs=2))
            work = st.enter_context(tc.tile_pool(name="work", bufs=2))
            ypool = st.enter_context(tc.tile_pool(name="ypool", bufs=3))
            ytpool = st.enter_context(tc.tile_pool(name="ytpool", bufs=1))
            stpool = st.enter_context(tc.tile_pool(name="stpool", bufs=1))
            wopool = st.enter_context(
                tc.tile_pool(name="wopool", bufs=16, side="right"))
            ps_proj = st.enter_context(
                tc.tile_pool(name="ps_proj", bufs=3, space="PSUM"))
            ps_st = st.enter_context(
                tc.tile_pool(name="ps_st", bufs=3, space="PSUM"))
            ps_out = st.enter_context(
                tc.tile_pool(name="ps_out", bufs=2, space="PSUM"))

            # chunk-0 xT goes first on its queues so the first projection
            # isn't stuck behind weight DMAs
            def xt_dma(xt, sc):
                KH = KB // 4
                di = None
                for xh in range(4):
                    di = nc.sync.dma_start(
                        xt[:, xh * KH * SUB:(xh + 1) * KH * SUB]
                          .rearrange("p (kb t) -> p kb t", kb=KH),
                        xT.ap()[xh * KH * 128:(xh + 1) * KH * 128,
                                sc * SUB:(sc + 1) * SUB]
                          .rearrange("(kb p) t -> p kb t", p=128))
                return di

            xt_anchor = {}
            store_anchor = {}
            a2a_inst = {}
            xt0 = xpool.tile([128, KB * SUB], BF16, tag="xt", name="xt")
            KH0 = KB // 8
            for xh in range(8):
                di = nc.sync.dma_start(
                    xt0[:, xh * KH0 * SUB:(xh + 1) * KH0 * SUB]
                       .rearrange("p (kb t) -> p kb t", kb=KH0),
                    xT.ap()[xh * KH0 * 128:(xh + 1) * KH0 * 128, 0:SUB]
                      .rearrange("(kb p) t -> p kb t", p=128))
            xt_anchor[0] = di

            # ---- persistent constants in SBUF
            wq_sb = const.tile([128, KB * HPC * DK], BF16)
            wk_sb = const.tile([128, KB * HPC * DK], BF16)
            wv_sb = const.tile([128, KB * HPC * DK], BF16)
            weng = {0: nc.scalar, 1: nc.scalar, 2: nc.gpsimd}
            for ti, (sb_t, dr) in enumerate(
                    ((wq_sb, wqT), (wk_sb, wkT), (wv_sb, wvT))):
                ngrp = 8 if ti == 0 else 4
                GW = KB // ngrp
                for g in range(ngrp):
                    m0 = g * GW * HPC * DK
                    weng[ti].dma_start(
                        sb_t[:, m0:m0 + GW * HPC * DK]
                            .rearrange("p (kb m) -> p kb m", kb=GW),
                        dr.ap()[g * GW * 128:(g + 1) * GW * 128, :]
                          .rearrange("(kb p) m -> p kb m", p=128))
            ropeC_sb = const.tile([DK, S], BF16)
            ropeS_sb = const.tile([DK, S], BF16)
            maskd_sb = const.tile([JB, 4 * QCH], BF16)
            nc.scalar.dma_start(ropeC_sb[:, :SUB], ropeC[:, :SUB])
            nc.scalar.dma_start(ropeS_sb[:, :SUB], ropeS[:, :SUB])
            nc.scalar.dma_start(ropeC_sb[:, SUB:], ropeC[:, SUB:])
            nc.scalar.dma_start(ropeS_sb[:, SUB:], ropeS[:, SUB:])
            nc.scalar.dma_start(maskd_sb[:], maskd[:])

            def rope_combine(ps_in, out_ap, s0, n):
                """out = ps_in * C + shuffle(ps_in) * S  (RoPE)."""
                qsh = work.tile([128, SUB], F32, tag="qsh", name="qsh")
                t1 = work.tile([128, SUB], BF16, tag="t1", name="t1")
                nc.vector.stream_shuffle(qsh[:, :n], ps_in, swap_mask)
                nc.any.tensor_tensor(
                    t1[:, :n], ps_in, ropeC_sb[:, s0:s0 + n], AluOpType.mult)
                nc.vector.tensor_tensor(
                    qsh[:, :n], qsh[:, :n], ropeS_sb[:, s0:s0 + n],
                    AluOpType.mult)
                nc.any.tensor_tensor(out_ap, t1[:, :n], qsh[:, :n],
                                     AluOpType.add)

            chain_engs = [nc.vector, nc.gpsimd]

            # Wo token-passes -------------------------------------------------
            # pass 0: chunks 0-3 (cols 0:256), interleaved into the second
            # half of the main loop; pass 1: chunks 4-6 (cols 256:448);
            # pass 2: chunk 7 (cols 448:512), reusing resident wo tiles.
            wo_engs = [nc.sync, nc.scalar]

            def load_woeb(eb, ei, anchor=None):
                wo_eb = wopool.tile([128, KB * DK], BF16, tag="wo",
                                    name="wo_eb")
                di = wo_engs[ei % 2].dma_start(
                    wo_eb[:],
                    woT.ap()[eb * D:(eb + 1) * D, :]
                       .rearrange("(p dl) e -> p (dl e)", p=128))
                if anchor is not None:
                    add_dep_helper(di.ins, anchor.ins, sync=True,
                                   reason="wo load after main loop")
                return wo_eb

            def load_yt(yt, chunks, ci0=0, anchors=None, eng=None):
                for ci, jc in enumerate(chunks):
                    di = (eng or nc.scalar).dma_start(
                        yt.rearrange("p (db c) -> p db c", db=KB)
                          [:, :, (ci0 + ci) * TPC:(ci0 + ci + 1) * TPC],
                        a2a_out[jc].rearrange("(db p) t -> p db t",
                                                   p=128))
                    if anchors is not None:
                        add_dep_helper(di.ins, anchors[ci].ins, sync=True,
                                       reason="yt load after its A2A landed")

            def wo_eb_mm(eb, yt, ncols, stage, wo_eb=None, ei=0):
                if wo_eb is None:
                    wo_eb = load_woeb(eb, ei)
                ps_w = ps_proj.tile([128, SUB], F32, tag="proj", name="ps_w")
                for dl in range(KB):
                    nc.tensor.matmul(
                        ps_w[:, :ncols],
                        wo_eb[:, dl * DK:(dl + 1) * DK],
                        yt[:, dl * ncols:(dl + 1) * ncols],
                        start=(dl == 0), stop=(dl == KB - 1))
                nc.any.tensor_copy(
                    stage[:, eb * ncols:(eb + 1) * ncols], ps_w[:, :ncols])

            def store_stage(stage, ncols, col0):
                nc.sync.dma_start(
                    outT.ap().rearrange("(eb p) c -> p eb c", p=128)
                        [:, :, col0:col0 + ncols],
                    stage.rearrange("p (eb c) -> p eb c", eb=KB))

            # ---------------- main pass over the sequence -------------------
            qT_tiles = {}
            kT_tiles = {}
            v_tiles = {}
            yt_p0 = [None]

            for sc in range(NSUB):
                b = sc // (NSUB // B)
                iq = sc % (NSUB // B)
                jc_glob = sc
                s0 = iq * SUB                   # position within batch

                if sc == 0:
                    xt = xt0
                else:
                    xt = xpool.tile([128, KB * SUB], BF16, tag="xt", name="xt")
                    xt_anchor[sc] = xt_dma(xt, sc)

                # ---- q/k projections + rope, both heads
                for h in range(HPC):
                    qT_tiles[h] = qpool.tile([128, QCH], BF16,
                                             tag=f"qT{h}", name="qT")
                    kT_tiles[(b, h, iq)] = kvpool.tile(
                        [128, QCH], BF16, tag="kT", name="kT")
                    for (w_sb, dst) in ((wq_sb, qT_tiles[h]),
                                        (wk_sb, kT_tiles[(b, h, iq)])):
                        psq = ps_proj.tile([128, SUB], F32, tag="proj", name="proj")
                        for kb in range(KB):
                            nc.tensor.matmul(
                                psq[:],
                                w_sb[:, kb * HPC * DK + h * DK:
                                     kb * HPC * DK + (h + 1) * DK],
                                xt[:, kb * SUB:(kb + 1) * SUB],
                                start=(kb == 0), stop=(kb == KB - 1))
                        rope_combine(psq[:], dst[:], s0, SUB)

                # ---- v projection, both heads, bf16 storage
                for tb in range(SUB // 128):
                    jb_b = iq * (SUB // 128) + tb
                    psv = ps_proj.tile([128, HPC * DK], F32, tag="proj",
                                       name="psv")
                    for kb in range(KB):
                        nc.tensor.matmul(
                            psv[:],
                            xt[:, kb * SUB + tb * 128:
                               kb * SUB + (tb + 1) * 128],
                            wv_sb[:, kb * HPC * DK:(kb + 1) * HPC * DK],
                            start=(kb == 0), stop=(kb == KB - 1))
                    vt = vpool.tile([128, HPC * DK], BF16, tag="v", name="vt")
                    nc.any.tensor_copy(vt[:], psv[:])
                    v_tiles[(b, jb_b)] = vt

                # ---- attention for the completed query chunk
                if True:
                    n_j = 4 * iq + 4
                    for h in range(HPC):
                        qT = qT_tiles[h]
                        ps_o = ps_out.tile([128, QCH], F32, tag="att_out", name="att_out")
                        p_tiles = {}

                        def emit_block(j):
                            jck, jr = j // 4, j % 4
                            m = j - 4 * iq
                            # diagonal block m: columns [0, 128m) are fully
                            # masked -> skip them and zero-fill p instead
                            q0 = 128 * m if m > 0 else 0
                            ps_s = ps_st.tile([JB, QCH], F32, tag="st", name="st")
                            nc.tensor.matmul(
                                ps_s[:, q0:],
                                kT_tiles[(b, h, jck)][:, jr * 128:
                                                      (jr + 1) * 128],
                                qT[:, q0:], start=True, stop=True)
                            if m >= 0:           # diagonal block: mask
                                # full width: the un-written [0,q0) region
                                # holds bounded stale scores; -30000 sends
                                # them to exp()==0, zero-filling p for free
                                nc.vector.tensor_tensor(
                                    ps_s[:], ps_s[:],
                                    maskd_sb[:, m * QCH:(m + 1) * QCH],
                                    AluOpType.add)
                            p_t = ppool.tile([JB, QCH], BF16, tag="p", name="p")
                            nc.scalar.activation(
                                p_t[:], ps_s[:],
                                mybir.ActivationFunctionType.Exp)
                            p_tiles[j] = p_t

                        # scores/exp run two blocks ahead of the AV matmuls
                        # so PE never waits on the Act engine
                        emit_block(0)
                        if n_j > 1:
                            emit_block(1)
                        # l = sum_k p: two sequential accumulator chains
                        # (even/odd j) spread across DVE/Pool/Act
                        acc = [None, None]
                        n_add = 0
                        cengs = (chain_engs if jc_glob < NCH - 1
                                 else [nc.vector])
                        for j in range(n_j):
                            if j + 2 < n_j:
                                emit_block(j + 2)
                            p_t = p_tiles[j]
                            nc.tensor.matmul(
                                ps_o[:], v_tiles[(b, j)][:, h * DK:
                                                         (h + 1) * DK],
                                p_t[:],
                                start=(j == 0), stop=(j == n_j - 1))
                            c = j % 2
                            if j >= 2:
                                eng = cengs[0]
                                n_add += 1
                                if acc[c] is None:
                                    a_t = lpool.tile([128, QCH], BF16,
                                                     tag="acc", name="acc")
                                    eng.tensor_tensor(
                                        a_t[:], p_tiles[c][:], p_t[:],
                                        AluOpType.add)
                                    acc[c] = a_t
                                else:
                                    eng.tensor_tensor(
                                        acc[c][:], acc[c][:], p_t[:],
                                        AluOpType.add)
                        if acc[0] is None:       # n_j == 4 has j = 0..3
                            acc = [p_tiles[0], p_tiles[1]]
                        p_acc = lpool.tile([128, QCH], BF16, tag="acc2", name="acc2")
                        cengs[0].tensor_tensor(
                            p_acc[:], acc[0][:], acc[1][:], AluOpType.add)
                        # gpsimd all-reduce output is broadcast across
                        # partitions -> no r-broadcast matmul needed
                        l_bc = lpool.tile([128, QCH], F32, tag="lbc", name="lbc")
                        nc.gpsimd.partition_all_reduce(
                            l_bc[:], p_acc[:], channels=128,
                            reduce_op=bass_isa.ReduceOp.add)
                        nc.vector.reciprocal(l_bc[:], l_bc[:])
                        y_sb = ypool.tile([128, QCH], BF16, tag="y", name="y")
                        nc.any.tensor_tensor(y_sb[:], ps_o[:], l_bc[:],
                                             AluOpType.mult)
                        # scatter into the AllToAll source layout:
                        # dst row m*256 + h*128 + p, col t <- src[p, m*64+t]
                        di = nc.gpsimd.dma_start(
                            a2a_in[jc_glob]
                            .rearrange("(m hp) t -> hp m t", m=N_CORES)
                            [h * 128:(h + 1) * 128],
                            y_sb.rearrange("p (m t) -> p m t", m=N_CORES))
                        if h == HPC - 1:
                            store_anchor[jc_glob] = di
                    a2a_inst[jc_glob] = nc.gpsimd.collective_compute(
                        "AllToAll", AluOpType.bypass,
                        replica_groups=[list(range(N_CORES))],
                        ins=[a2a_in[jc_glob].opt()],
                        outs=[a2a_out[jc_glob].opt()])


            # ---- Wo pass A: chunks 0-3 (cols 0:256).  Runs entirely in
            # the shadow of the last AllToAll: those chunks' y landed long
            # ago.  Every staging DMA is anchored so the dataflow
            # scheduler cannot hoist its wait into the main loop.
            yt_p0[0] = ytpool.tile([128, KB * 4 * TPC], BF16,
                                   tag="yt0", name="yt0")
            load_yt(yt_p0[0], range(4),
                    anchors=[store_anchor[c + 1] for c in range(4)])
            yt_p1 = ytpool.tile([128, KB * 3 * TPC], BF16, tag="yt1",
                                name="yt1")
            load_yt(yt_p1, range(4, 7),
                    anchors=[store_anchor[5], store_anchor[6],
                             store_anchor[7]])
            stage_a = stpool.tile([128, KB * 4 * TPC], BF16, tag="sta",
                                  name="sta")
            stage_b1 = stpool.tile([128, KB * 3 * TPC], BF16, tag="stb1",
                                   name="stb1")
            stage_b2 = stpool.tile([128, KB * TPC], BF16, tag="stb2",
                                   name="stb2")
            p0_pref = {eb: load_woeb(eb, eb, anchor=xt_anchor[5])
                       for eb in range(2)}
            for eb in range(KB):
                if eb + 2 < KB:       # stay 2 loads ahead of the matmuls
                    p0_pref[eb + 2] = load_woeb(eb + 2, eb,
                                                anchor=xt_anchor[5])
                wo_eb_mm(eb, yt_p0[0], 4 * TPC, stage_a,
                         wo_eb=p0_pref.pop(eb), ei=eb)

            store_stage(stage_a, 4 * TPC, 0)

            # ---- Wo pass B1: chunks 4-6 (cols 256:448) -- none of this
            # waits on the final AllToAll, so it fills the A2A-7 shadow
            # right after pass A.
            b1_tiles = {eb: load_woeb(eb, eb, anchor=xt_anchor[6])
                        for eb in range(2)}
            for eb in range(KB):
                if eb + 2 < KB:
                    b1_tiles[eb + 2] = load_woeb(eb + 2, eb,
                                                 anchor=xt_anchor[6])
                wo_eb_mm(eb, yt_p1, 3 * TPC, stage_b1,
                         wo_eb=b1_tiles[eb], ei=eb)

            store_stage(stage_b1, 3 * TPC, 4 * TPC)

            # ---- Wo pass B2: chunk 7 (cols 448:512), gated on the final
            # AllToAll.  The last wopool-1 tiles of B1 are still resident:
            # run those first, re-stream the rest.
            yt_p2 = ytpool.tile([128, KB * TPC], BF16, tag="yt2",
                                name="yt2")
            load_yt(yt_p2, range(7, 8),
                    anchors=[store_anchor[7]], eng=nc.gpsimd)
            for eb in range(KB - 13, KB):
                wo_eb_mm(eb, yt_p2, TPC, stage_b2, wo_eb=b1_tiles[eb])
            for ei, eb in enumerate(range(KB - 13)):
                wo_eb_mm(eb, yt_p2, TPC, stage_b2, ei=ei)
            # rows 3..15 (resident-weight blocks) finish first: store them
            # while the three re-streamed blocks compute, leaving only a
            # tiny final store on the critical path
            for q4 in range(4):
                nc.sync.dma_start(
                    outT.ap().rearrange("(eb p) c -> p eb c", p=128)
                        [:, q4 * 4:(q4 + 1) * 4, 7 * TPC:],
                    stage_b2.rearrange("p (eb c) -> p eb c", eb=KB)
                            [:, q4 * 4:(q4 + 1) * 4])

    nc.finalize()
    return nc


# ---------------------------------------------------------------- host
def _host_inputs(x, W_q, W_k, W_v, W_o):
    import ml_dtypes
    bf = np.dtype(ml_dtypes.bfloat16)
    xT = np.ascontiguousarray(
        x.reshape(TOK, D).T).astype(bf)                        # [D, TOK]

    # W_o.T tiled (eb, p, dl, e'): row eb*2048 + p*16 + dl, col e'
    woT = np.ascontiguousarray(
        W_o.T.reshape(KB, 128, KB, DK).transpose(2, 1, 0, 3)
        .reshape(KB * D, DK)).astype(bf)

    # RoPE tables, expanded to [DK, S] with interleaved pairs; the sign
    # table carries -sin on even rows, +sin on odd rows.
    i = np.arange(0, DK, 2, dtype=np.float32)
    theta = 1.0 / (ROPE_BASE ** (i / DK))                      # [64]
    pos = np.arange(S, dtype=np.float32)
    freqs = pos[:, None] * theta[None, :]                      # [S, 64]
    cos_t, sin_t = np.cos(freqs), np.sin(freqs)
    ropeC = np.empty((DK, S), np.float32)
    ropeS = np.empty((DK, S), np.float32)
    ropeC[0::2] = cos_t.T
    ropeC[1::2] = cos_t.T
    ropeS[0::2] = -sin_t.T
    ropeS[1::2] = sin_t.T

    # diagonal causal masks: block m (of the 4 key blocks overlapping a
    # 512-query chunk) keeps kk <= qq - 128*m
    kk = np.arange(JB)[:, None]
    qq = np.arange(QCH)[None, :]
    maskd = np.concatenate(
        [np.where(kk <= qq - 128 * m, 0.0, MASK_NEG).astype(np.float32)
         for m in range(4)], axis=1).astype(bf)                # [128, 4*512]

    scale = 1.0 / np.sqrt(np.float32(DK))
    in_maps = []
    for c in range(N_CORES):
        rows = slice(c * HPC * DK, (c + 1) * HPC * DK)
        in_maps.append({
            "xT": xT,
            "wqT": np.ascontiguousarray(
                (W_q[rows] * scale).T).astype(bf),
            "wkT": np.ascontiguousarray(W_k[rows].T).astype(bf),
            "wvT": np.ascontiguousarray(W_v[rows].T).astype(bf),
            "woT": woT,
            "ropeC": ropeC.astype(bf),
            "ropeS": ropeS.astype(bf),
            "maskd": maskd,
        })
    return in_maps


def kernel(x, W_q, W_k, W_v, W_o):
    x = np.asarray(x, dtype=np.float32)
    W_q = np.asarray(W_q, dtype=np.float32)
    W_k = np.asarray(W_k, dtype=np.float32)
    W_v = np.asarray(W_v, dtype=np.float32)
    W_o = np.asarray(W_o, dtype=np.float32)

    if "nc" not in _CACHE:
        _CACHE["nc"] = _build_nc()
    nc = _CACHE["nc"]

    in_maps = _host_inputs(x, W_q, W_k, W_v, W_o)
    res = bass_utils.run_bass_kernel_spmd(
        nc, in_maps, core_ids=list(range(N_CORES)))

    # outT per core: [D, 512] f32; col 64*jc + t -> token 512*jc + 64*c + t
    out_T = np.empty((D, TOK), np.float32)
    for c in range(N_CORES):
        cols = (QCH * np.arange(NCH)[:, None] + TPC * c
                + np.arange(TPC)[None, :]).ravel()
        out_T[:, cols] = res.results[c]["outT"].astype(np.float32)
    return np.ascontiguousarray(out_T.T).reshape(B, S, D).astype(np.float32)


# revision 68
# speedup vs baseline: 1.4902x; 1.0093x over previous
"""Causal self-attention with RoPE on 8 Trainium2 NeuronCores.

Sharding: tensor-parallel over heads (16 heads -> 2 per core) for the
QKV projections, RoPE and attention.  The attention output is
re-sharded token-wise with one small AllToAll per 512-token chunk
(bf16 payload), so each core ends up with the full 2048-dim y vector
for 64 tokens of every chunk; the output projection then runs
token-parallel with no all-reduce.

Single pass over the sequence: both heads' q/k/v are produced from one
streaming of x (the baseline used one pass per head and loaded x
twice).

Key engine-placement choices (cost-model driven):
  - softmax denominator l = sum_k p is NOT computed with PE matmuls
    (those cost as much as the score matmuls); instead p-blocks are
    summed with a shallow tree of tensor_tensor adds spread across
    DVE/Act/Pool and one gpsimd.partition_all_reduce, whose output is
    already broadcast across partitions (also kills the r-broadcast
    matmul of the baseline).
  - y, v and W_o are bf16 (halves the collective payload and the Wo
    weight traffic; bf16 matmuls run at full PE rate).
  - W_o streams through SBUF per token-pass; the last pass reuses the
    still-resident tiles of the previous pass for its first blocks.

Shapes (hardcoded): x [2, 2048, 2048], W_* [2048, 2048], 16 heads,
d_k = 128, fp32 in/out.
"""

import sys

for _p in ("/opt/trn_rl_repo", "/opt/pypackages"):
    if _p not in sys.path:
        sys.path.insert(0, _p)

import numpy as np

import concourse.bass as bass
import concourse.bacc as bacc
import concourse.mybir as mybir
import concourse.tile as tile
from concourse import bass_utils
from concourse import bass_isa
from concourse.alu_op_type import AluOpType
from concourse.tile import add_dep_helper

# ---------------------------------------------------------------- config
N_CORES = 8
B, S, D = 2, 2048, 2048
H = 16
DK = D // H              # 128
HPC = H // N_CORES       # 2 heads per core
TOK = B * S              # 4096
SUB = 512                # token sub-chunk for projections (= one chunk)
QCH = 512                # attention query chunk
JB = 128                 # attention key block
NSUB = TOK // SUB        # 16
NCH = TOK // QCH         # 8 query chunks
KB = D // 128            # 16 contraction blocks
TPC = QCH // N_CORES     # 64 tokens per (chunk, core) after AllToAll
ROPE_BASE = 10000.0
MASK_NEG = -30000.0

F32 = mybir.dt.float32
F32R = mybir.dt.float32r
BF16 = mybir.dt.bfloat16

_CACHE = {}


def _build_nc():
    dt = F32R
    nc = bacc.Bacc("TRN2", target_bir_lowering=False, debug=False,
                   num_devices=N_CORES)

    xT = nc.dram_tensor("xT", [D, TOK], BF16, kind="ExternalInput")
    wqT = nc.dram_tensor("wqT", [D, HPC * DK], BF16, kind="ExternalInput")
    wkT = nc.dram_tensor("wkT", [D, HPC * DK], BF16, kind="ExternalInput")
    wvT = nc.dram_tensor("wvT", [D, HPC * DK], BF16, kind="ExternalInput")
    # W_o.T in bf16, tiled (eb, p, dl, e'): row = eb*2048 + p*16 + dl
    woT = nc.dram_tensor("woT", [KB * D, DK], BF16, kind="ExternalInput")
    ropeC = nc.dram_tensor("ropeC", [DK, S], BF16, kind="ExternalInput")
    ropeS = nc.dram_tensor("ropeS", [DK, S], BF16, kind="ExternalInput")
    maskd = nc.dram_tensor("maskd", [JB, 4 * QCH], BF16, kind="ExternalInput")
    # out columns ordered (jc 0..7, t 0..63): global token 512*jc + 64*c + t
    outT = nc.dram_tensor("outT", [D, QCH], BF16, kind="ExternalOutput")

    swap_mask = [i ^ 1 for i in range(32)]

    import contextlib
    with tile.TileContext(nc) as tc:
        with contextlib.ExitStack() as st:
            dram = st.enter_context(
                tc.tile_pool(name="dram", bufs=1, space="DRAM"))
            a2a_in = [dram.tile([N_CORES * HPC * DK, TPC], BF16,
                                name=f"a2ain{j}") for j in range(NCH)]
            a2a_out = [dram.tile([N_CORES * HPC * DK, TPC], BF16,
                                 name=f"a2aout{j}") for j in range(NCH)]

            const = st.enter_context(tc.tile_pool(name="const", bufs=1))
            xpool = st.enter_context(
                tc.tile_pool(name="xpool", bufs=2, side="right"))
            qpool = st.enter_context(
                tc.tile_pool(name="qpool", bufs=2, side="right"))
            kvpool = st.enter_context(
                tc.tile_pool(name="kvpool", bufs=8, side="right"))
            vpool = st.enter_context(
                tc.tile_pool(name="vpool", bufs=15, side="right"))
            ppool = st.enter_context(tc.tile_pool(name="ppool", bufs=7))
            lpool = st.enter_context(tc.tile_pool(name="lpool", bufs=2))
            work = st.enter_context(tc.tile_pool(name="work", bufs=2))
            ypool = st.enter_context(tc.tile_pool(name="ypool", bufs=3))
            ytpool = st.enter_context(tc.tile_pool(name="ytpool", bufs=1))
            stpool = st.enter_context(tc.tile_pool(name="stpool", bufs=1))
            wopool = st.enter_context(
                tc.tile_pool(name="wopool", bufs=16, side="right"))
            ps_proj = st.enter_context(
                tc.tile_pool(name="ps_proj", bufs=3, space="PSUM"))
            ps_st = st.enter_context(
                tc.tile_pool(name="ps_st", bufs=3, space="PSUM"))
            ps_out = st.enter_context(
                tc.tile_pool(name="ps_out", bufs=2, space="PSUM"))

            # chunk-0 xT goes first on its queues so the first projection
            # isn't stuck behind weight DMAs
            def xt_dma(xt, sc):
                KH = KB // 4
                di = None
                for xh in range(4):
                    di = nc.sync.dma_start(
                        xt[:, xh * KH * SUB:(xh + 1) * KH * SUB]
                          .rearrange("p (kb t) -> p kb t", kb=KH),
                        xT.ap()[xh * KH * 128:(xh + 1) * KH * 128,
                                sc * SUB:(sc + 1) * SUB]
                          .rearrange("(kb p) t -> p kb t", p=128))
                return di

            xt_anchor = {}
            store_anchor = {}
            a2a_inst = {}
            xt0 = xpool.tile([128, KB * SUB], BF16, tag="xt", name="xt")
            KH0 = KB // 8
            for xh in range(8):
                di = nc.sync.dma_start(
                    xt0[:, xh * KH0 * SUB:(xh + 1) * KH0 * SUB]
                       .rearrange("p (kb t) -> p kb t", kb=KH0),
                    xT.ap()[xh * KH0 * 128:(xh + 1) * KH0 * 128, 0:SUB]
                      .rearrange("(kb p) t -> p kb t", p=128))
            xt_anchor[0] = di

            # ---- persistent constants in SBUF
            wq_sb = const.tile([128, KB * HPC * DK], BF16)
            wk_sb = const.tile([128, KB * HPC * DK], BF16)
            wv_sb = const.tile([128, KB * HPC * DK], BF16)
            weng = {0: nc.scalar, 1: nc.scalar, 2: nc.gpsimd}
            for ti, (sb_t, dr) in enumerate(
                    ((wq_sb, wqT), (wk_sb, wkT), (wv_sb, wvT))):
                ngrp = 8 if ti == 0 else 4
                GW = KB // ngrp
                for g in range(ngrp):
                    m0 = g * GW * HPC * DK
                    weng[ti].dma_start(
                        sb_t[:, m0:m0 + GW * HPC * DK]
                            .rearrange("p (kb m) -> p kb m", kb=GW),
                        dr.ap()[g * GW * 128:(g + 1) * GW * 128, :]
                          .rearrange("(kb p) m -> p kb m", p=128))
            ropeC_sb = const.tile([DK, S], BF16)
            ropeS_sb = const.tile([DK, S], BF16)
            maskd_sb = const.tile([JB, 4 * QCH], BF16)
            nc.scalar.dma_start(ropeC_sb[:, :SUB], ropeC[:, :SUB])
            nc.scalar.dma_start(ropeS_sb[:, :SUB], ropeS[:, :SUB])
            nc.scalar.dma_start(ropeC_sb[:, SUB:], ropeC[:, SUB:])
            nc.scalar.dma_start(ropeS_sb[:, SUB:], ropeS[:, SUB:])
            nc.scalar.dma_start(maskd_sb[:], maskd[:])

            def rope_combine(ps_in, out_ap, s0, n):
                """out = ps_in * C + shuffle(ps_in) * S  (RoPE)."""
                qsh = work.tile([128, SUB], F32, tag="qsh", name="qsh")
                t1 = work.tile([128, SUB], BF16, tag="t1", name="t1")
                nc.vector.stream_shuffle(qsh[:, :n], ps_in, swap_mask)
                nc.any.tensor_tensor(
                    t1[:, :n], ps_in, ropeC_sb[:, s0:s0 + n], AluOpType.mult)
                nc.vector.tensor_tensor(
                    qsh[:, :n], qsh[:, :n], ropeS_sb[:, s0:s0 + n],
                    AluOpType.mult)
                nc.any.tensor_tensor(out_ap, t1[:, :n], qsh[:, :n],
                                     AluOpType.add)

            chain_engs = [nc.vector, nc.gpsimd]

            # Wo token-passes -------------------------------------------------
            # pass 0: chunks 0-3 (cols 0:256), interleaved into the second
            # half of the main loop; pass 1: chunks 4-6 (cols 256:448);
            # pass 2: chunk 7 (cols 448:512), reusing resident wo tiles.
            wo_engs = [nc.sync, nc.scalar]

            def load_woeb(eb, ei, anchor=None):
                wo_eb = wopool.tile([128, KB * DK], BF16, tag="wo",
                                    name="wo_eb")
                di = wo_engs[ei % 2].dma_start(
                    wo_eb[:],
                    woT.ap()[eb * D:(eb + 1) * D, :]
                       .rearrange("(p dl) e -> p (dl e)", p=128))
                if anchor is not None:
                    add_dep_helper(di.ins, anchor.ins, sync=True,
                                   reason="wo load after main loop")
                return wo_eb

            def load_yt(yt, chunks, ci0=0, anchors=None, eng=None):
                for ci, jc in enumerate(chunks):
                    di = (eng or nc.scalar).dma_start(
                        yt.rearrange("p (db c) -> p db c", db=KB)
                          [:, :, (ci0 + ci) * TPC:(ci0 + ci + 1) * TPC],
                        a2a_out[jc].rearrange("(db p) t -> p db t",
                                                   p=128))
                    if anchors is not None:
                        add_dep_helper(di.ins, anchors[ci].ins, sync=True,
                                       reason="yt load after its A2A landed")

            def wo_eb_mm(eb, yt, ncols, stage, wo_eb=None, ei=0):
                if wo_eb is None:
                    wo_eb = load_woeb(eb, ei)
                ps_w = ps_proj.tile([128, SUB], F32, tag="proj", name="ps_w")
                for dl in range(KB):
                    nc.tensor.matmul(
                        ps_w[:, :ncols],
                        wo_eb[:, dl * DK:(dl + 1) * DK],
                        yt[:, dl * ncols:(dl + 1) * ncols],
                        start=(dl == 0), stop=(dl == KB - 1))
                nc.any.tensor_copy(
                    stage[:, eb * ncols:(eb + 1) * ncols], ps_w[:, :ncols])

            def store_stage(stage, ncols, col0):
                nc.sync.dma_start(
                    outT.ap().rearrange("(eb p) c -> p eb c", p=128)
                        [:, :, col0:col0 + ncols],
                    stage.rearrange("p (eb c) -> p eb c", eb=KB))

            # ---------------- main pass over the sequence -------------------
            qT_tiles = {}
            kT_tiles = {}
            v_tiles = {}
            yt_p0 = [None]

            for sc in range(NSUB):
                b = sc // (NSUB // B)
                iq = sc % (NSUB // B)
                jc_glob = sc
                s0 = iq * SUB                   # position within batch

                if sc == 0:
                    xt = xt0
                else:
                    xt = xpool.tile([128, KB * SUB], BF16, tag="xt", name="xt")
                    xt_anchor[sc] = xt_dma(xt, sc)

                # ---- q/k projections + rope, both heads
                for h in range(HPC):
                    qT_tiles[h] = qpool.tile([128, QCH], BF16,
                                             tag=f"qT{h}", name="qT")
                    kT_tiles[(b, h, iq)] = kvpool.tile(
                        [128, QCH], BF16, tag="kT", name="kT")
                    for (w_sb, dst) in ((wq_sb, qT_tiles[h]),
                                        (wk_sb, kT_tiles[(b, h, iq)])):
                        psq = ps_proj.tile([128, SUB], F32, tag="proj", name="proj")
                        for kb in range(KB):
                            nc.tensor.matmul(
                                psq[:],
                                w_sb[:, kb * HPC * DK + h * DK:
                                     kb * HPC * DK + (h + 1) * DK],
                                xt[:, kb * SUB:(kb + 1) * SUB],
                                start=(kb == 0), stop=(kb == KB - 1))
                        rope_combine(psq[:], dst[:], s0, SUB)

                # ---- v projection, both heads, bf16 storage
                for tb in range(SUB // 128):
                    jb_b = iq * (SUB // 128) + tb
                    psv = ps_proj.tile([128, HPC * DK], F32, tag="proj",
                                       name="psv")
                    for kb in range(KB):
                        nc.tensor.matmul(
                            psv[:],
                            xt[:, kb * SUB + tb * 128:
                               kb * SUB + (tb + 1) * 128],
                            wv_sb[:, kb * HPC * DK:(kb + 1) * HPC * DK],
                            start=(kb == 0), stop=(kb == KB - 1))
                    vt = vpool.tile([128, HPC * DK], BF16, tag="v", name="vt")
                    nc.any.tensor_copy(vt[:], psv[:])
                    v_tiles[(b, jb_b)] = vt

                # ---- attention for the completed query chunk
                if True:
                    n_j = 4 * iq + 4
                    for h in range(HPC):
                        qT = qT_tiles[h]
                        ps_o = ps_out.tile([128, QCH], F32, tag="att_out", name="att_out")
                        p_tiles = {}

                        def emit_block(j):
                            jck, jr = j // 4, j % 4
                            m = j - 4 * iq
                            # diagonal block m: columns [0, 128m) are fully
                            # masked -> skip them and zero-fill p instead
                            q0 = 128 * m if m > 0 else 0
                            ps_s = ps_st.tile([JB, QCH], F32, tag="st", name="st")
                            nc.tensor.matmul(
                                ps_s[:, q0:],
                                kT_tiles[(b, h, jck)][:, jr * 128:
                                                      (jr + 1) * 128],
                                qT[:, q0:], start=True, stop=True)
                            if m >= 0:           # diagonal block: mask
                                # full width: the un-written [0,q0) region
                                # holds bounded stale scores; -30000 sends
                                # them to exp()==0, zero-filling p for free
                                nc.vector.tensor_tensor(
                                    ps_s[:], ps_s[:],
                                    maskd_sb[:, m * QCH:(m + 1) * QCH],
                                    AluOpType.add)
                            p_t = ppool.tile([JB, QCH], BF16, tag="p", name="p")
                            nc.scalar.activation(
                                p_t[:], ps_s[:],
                                mybir.ActivationFunctionType.Exp)
                            p_tiles[j] = p_t

                        # scores/exp run two blocks ahead of the AV matmuls
                        # so PE never waits on the Act engine
                        emit_block(0)
                        if n_j > 1:
                            emit_block(1)
                        # l = sum_k p: two sequential accumulator chains
                        # (even/odd j) spread across DVE/Pool/Act
                        acc = [None, None]
                        n_add = 0
                        cengs = (chain_engs if jc_glob < NCH - 1
                                 else [nc.vector])
                        for j in range(n_j):
                            if j + 2 < n_j:
                                emit_block(j + 2)
                            p_t = p_tiles[j]
                            # diagonal blocks: columns [0,128m) of p are
                            # exact zeros -- skip them (j==0 is always
                            # full width, so the PSUM group opens whole)
                            m = j - 4 * iq
                            q0 = 128 * m if m > 0 else 0
                            nc.tensor.matmul(
                                ps_o[:, q0:], v_tiles[(b, j)][:, h * DK:
                                                              (h + 1) * DK],
                                p_t[:, q0:],
                                start=(j == 0), stop=(j == n_j - 1))
                            c = j % 2
                            if j >= 2:
                                eng = cengs[0]
                                n_add += 1
                                if acc[c] is None:
                                    a_t = lpool.tile([128, QCH], BF16,
                                                     tag="acc", name="acc")
                                    eng.tensor_tensor(
                                        a_t[:], p_tiles[c][:], p_t[:],
                                        AluOpType.add)
                                    acc[c] = a_t
                                else:
                                    eng.tensor_tensor(
                                        acc[c][:], acc[c][:], p_t[:],
                                        AluOpType.add)
                        if acc[0] is None:       # n_j == 4 has j = 0..3
                            acc = [p_tiles[0], p_tiles[1]]
                        p_acc = lpool.tile([128, QCH], BF16, tag="acc2", name="acc2")
                        cengs[0].tensor_tensor(
                            p_acc[:], acc[0][:], acc[1][:], AluOpType.add)
                        # gpsimd all-reduce output is broadcast across
                        # partitions -> no r-broadcast matmul needed
                        l_bc = lpool.tile([128, QCH], F32, tag="lbc", name="lbc")
                        nc.gpsimd.partition_all_reduce(
                            l_bc[:], p_acc[:], channels=128,
                            reduce_op=bass_isa.ReduceOp.add)
                        nc.vector.reciprocal(l_bc[:], l_bc[:])
                        y_sb = ypool.tile([128, QCH], BF16, tag="y", name="y")
                        nc.any.tensor_tensor(y_sb[:], ps_o[:], l_bc[:],
                                             AluOpType.mult)
                        # scatter into the AllToAll source layout:
                        # dst row m*256 + h*128 + p, col t <- src[p, m*64+t]
                        di = nc.gpsimd.dma_start(
                            a2a_in[jc_glob]
                            .rearrange("(m hp) t -> hp m t", m=N_CORES)
                            [h * 128:(h + 1) * 128],
                            y_sb.rearrange("p (m t) -> p m t", m=N_CORES))
                        if h == HPC - 1:
                            store_anchor[jc_glob] = di
                    a2a_inst[jc_glob] = nc.gpsimd.collective_compute(
                        "AllToAll", AluOpType.bypass,
                        replica_groups=[list(range(N_CORES))],
                        ins=[a2a_in[jc_glob].opt()],
                        outs=[a2a_out[jc_glob].opt()])


            # ---- Wo pass A: chunks 0-3 (cols 0:256).  Runs entirely in
            # the shadow of the last AllToAll: those chunks' y landed long
            # ago.  Every staging DMA is anchored so the dataflow
            # scheduler cannot hoist its wait into the main loop.
            yt_p0[0] = ytpool.tile([128, KB * 4 * TPC], BF16,
                                   tag="yt0", name="yt0")
            load_yt(yt_p0[0], range(4),
                    anchors=[store_anchor[c + 1] for c in range(4)])
            yt_p1 = ytpool.tile([128, KB * 3 * TPC], BF16, tag="yt1",
                                name="yt1")
            load_yt(yt_p1, range(4, 7),
                    anchors=[store_anchor[5], store_anchor[6],
                             store_anchor[7]])
            stage_a = stpool.tile([128, KB * 4 * TPC], BF16, tag="sta",
                                  name="sta")
            stage_b1 = stpool.tile([128, KB * 3 * TPC], BF16, tag="stb1",
                                   name="stb1")
            stage_b2 = stpool.tile([128, KB * TPC], BF16, tag="stb2",
                                   name="stb2")
            p0_pref = {eb: load_woeb(eb, eb, anchor=xt_anchor[5])
                       for eb in range(2)}
            for eb in range(KB):
                if eb + 2 < KB:       # stay 2 loads ahead of the matmuls
                    p0_pref[eb + 2] = load_woeb(eb + 2, eb,
                                                anchor=xt_anchor[5])
                wo_eb_mm(eb, yt_p0[0], 4 * TPC, stage_a,
                         wo_eb=p0_pref.pop(eb), ei=eb)

            store_stage(stage_a, 4 * TPC, 0)

            # ---- Wo pass B1: chunks 4-6 (cols 256:448) -- none of this
            # waits on the final AllToAll, so it fills the A2A-7 shadow
            # right after pass A.
            b1_tiles = {eb: load_woeb(eb, eb, anchor=xt_anchor[6])
                        for eb in range(2)}
            for eb in range(KB):
                if eb + 2 < KB:
                    b1_tiles[eb + 2] = load_woeb(eb + 2, eb,
                                                 anchor=xt_anchor[6])
                wo_eb_mm(eb, yt_p1, 3 * TPC, stage_b1,
                         wo_eb=b1_tiles[eb], ei=eb)

            store_stage(stage_b1, 3 * TPC, 4 * TPC)

            # ---- Wo pass B2: chunk 7 (cols 448:512), gated on the final
            # AllToAll.  The last wopool-1 tiles of B1 are still resident:
            # run those first, re-stream the rest.
            yt_p2 = ytpool.tile([128, KB * TPC], BF16, tag="yt2",
                                name="yt2")
            load_yt(yt_p2, range(7, 8),
                    anchors=[store_anchor[7]], eng=nc.gpsimd)
            for eb in range(KB - 13, KB):
                wo_eb_mm(eb, yt_p2, TPC, stage_b2, wo_eb=b1_tiles[eb])
            for ei, eb in enumerate(range(KB - 13)):
                wo_eb_mm(eb, yt_p2, TPC, stage_b2, ei=ei)
            # rows 3..15 (resident-weight blocks) finish first: store them
            # while the three re-streamed blocks compute, leaving only a
            # tiny final store on the critical path
            for q4 in range(4):
                nc.sync.dma_start(
                    outT.ap().rearrange("(eb p) c -> p eb c", p=128)
                        [:, q4 * 4:(q4 + 1) * 4, 7 * TPC:],
                    stage_b2.rearrange("p (eb c) -> p eb c", eb=KB)
                            [:, q4 * 4:(q4 + 1) * 4])

    nc.finalize()
    return nc


# ---------------------------------------------------------------- host
def _host_inputs(x, W_q, W_k, W_v, W_o):
    import ml_dtypes
    bf = np.dtype(ml_dtypes.bfloat16)
    xT = np.ascontiguousarray(
        x.reshape(TOK, D).T).astype(bf)                        # [D, TOK]

    # W_o.T tiled (eb, p, dl, e'): row eb*2048 + p*16 + dl, col e'
    woT = np.ascontiguousarray(
        W_o.T.reshape(KB, 128, KB, DK).transpose(2, 1, 0, 3)
        .reshape(KB * D, DK)).astype(bf)

    # RoPE tables, expanded to [DK, S] with interleaved pairs; the sign
    # table carries -sin on even rows, +sin on odd rows.
    i = np.arange(0, DK, 2, dtype=np.float32)
    theta = 1.0 / (ROPE_BASE ** (i / DK))                      # [64]
    pos = np.arange(S, dtype=np.float32)
    freqs = pos[:, None] * theta[None, :]                      # [S, 64]
    cos_t, sin_t = np.cos(freqs), np.sin(freqs)
    ropeC = np.empty((DK, S), np.float32)
    ropeS = np.empty((DK, S), np.float32)
    ropeC[0::2] = cos_t.T
    ropeC[1::2] = cos_t.T
    ropeS[0::2] = -sin_t.T
    ropeS[1::2] = sin_t.T

    # diagonal causal masks: block m (of the 4 key blocks overlapping a
    # 512-query chunk) keeps kk <= qq - 128*m
    kk = np.arange(JB)[:, None]
    qq = np.arange(QCH)[None, :]
    maskd = np.concatenate(
        [np.where(kk <= qq - 128 * m, 0.0, MASK_NEG).astype(np.float32)
         for m in range(4)], axis=1).astype(bf)                # [128, 4*512]

    scale = 1.0 / np.sqrt(np.float32(DK))
    in_maps = []
    for c in range(N_CORES):
        rows = slice(c * HPC * DK, (c + 1) * HPC * DK)
        in_maps.append({
            "xT": xT,
            "wqT": np.ascontiguousarray(
                (W_q[rows] * scale).T).astype(bf),
            "wkT": np.ascontiguousarray(W_k[rows].T).astype(bf),
            "wvT": np.ascontiguousarray(W_v[rows].T).astype(bf),
            "woT": woT,
            "ropeC": ropeC.astype(bf),
            "ropeS": ropeS.astype(bf),
            "maskd": maskd,
        })
    return in_maps


def kernel(x, W_q, W_k, W_v, W_o):
    x = np.asarray(x, dtype=np.float32)
    W_q = np.asarray(W_q, dtype=np.float32)
    W_k = np.asarray(W_k, dtype=np.float32)
    W_v = np.asarray(W_v, dtype=np.float32)
    W_o = np.asarray(W_o, dtype=np.float32)

    if "nc" not in _CACHE:
        _CACHE["nc"] = _build_nc()
    nc = _CACHE["nc"]

    in_maps = _host_inputs(x, W_q, W_k, W_v, W_o)
    res = bass_utils.run_bass_kernel_spmd(
        nc, in_maps, core_ids=list(range(N_CORES)))

    # outT per core: [D, 512] f32; col 64*jc + t -> token 512*jc + 64*c + t
    out_T = np.empty((D, TOK), np.float32)
    for c in range(N_CORES):
        cols = (QCH * np.arange(NCH)[:, None] + TPC * c
                + np.arange(TPC)[None, :]).ravel()
        out_T[:, cols] = res.results[c]["outT"].astype(np.float32)
    return np.ascontiguousarray(out_T.T).reshape(B, S, D).astype(np.float32)
